# revision 22
# baseline (speedup 1.0000x reference)
"""Trainium2 Bass kernel for nn_GAT_Vanilla (2-layer GAT + BN/ELU + MLP head).

Strategy (8 NeuronCores, graph/data parallel, dst-major edge layout):
- Nodes sorted by in-degree and chopped into 128-node bins (degree
  homogeneous); bins dealt round-robin to the 8 cores so every core gets
  an identical block-T profile. Edges are laid dst-major: partition lane
  = dst slot, free dim = (j-th incoming edge, feature).
- 3 SPMD launches: A) per-node matmuls producing h1/s1/d1/x_p^T;
  B) layer-1 edge phase (softmax-weighted aggregation) + layer-2 node
  matmul; C) layer-2 edge phase + residual + MLP head + log_softmax.
- Between launches the host routes device-computed per-node values into
  per-edge arrays (pure indexing/gather, no math) - the halo exchange.
- On device per block group: e=s+d, leaky (Pool); exp duplicated x2
  (Act, keeps DVE packed-2x mode); one DVE multiply v*=ex; PE
  transpose-accumulate (matmul vs identity) scatter-sums into PSUM
  producing agg^T feat-major; 1/z and BN-scale ride a tiny head-expand
  matmul; BN bias + ELU fused into Act ops. fp16 data, f32 accumulate.

Self-contained: only needs numpy + the concourse/bass stack.
"""

import numpy as np

import concourse.bass as bass
import concourse.bacc as bacc
import concourse.tile as tile
from concourse import mybir
from concourse.bass_utils import run_bass_kernel_spmd

F32 = mybir.dt.float32
F16 = mybir.dt.float16

# ---- problem constants (hardcoded per harness contract) ----
N, E, IN, HD, NH, OUT = 100000, 800000, 128, 32, 4, 40
D = HD * NH  # 128
EPS_BN = 1e-5
SPAD = -300.0  # pad-edge s value -> leaky -> -60 -> exp == 0

NCORES = 8
NBLK = 98   # blocks (bins) per core; 8*98*128 = 100352 slots >= N
GB = 4      # blocks per group (uniform T within a group)

PROFILE = False
LAST_EXEC_NS = []


class Cfg:
    def __init__(self, n=N, e=E, ncores=NCORES, nblk=NBLK, gb=GB):
        self.n, self.e, self.ncores, self.nblk, self.gb = n, e, ncores, nblk, gb
        self.slots = nblk * 128
        # set by host_prep:
        self.Tpad = None    # per-block T (uniform within each group)
        self.offs = None    # per-block tile offset, offs[nblk] = SumT
        self.SumT = None
        self.groups = None  # list of (g0, nb, T)


class Prep:
    pass


# ----------------------------------------------------------------------------
# Host preprocessing: degree-sorted binning + per-edge index matrices
# ----------------------------------------------------------------------------

def host_prep(edge_index, cfg: Cfg):
    n, e = cfg.n, cfg.e
    ncores, nblk, gb = cfg.ncores, cfg.nblk, cfg.gb
    nbins = ncores * nblk
    nslots = nbins * 128
    src = np.concatenate([edge_index[0], np.arange(n)]).astype(np.int64)
    dst = np.concatenate([edge_index[1], np.arange(n)]).astype(np.int64)
    deg = np.bincount(dst, minlength=n)

    # degree-sorted consecutive bins of 128 nodes; bin k -> core k%ncores,
    # block position k//ncores, so all cores share one T profile.
    order = np.argsort(-deg, kind="stable")
    binrank = np.arange(n) // 128          # bin of i-th sorted node
    lane = np.arange(n) % 128
    core_of = np.empty(n, np.int64)
    gpos_of = np.empty(n, np.int64)
    lane_of = np.empty(n, np.int64)
    core_of[order] = binrank % ncores
    gpos_of[order] = binrank // ncores
    lane_of[order] = lane
    slot_of = core_of * (nblk * 128) + gpos_of * 128 + lane_of

    # per-bin max degree -> per-block-position profile -> group-pad
    degs_sorted = deg[order]
    nb_used = (n + 127) // 128
    Tbin = np.ones(nbins, np.int64)
    maxs = np.maximum.reduceat(degs_sorted, np.arange(0, n, 128))
    Tbin[:nb_used] = np.maximum(maxs, 1)
    Tprof = np.ones(nblk, np.int64)
    for g in range(nblk):
        lo, hi = g * ncores, min((g + 1) * ncores, nbins)
        Tprof[g] = max(1, Tbin[lo:hi].max())
    Tpad = Tprof.copy()
    groups = []
    g = 0
    while g < nblk:
        nb = min(gb, nblk - g)
        T = int(Tprof[g:g + nb].max())
        Tpad[g:g + nb] = T
        groups.append((g, nb, T))
        g += nb
    offs = np.zeros(nblk + 1, np.int64)
    np.cumsum(Tpad, out=offs[1:])
    SumT = int(offs[nblk])
    cfg.Tpad, cfg.offs, cfg.SumT, cfg.groups = Tpad, offs, SumT, groups

    # edge -> (core, column, lane) in dst-major layout
    dslot = slot_of[dst]
    eorder = np.argsort(dslot, kind="stable")
    src_s, dslot_s = src[eorder], dslot[eorder]
    # j = rank within dst
    uniq, starts_idx = np.unique(dslot_s, return_index=True)
    j_of = np.arange(e + n, dtype=np.int64)
    j_of -= np.repeat(starts_idx, np.diff(np.append(starts_idx, e + n)))
    ecore = dslot_s // (nblk * 128)
    egpos = (dslot_s // 128) % nblk
    elane = dslot_s % 128
    ecol = offs[egpos] + j_of

    # per-core index matrices [128, SumT]:
    #   Iv: row into h_ext (nslots -> zero row)
    #   Is: row into s_ext (nslots -> SPAD row, nslots+1 -> zero row)
    prep = Prep()
    prep.Iv, prep.Is = [], []
    for c in range(ncores):
        iv = np.full((128, SumT), nslots, np.int64)
        is_ = np.full((128, SumT), nslots, np.int64)
        m = ecore == c
        iv[elane[m], ecol[m]] = slot_of[src_s[m]]
        is_[elane[m], ecol[m]] = slot_of[src_s[m]]
        prep.Iv.append(iv)
        prep.Is.append(is_)
    # dummy slots (no real node): one self edge with v=0, s=0 at j=0
    have = np.zeros(nslots, bool)
    have[slot_of] = True
    dumm = np.nonzero(~have)[0]
    dc = dumm // (nblk * 128)
    dg = (dumm // 128) % nblk
    dl = dumm % 128
    for c in range(ncores):
        m = dc == c
        prep.Is[c][dl[m], offs[dg[m]]] = nslots + 1  # s = 0
        # Iv stays nslots (v = 0)

    # own-node slot grid for d routing [128, nblk]
    g_grid = np.broadcast_to(np.arange(nblk)[None, :], (128, nblk))
    l_grid = np.broadcast_to(np.arange(128)[:, None], (128, nblk))
    prep.dgrid = [c * nblk * 128 + g_grid * 128 + l_grid for c in range(ncores)]

    prep.slot_of = slot_of
    prep.cfg = cfg
    return prep


def route_edge_arrays(prep, cfg, h_all16, s_all16, d_all16):
    """h_all16 [nslots,128] f16, s_all16 [nslots,4] f16, d_all16 [nslots,4].
    Returns per-core (v [128,SumT,128], s [128,SumT,4], d [128,nblk,4])."""
    nslots = cfg.ncores * cfg.slots
    h_ext = np.vstack([h_all16, np.zeros((1, 128), np.float16)])
    s_ext = np.vstack([s_all16,
                       np.full((1, 4), SPAD, np.float16),
                       np.zeros((1, 4), np.float16)])
    out = []
    for c in range(cfg.ncores):
        v = h_ext[prep.Iv[c]]
        s = s_ext[prep.Is[c]]
        d = d_all16[prep.dgrid[c]]
        out.append((v, s, d))
    return out


# ----------------------------------------------------------------------------
# Device kernels
# ----------------------------------------------------------------------------

def build_launch_a(cfg: Cfg):
    """xpT = res_W^T x^T + res_b ; [h1|s1|d1] = x @ (res_W@W1cat) + res_b@W1cat."""
    nc = bacc.Bacc("TRN2", target_bir_lowering=False, debug=False,
                   num_devices=cfg.ncores)
    nblk = cfg.nblk
    xT = nc.dram_tensor("xT", [128, cfg.slots], F16, kind="ExternalInput").ap()
    rw = nc.dram_tensor("rw", [128, 128], F16, kind="ExternalInput").ap()
    rb = nc.dram_tensor("rb", [128, 1], F32, kind="ExternalInput").ap()
    wfh = nc.dram_tensor("wfh", [128, 128], F16, kind="ExternalInput").ap()
    wfsd = nc.dram_tensor("wfsd", [128, 8], F16, kind="ExternalInput").ap()
    ones1 = nc.dram_tensor("ones1", [1, 128], F16, kind="ExternalInput").ap()
    bbh = nc.dram_tensor("bbh", [1, 128], F16, kind="ExternalInput").ap()
    bbsd = nc.dram_tensor("bbsd", [1, 8], F16, kind="ExternalInput").ap()
    h1 = nc.dram_tensor("h1", [128, nblk, 128], F16, kind="ExternalOutput").ap()
    sd1 = nc.dram_tensor("sd1", [128, nblk, 8], F16, kind="ExternalOutput").ap()
    xpT = nc.dram_tensor("xpT", [128, nblk, 128], F16,
                         kind="ExternalOutput").ap()

    with tile.TileContext(nc) as tc:
        with (
            tc.tile_pool(name="const", bufs=1) as cp,
            tc.tile_pool(name="io", bufs=3) as iop,
            tc.tile_pool(name="psx", bufs=2, space="PSUM") as psx_p,
            tc.tile_pool(name="psh", bufs=2, space="PSUM") as psh_p,
            tc.tile_pool(name="pss", bufs=2, space="PSUM") as pss_p,
        ):
            rw_t = cp.tile([128, 128], F16)
            nc.sync.dma_start(out=rw_t[:], in_=rw)
            rb_t = cp.tile([128, 1], F32)
            nc.sync.dma_start(out=rb_t[:], in_=rb)
            wfh_t = cp.tile([128, 128], F16)
            nc.sync.dma_start(out=wfh_t[:], in_=wfh)
            wfsd_t = cp.tile([128, 8], F16)
            nc.sync.dma_start(out=wfsd_t[:], in_=wfsd)
            on_t = cp.tile([1, 128], F16)
            nc.sync.dma_start(out=on_t[:], in_=ones1)
            bbh_t = cp.tile([1, 128], F16)
            nc.sync.dma_start(out=bbh_t[:], in_=bbh)
            bbsd_t = cp.tile([1, 8], F16)
            nc.sync.dma_start(out=bbsd_t[:], in_=bbsd)

            for (g0, nb, _T) in cfg.groups:
                xt = iop.tile([128, GB, 128], F16, tag="xt")
                nc.sync.dma_start(
                    out=xt[:, 0:nb, :],
                    in_=xT[:, g0 * 128:(g0 + nb) * 128].rearrange(
                        "p (b q) -> p b q", b=nb))
                psx = psx_p.tile([128, GB, 128], F32, tag="x")
                psh = psh_p.tile([128, GB, 128], F32, tag="h")
                pss = pss_p.tile([128, GB, 8], F32, tag="s")
                for b in range(nb):
                    nc.tensor.matmul(out=psx[:, b, :], lhsT=rw_t[:],
                                     rhs=xt[:, b, :], start=True, stop=True)
                    nc.tensor.matmul(out=psh[:, b, :], lhsT=xt[:, b, :],
                                     rhs=wfh_t[:], start=True, stop=False)
                    nc.tensor.matmul(out=psh[:, b, :], lhsT=on_t[:],
                                     rhs=bbh_t[:], start=False, stop=True)
                    nc.tensor.matmul(out=pss[:, b, :], lhsT=xt[:, b, :],
                                     rhs=wfsd_t[:], start=True, stop=False)
                    nc.tensor.matmul(out=pss[:, b, :], lhsT=on_t[:],
                                     rhs=bbsd_t[:], start=False, stop=True)
                xpc = iop.tile([128, GB, 128], F16, tag="xpc")
                nc.scalar.activation(out=xpc[:, 0:nb, :], in_=psx[:, 0:nb, :],
                                     func=mybir.ActivationFunctionType.Identity,
                                     bias=rb_t[:])
                h1c = iop.tile([128, GB, 128], F16, tag="h1c")
                nc.scalar.copy(h1c[:, 0:nb, :], psh[:, 0:nb, :])
                sdc = iop.tile([128, GB, 8], F16, tag="sdc")
                nc.vector.tensor_copy(sdc[:, 0:nb, :], pss[:, 0:nb, :])
                nc.sync.dma_start(out=xpT[:, g0:g0 + nb, :], in_=xpc[:, 0:nb, :])
                nc.scalar.dma_start(out=h1[:, g0:g0 + nb, :], in_=h1c[:, 0:nb, :])
                nc.gpsimd.dma_start(out=sd1[:, g0:g0 + nb, :], in_=sdc[:, 0:nb, :])
    nc.compile()
    return nc


def build_launch_edge(cfg: Cfg, final: bool):
    """Edge phase (softmax-weighted aggregation, BN+ELU) for one layer.
    final=False: + layer-2 node matmul (h2/sd2 out).
    final=True:  + residual + MLP head + log_softmax (fin out)."""
    nc = bacc.Bacc("TRN2", target_bir_lowering=False, debug=False,
                   num_devices=cfg.ncores)
    nblk, SumT = cfg.nblk, cfg.SumT
    Tmax = cfg.groups[0][2]
    GT = GB * Tmax

    v_ap = nc.dram_tensor("v", [128, SumT, 128], F16, kind="ExternalInput").ap()
    s_ap = nc.dram_tensor("s", [128, SumT, 4], F16, kind="ExternalInput").ap()
    d_ap = nc.dram_tensor("d", [128, nblk, 4], F16, kind="ExternalInput").ap()
    kT = nc.dram_tensor("kT", [128, 1], F32, kind="ExternalInput").ap()
    cT = nc.dram_tensor("cT", [128, 1], F32, kind="ExternalInput").ap()
    idf16 = nc.dram_tensor("idf16", [128, 128], F16, kind="ExternalInput").ap()
    if not final:
        w2h = nc.dram_tensor("w2h", [128, 128], F16, kind="ExternalInput").ap()
        w2sd = nc.dram_tensor("w2sd", [128, 8], F16, kind="ExternalInput").ap()
        h2 = nc.dram_tensor("h2", [128, nblk, 128], F16,
                            kind="ExternalOutput").ap()
        sd2 = nc.dram_tensor("sd2", [128, nblk, 8], F16,
                             kind="ExternalOutput").ap()
    else:
        xpT = nc.dram_tensor("xpT", [128, nblk, 128], F16,
                             kind="ExternalInput").ap()
        wc1 = nc.dram_tensor("wc1", [128, 64], F16, kind="ExternalInput").ap()
        ccT = nc.dram_tensor("ccT", [64, 1], F32, kind="ExternalInput").ap()
        wc2a = nc.dram_tensor("wc2a", [65, 40], F16, kind="ExternalInput").ap()
        fin = nc.dram_tensor("fin", [128, nblk, 40], F32,
                             kind="ExternalOutput").ap()

    EXP = mybir.ActivationFunctionType.Exp
    RELU = mybir.ActivationFunctionType.Relu
    with tile.TileContext(nc) as tc:
        with (
            tc.tile_pool(name="const", bufs=1) as cp,
            tc.tile_pool(name="vp", bufs=2) as vp,
            tc.tile_pool(name="sp", bufs=2) as sp,
            tc.tile_pool(name="up", bufs=2) as up,
            tc.tile_pool(name="psagg", bufs=2, space="PSUM") as psagg_p,
            tc.tile_pool(name="psh", bufs=2, space="PSUM") as psh_p,
            tc.tile_pool(name="pssd", bufs=2, space="PSUM") as pssd_p,
            tc.tile_pool(name="psprt", bufs=2, space="PSUM") as psprt_p,
        ):
            kT_t = cp.tile([128, 1], F32)
            nc.sync.dma_start(out=kT_t[:], in_=kT)
            cT_t = cp.tile([128, 1], F32)
            nc.sync.dma_start(out=cT_t[:], in_=cT)
            idf16_t = cp.tile([128, 128], F16)
            nc.sync.dma_start(out=idf16_t[:], in_=idf16)
            if not final:
                w2h_t = cp.tile([128, 128], F16)
                nc.sync.dma_start(out=w2h_t[:], in_=w2h)
                w2sd_t = cp.tile([128, 8], F16)
                nc.sync.dma_start(out=w2sd_t[:], in_=w2sd)
            else:
                wc1_t = cp.tile([128, 64], F16)
                nc.sync.dma_start(out=wc1_t[:], in_=wc1)
                ccT_t = cp.tile([64, 1], F32)
                nc.sync.dma_start(out=ccT_t[:], in_=ccT)
                wc2a_t = cp.tile([65, 40], F16)
                nc.sync.dma_start(out=wc2a_t[:], in_=wc2a)
                ysb = cp.tile([128, nblk, 40], F32)

            for (g0, nb, T) in cfg.groups:
                off = int(cfg.offs[g0])
                nt = nb * T
                vt = vp.tile([128, GT, 128], F16, tag="v")
                nc.sync.dma_start(out=vt[:, 0:nt, :],
                                  in_=v_ap[:, off:off + nt, :])
                st = sp.tile([128, GT, 4], F16, tag="s")
                nc.sync.dma_start(out=st[:, 0:nt, :],
                                  in_=s_ap[:, off:off + nt, :])
                dt_ = sp.tile([128, GB, 4], F16, tag="d")
                nc.scalar.dma_start(out=dt_[:, 0:nb, :],
                                    in_=d_ap[:, g0:g0 + nb, :])

                # e = leaky(s + d)  [p, (b j) h] fp16
                et = sp.tile([128, GT, 4], F16, tag="e")
                e_bjh = et[:, 0:nt, :].rearrange("p (b j) h -> p b j h", b=nb)
                d_b = dt_[:, 0:nb, :].unsqueeze(2).to_broadcast(
                    [128, nb, T, 4])
                s_bjh = st[:, 0:nt, :].rearrange("p (b j) h -> p b j h", b=nb)
                nc.gpsimd.tensor_tensor(out=e_bjh, in0=s_bjh, in1=d_b,
                                        op=mybir.AluOpType.add)
                e_flat = et[:, 0:nt, :].rearrange("p a h -> p (a h)")
                nc.vector.scalar_tensor_tensor(
                    out=e_flat, in0=e_flat, scalar=0.2, in1=e_flat,
                    op0=mybir.AluOpType.mult, op1=mybir.AluOpType.max)

                # ex2 = exp(e) duplicated x2 (Act); ez = exp(e) (Act)
                ex2t = sp.tile([128, GT * 8], F16, tag="ex2")
                ex2_v = ex2t[:, 0:nt * 8].rearrange("p (a two) -> p a two",
                                                    two=2)
                e_b2 = e_flat.unsqueeze(-1).to_broadcast([128, nt * 4, 2])
                nc.scalar.activation(out=ex2_v, in_=e_b2, func=EXP)
                ezt = sp.tile([128, GT * 4], F16, tag="ez")
                nc.scalar.activation(out=ezt[:, 0:nt * 4], in_=e_flat,
                                     func=EXP)
                # z[p, b, h] = sum_j ez
                zt = sp.tile([128, GB, 4], F32, tag="z")
                ez_bhj = ezt[:, 0:nt * 4].rearrange(
                    "p (b j h) -> p b h j", b=nb, j=T, h=4)
                nc.vector.tensor_reduce(out=zt[:, 0:nb, :], in_=ez_bhj,
                                        axis=mybir.AxisListType.X,
                                        op=mybir.AluOpType.add)
                zrt = sp.tile([128, GB * 4], F32, tag="zr")
                nc.vector.reciprocal(
                    zrt[:, 0:nb * 4],
                    zt[:, 0:nb, :].rearrange("p b h -> p (b h)"))
                # alpha = ex / z  (per block; zr broadcast over j and pair)
                for b in range(nb):
                    ex2_blk = ex2t[:, b * T * 8:(b + 1) * T * 8].rearrange(
                        "p (j h two) -> p j h two", j=T, h=4, two=2)
                    zr_b = zrt[:, b * 4:(b + 1) * 4].unsqueeze(1)\
                        .unsqueeze(-1).to_broadcast([128, T, 4, 2])
                    nc.vector.tensor_tensor(out=ex2_blk, in0=ex2_blk,
                                            in1=zr_b,
                                            op=mybir.AluOpType.mult)

                # w = v * alpha (DVE packed 2x via pair-duplicated alpha)
                ex2_b = ex2t[:, 0:nt * 8].rearrange(
                    "p (a two) -> p a two", two=2).unsqueeze(2).to_broadcast(
                    [128, nt * 4, 16, 2])
                w_m = vt[:, 0:nt, :].rearrange(
                    "p bj (h c16 two) -> p (bj h) c16 two", h=4, c16=16,
                    two=2)
                nc.vector.tensor_tensor(out=w_m, in0=w_m, in1=ex2_b,
                                        op=mybir.AluOpType.mult)

                # aggT[feat, b, dst] += w_j^T  (PE transpose-accumulate)
                psagg = psagg_p.tile([128, GB, 128], F32, tag="agg")
                for b in range(nb):
                    for j in range(T):
                        nc.tensor.matmul(out=psagg[:, b, :],
                                         lhsT=vt[:, b * T + j, :],
                                         rhs=idf16_t[:],
                                         start=(j == 0), stop=(j == T - 1))
                # BN+ELU: g=exp(k*agg+c), t1=relu(k*agg+c), u=min(g-1,t1)
                gt = up.tile([128, GB, 128], F16, tag="g")
                nc.scalar.activation(out=gt[:, 0:nb, :],
                                     in_=psagg[:, 0:nb, :],
                                     func=EXP, bias=cT_t[:], scale=kT_t[:])
                t1t = up.tile([128, GB, 128], F16, tag="t1")
                nc.scalar.activation(out=t1t[:, 0:nb, :],
                                     in_=psagg[:, 0:nb, :],
                                     func=RELU, bias=cT_t[:], scale=kT_t[:])
                ut = up.tile([128, GB, 128], F16, tag="u")
                nc.vector.scalar_tensor_tensor(
                    out=ut[:, 0:nb, :], in0=gt[:, 0:nb, :], scalar=-1.0,
                    in1=t1t[:, 0:nb, :],
                    op0=mybir.AluOpType.add, op1=mybir.AluOpType.min)

                if not final:
                    psh = psh_p.tile([128, GB, 128], F32, tag="h2")
                    pss = pssd_p.tile([128, GB, 8], F32, tag="sd2")
                    for b in range(nb):
                        nc.tensor.matmul(out=psh[:, b, :], lhsT=ut[:, b, :],
                                         rhs=w2h_t[:], start=True, stop=True)
                        nc.tensor.matmul(out=pss[:, b, :], lhsT=ut[:, b, :],
                                         rhs=w2sd_t[:], start=True, stop=True)
                    h2c = up.tile([128, GB, 128], F16, tag="h2c")
                    nc.scalar.copy(h2c[:, 0:nb, :], psh[:, 0:nb, :])
                    sdc = up.tile([128, GB, 8], F16, tag="sdc")
                    nc.vector.tensor_copy(sdc[:, 0:nb, :], pss[:, 0:nb, :])
                    nc.scalar.dma_start(out=h2[:, g0:g0 + nb, :],
                                        in_=h2c[:, 0:nb, :])
                    nc.gpsimd.dma_start(out=sd2[:, g0:g0 + nb, :],
                                        in_=sdc[:, 0:nb, :])
                else:
                    xpt = up.tile([128, GB, 128], F16, tag="xpt")
                    nc.scalar.dma_start(out=xpt[:, 0:nb, :],
                                        in_=xpT[:, g0:g0 + nb, :])
                    nc.vector.tensor_tensor(out=ut[:, 0:nb, :],
                                            in0=ut[:, 0:nb, :],
                                            in1=xpt[:, 0:nb, :],
                                            op=mybir.AluOpType.add)
                    # MLP head: pr = h @ Wc1k ; r1 = relu(pr + cc) ; y = r1a @ Wc2a
                    pspr = psh_p.tile([128, GB, 64], F32, tag="pr")
                    for b in range(nb):
                        nc.tensor.matmul(out=pspr[:, b, :], lhsT=ut[:, b, :],
                                         rhs=wc1_t[:], start=True, stop=True)
                    prsb = up.tile([128, GB, 64], F16, tag="prsb")
                    nc.vector.tensor_copy(prsb[:, 0:nb, :], pspr[:, 0:nb, :])
                    psprT = psprt_p.tile([128, GB, 128], F32, tag="prT")
                    for b in range(nb):
                        nc.tensor.matmul(out=psprT[0:64, b, :],
                                         lhsT=prsb[:, b, :],
                                         rhs=idf16_t[:],
                                         start=True, stop=True)
                    r1sb = up.tile([65, GB, 128], F16, tag="r1")
                    nc.scalar.activation(out=r1sb[0:64, 0:nb, :],
                                         in_=psprT[0:64, 0:nb, :],
                                         func=RELU, bias=ccT_t[:])
                    nc.vector.memset(r1sb[64:65, :, :], 1.0)
                    psy = pssd_p.tile([128, GB, 40], F32, tag="y")
                    for b in range(nb):
                        nc.tensor.matmul(out=psy[:, b, :],
                                         lhsT=r1sb[:, b, :],
                                         rhs=wc2a_t[:], start=True, stop=True)
                    nc.scalar.copy(ysb[:, g0:g0 + nb, :], psy[:, 0:nb, :])

            if final:
                # log_softmax over the class dim, batched across all blocks
                eyt = cp.tile([128, nblk * 40], F16)
                nc.scalar.activation(
                    out=eyt[:], in_=ysb[:].rearrange("p a c -> p (a c)"),
                    func=EXP)
                zs = cp.tile([128, nblk], F32)
                nc.vector.tensor_reduce(
                    out=zs[:], in_=eyt[:].rearrange("p (a c) -> p a c", c=40),
                    axis=mybir.AxisListType.X, op=mybir.AluOpType.add)
                lnz = cp.tile([128, nblk], F32)
                nc.scalar.activation(out=lnz[:], in_=zs[:],
                                     func=mybir.ActivationFunctionType.Ln)
                finsb = cp.tile([128, nblk, 40], F32)
                lnz_b = lnz[:].unsqueeze(-1).to_broadcast([128, nblk, 40])
                nc.vector.tensor_tensor(out=finsb[:], in0=ysb[:], in1=lnz_b,
                                        op=mybir.AluOpType.subtract)
                nc.sync.dma_start(out=fin, in_=finsb[:])
    nc.compile()
    return nc


# ----------------------------------------------------------------------------
# Host orchestration
# ----------------------------------------------------------------------------

_cache = {}


def _get(key, fn):
    if key not in _cache:
        _cache[key] = fn()
    return _cache[key]


def _amat(a):
    m = np.zeros((D, NH), np.float32)
    for h in range(NH):
        m[h * HD:(h + 1) * HD, h] = a[h]
    return m


def _run(nc, in_maps, cfg, tag):
    res = run_bass_kernel_spmd(nc, in_maps, list(range(cfg.ncores)),
                               trace=PROFILE)
    if PROFILE:
        LAST_EXEC_NS.append((tag, res.exec_time_ns))
    return res.results


def _slotify(arr, cdim):
    """[128, nblk, c] device layout -> [slots, c] (slot = g*128 + lane)."""
    return arr.transpose(1, 0, 2).reshape(-1, cdim)


def kernel(x, edge_index, res_W, res_b,
           W1, as1, ad1, b1, g1, be1, rm1, rv1,
           W2, as2, ad2, b2, g2, be2, rm2, rv2,
           Wc1, bc1, gc, bec, rmc, rvc, Wc2, bc2,
           _cfg=None):
    cfg = _cfg or _get("cfg", lambda: Cfg())
    x = np.asarray(x, np.float32)
    edge_index = np.asarray(edge_index)

    ekey = ("prep", hash(edge_index.tobytes()))
    prep = _get(ekey, lambda: host_prep(np.asarray(edge_index, np.int64), cfg))
    nslots = cfg.ncores * cfg.slots

    def fold_bn(g_, be_, rm_, rv_, bias):
        k = (g_ / np.sqrt(rv_ + EPS_BN)).astype(np.float32)
        c = ((bias - rm_) * k + be_).astype(np.float32)
        return k, c

    k1, c1 = fold_bn(g1, be1, rm1, rv1, b1)
    k2, c2 = fold_bn(g2, be2, rm2, rv2, b2)
    kc, cc = fold_bn(gc, bec, rmc, rvc, bc1)

    W1cat = np.concatenate(
        [W1, W1 @ _amat(as1), W1 @ _amat(ad1)], axis=1).astype(np.float32)
    W2cat = np.concatenate(
        [W2, W2 @ _amat(as2), W2 @ _amat(ad2)], axis=1).astype(np.float32)
    Wfold = (res_W.astype(np.float32) @ W1cat)
    bb = (res_b.astype(np.float32) @ W1cat)

    ident16 = np.eye(128, dtype=np.float16)

    # ---- launch A ----
    x_sl = np.zeros((nslots, IN), np.float32)
    x_sl[prep.slot_of] = x
    nc_a = _get(("A",), lambda: build_launch_a(cfg))
    in_a = []
    for c in range(cfg.ncores):
        xs = x_sl[c * cfg.slots:(c + 1) * cfg.slots]
        in_a.append(dict(
            xT=np.ascontiguousarray(xs.T).astype(np.float16),
            rw=res_W.astype(np.float16),
            rb=np.asarray(res_b, np.float32).reshape(128, 1),
            wfh=Wfold[:, 0:128].astype(np.float16),
            wfsd=Wfold[:, 128:136].astype(np.float16),
            ones1=np.ones((1, 128), np.float16),
            bbh=bb[0:128].reshape(1, 128).astype(np.float16),
            bbsd=bb[128:136].reshape(1, 8).astype(np.float16)))
    res_a = _run(nc_a, in_a, cfg, "A")

    h1_all = np.concatenate(
        [_slotify(res_a[c]["h1"], 128) for c in range(cfg.ncores)])
    sd1_all = np.concatenate(
        [_slotify(res_a[c]["sd1"], 8) for c in range(cfg.ncores)])
    xp_dev = [res_a[c]["xpT"] for c in range(cfg.ncores)]

    # ---- launch B ----
    vsd1 = route_edge_arrays(prep, cfg, h1_all, sd1_all[:, 0:4],
                             sd1_all[:, 4:8])
    key_e = (cfg.SumT, tuple(g[2] for g in cfg.groups))
    nc_b = _get(("B", key_e), lambda: build_launch_edge(cfg, final=False))
    in_b = []
    for c in range(cfg.ncores):
        v, s, d = vsd1[c]
        in_b.append(dict(
            v=v, s=s, d=d, kT=k1.reshape(128, 1),
            cT=c1.reshape(128, 1), idf16=ident16,
            w2h=W2cat[:, 0:128].astype(np.float16),
            w2sd=W2cat[:, 128:136].astype(np.float16)))
    res_b_ = _run(nc_b, in_b, cfg, "B")

    h2_all = np.concatenate(
        [_slotify(res_b_[c]["h2"], 128) for c in range(cfg.ncores)])
    sd2_all = np.concatenate(
        [_slotify(res_b_[c]["sd2"], 8) for c in range(cfg.ncores)])

    # ---- launch C ----
    vsd2 = route_edge_arrays(prep, cfg, h2_all, sd2_all[:, 0:4],
                             sd2_all[:, 4:8])
    nc_c = _get(("C", key_e), lambda: build_launch_edge(cfg, final=True))
    Wc1k = (Wc1.astype(np.float32) * kc[None, :]).astype(np.float16)
    Wc2a = np.vstack([np.asarray(Wc2, np.float32),
                      np.asarray(bc2, np.float32).reshape(1, OUT)]
                     ).astype(np.float16)
    in_c = []
    for c in range(cfg.ncores):
        v, s, d = vsd2[c]
        in_c.append(dict(
            v=v, s=s, d=d, kT=k2.reshape(128, 1),
            cT=c2.reshape(128, 1), idf16=ident16,
            xpT=xp_dev[c], wc1=Wc1k, ccT=cc.reshape(64, 1),
            wc2a=Wc2a))
    res_c = _run(nc_c, in_c, cfg, "C")

    fin_slots = np.concatenate(
        [_slotify(res_c[c]["fin"], 40) for c in range(cfg.ncores)])
    return np.ascontiguousarray(fin_slots[prep.slot_of]).astype(np.float32)


# revision 60
# speedup vs baseline: 1.2790x; 1.2790x over previous
"""Trainium2 Bass kernel for nn_GAT_Vanilla (2-layer GAT + BN/ELU + MLP head).

Strategy (8 NeuronCores, graph/data parallel, dst-major edge layout):
- Nodes sorted by in-degree and chopped into 128-node bins (degree
  homogeneous); bins dealt round-robin to the 8 cores so every core gets
  an identical block-T profile. Edges are laid dst-major: partition lane
  = dst slot, free dim = (j-th incoming edge, feature).
- 3 SPMD launches: A) per-node matmuls producing h1/s1/d1/x_p^T;
  B) layer-1 edge phase (softmax-weighted aggregation) + layer-2 node
  matmul; C) layer-2 edge phase + residual + MLP head + log_softmax.
- Between launches the host routes device-computed per-node values into
  per-edge arrays (pure indexing/gather, no math) - the halo exchange.
- On device per block group: e=s+d, leaky (Pool); exp duplicated x2
  (Act, keeps DVE packed-2x mode); one DVE multiply v*=ex; PE
  transpose-accumulate (matmul vs identity) scatter-sums into PSUM
  producing agg^T feat-major; 1/z and BN-scale ride a tiny head-expand
  matmul; BN bias + ELU fused into Act ops. fp16 data, f32 accumulate.

Self-contained: only needs numpy + the concourse/bass stack.
"""

import numpy as np

import concourse.bass as bass
import concourse.bacc as bacc
import concourse.tile as tile
from concourse import mybir
from concourse.bass_utils import run_bass_kernel_spmd

F32 = mybir.dt.float32
F16 = mybir.dt.float16

# ---- problem constants (hardcoded per harness contract) ----
N, E, IN, HD, NH, OUT = 100000, 800000, 128, 32, 4, 40
D = HD * NH  # 128
EPS_BN = 1e-5
SPAD = -300.0  # pad-edge s value -> leaky -> -60 -> exp == 0

NCORES = 8
NBLK = 98   # blocks (bins) per core; 8*98*128 = 100352 slots >= N
GB = 4      # blocks per group (uniform T within a group)

PROFILE = False
LAST_EXEC_NS = []


class Cfg:
    def __init__(self, n=N, e=E, ncores=NCORES, nblk=NBLK, gb=GB):
        self.n, self.e, self.ncores, self.nblk, self.gb = n, e, ncores, nblk, gb
        self.slots = nblk * 128
        # set by host_prep:
        self.Tpad = None    # per-block T (uniform within each group)
        self.offs = None    # per-block tile offset, offs[nblk] = SumT
        self.SumT = None
        self.groups = None  # list of (g0, nb, T)


class Prep:
    pass


# ----------------------------------------------------------------------------
# Host preprocessing: degree-sorted binning + per-edge index matrices
# ----------------------------------------------------------------------------

def host_prep(edge_index, cfg: Cfg):
    n, e = cfg.n, cfg.e
    ncores, nblk, gb = cfg.ncores, cfg.nblk, cfg.gb
    nbins = ncores * nblk
    nslots = nbins * 128
    src = np.concatenate([edge_index[0], np.arange(n)]).astype(np.int64)
    dst = np.concatenate([edge_index[1], np.arange(n)]).astype(np.int64)
    deg = np.bincount(dst, minlength=n)

    # degree-sorted consecutive bins of 128 nodes; bin k -> core k%ncores,
    # block position k//ncores, so all cores share one T profile.
    order = np.argsort(-deg, kind="stable")
    binrank = np.arange(n) // 128          # bin of i-th sorted node
    lane = np.arange(n) % 128
    core_of = np.empty(n, np.int64)
    gpos_of = np.empty(n, np.int64)
    lane_of = np.empty(n, np.int64)
    core_of[order] = binrank % ncores
    gpos_of[order] = binrank // ncores
    lane_of[order] = lane
    slot_of = core_of * (nblk * 128) + gpos_of * 128 + lane_of

    # per-bin max degree -> per-block-position profile -> group-pad
    degs_sorted = deg[order]
    nb_used = (n + 127) // 128
    Tbin = np.ones(nbins, np.int64)
    maxs = np.maximum.reduceat(degs_sorted, np.arange(0, n, 128))
    Tbin[:nb_used] = np.maximum(maxs, 1)
    Tprof = np.ones(nblk, np.int64)
    for g in range(nblk):
        lo, hi = g * ncores, min((g + 1) * ncores, nbins)
        Tprof[g] = max(1, Tbin[lo:hi].max())
    Tpad = Tprof.copy()
    groups = []
    g = 0
    while g < nblk:
        nb = min(gb, nblk - g)
        T = int(Tprof[g:g + nb].max())
        Tpad[g:g + nb] = T
        groups.append((g, nb, T))
        g += nb
    offs = np.zeros(nblk + 1, np.int64)
    np.cumsum(Tpad, out=offs[1:])
    SumT = int(offs[nblk])
    cfg.Tpad, cfg.offs, cfg.SumT, cfg.groups = Tpad, offs, SumT, groups

    # edge -> (core, column, lane) in dst-major layout
    dslot = slot_of[dst]
    eorder = np.argsort(dslot, kind="stable")
    src_s, dslot_s = src[eorder], dslot[eorder]
    # j = rank within dst
    uniq, starts_idx = np.unique(dslot_s, return_index=True)
    j_of = np.arange(e + n, dtype=np.int64)
    j_of -= np.repeat(starts_idx, np.diff(np.append(starts_idx, e + n)))
    ecore = dslot_s // (nblk * 128)
    egpos = (dslot_s // 128) % nblk
    elane = dslot_s % 128
    ecol = offs[egpos] + j_of

    # per-core index matrices [128, SumT]:
    #   Iv: row into h_ext (nslots -> zero row)
    #   Is: row into s_ext (nslots -> SPAD row, nslots+1 -> zero row)
    prep = Prep()
    prep.Iv, prep.Is = [], []
    for c in range(ncores):
        iv = np.full((128, SumT), nslots, np.int64)
        is_ = np.full((128, SumT), nslots, np.int64)
        m = ecore == c
        iv[elane[m], ecol[m]] = slot_of[src_s[m]]
        is_[elane[m], ecol[m]] = slot_of[src_s[m]]
        prep.Iv.append(iv)
        prep.Is.append(is_)
    # dummy slots (no real node): one self edge with v=0, s=0 at j=0
    have = np.zeros(nslots, bool)
    have[slot_of] = True
    dumm = np.nonzero(~have)[0]
    dc = dumm // (nblk * 128)
    dg = (dumm // 128) % nblk
    dl = dumm % 128
    for c in range(ncores):
        m = dc == c
        prep.Is[c][dl[m], offs[dg[m]]] = nslots + 1  # s = 0
        # Iv stays nslots (v = 0)

    # own-node slot grid for d routing [128, nblk]
    g_grid = np.broadcast_to(np.arange(nblk)[None, :], (128, nblk))
    l_grid = np.broadcast_to(np.arange(128)[:, None], (128, nblk))
    prep.dgrid = [c * nblk * 128 + g_grid * 128 + l_grid for c in range(ncores)]

    prep.slot_of = slot_of
    prep.cfg = cfg
    return prep


def route_edge_arrays(prep, cfg, h_all16, s_all16, d_all16):
    """h_all16 [nslots,128] f16, s_all16 [nslots,4] f16, d_all16 [nslots,4].
    Returns per-core (v [128,SumT,128], sdin [128,SumT+nblk,4]) where sdin
    interleaves per group: [s cols of its blocks | d cols of its blocks]."""
    h_ext = np.vstack([h_all16, np.zeros((1, 128), np.float16)])
    s_ext = np.vstack([s_all16,
                       np.full((1, 4), SPAD, np.float16),
                       np.zeros((1, 4), np.float16)])
    out = []
    for c in range(cfg.ncores):
        v = h_ext[prep.Iv[c]]
        s = s_ext[prep.Is[c]]
        d = d_all16[prep.dgrid[c]]
        sdin = np.zeros((128, cfg.SumT + cfg.nblk, 4), np.float16)
        for (g0, nb, T) in cfg.groups:
            off = int(cfg.offs[g0])
            nt = nb * T
            base = off + g0
            sdin[:, base:base + nt] = s[:, off:off + nt]
            sdin[:, base + nt:base + nt + nb] = d[:, g0:g0 + nb]
        out.append((v, sdin))
    return out


# ----------------------------------------------------------------------------
# Device kernels
# ----------------------------------------------------------------------------

def build_launch_a(cfg: Cfg):
    """xpT = res_W^T x^T + res_b ; [h1|s1|d1] = x @ (res_W@W1cat) + res_b@W1cat."""
    nc = bacc.Bacc("TRN2", target_bir_lowering=False, debug=False,
                   num_devices=cfg.ncores)
    nblk = cfg.nblk
    xT = nc.dram_tensor("xT", [128, cfg.slots], F16, kind="ExternalInput").ap()
    rw = nc.dram_tensor("rw", [128, 128], F16, kind="ExternalInput").ap()
    rb = nc.dram_tensor("rb", [128, 1], F32, kind="ExternalInput").ap()
    wfh = nc.dram_tensor("wfh", [128, 128], F16, kind="ExternalInput").ap()
    wfsd = nc.dram_tensor("wfsd", [128, 8], F16, kind="ExternalInput").ap()
    ones1 = nc.dram_tensor("ones1", [1, 128], F16, kind="ExternalInput").ap()
    bbh = nc.dram_tensor("bbh", [1, 128], F16, kind="ExternalInput").ap()
    bbsd = nc.dram_tensor("bbsd", [1, 8], F16, kind="ExternalInput").ap()
    hsd1 = nc.dram_tensor("hsd1", [128, nblk, 136], F16,
                          kind="ExternalOutput").ap()
    xpT = nc.dram_tensor("xpT", [128, nblk, 128], F16,
                         kind="ExternalOutput").ap()

    CHA = 8  # blocks per IO chunk (two PSUM sub-groups of 4)
    with tile.TileContext(nc) as tc:
        with (
            tc.tile_pool(name="const", bufs=1) as cp,
            tc.tile_pool(name="io", bufs=3) as iop,
            tc.tile_pool(name="psx", bufs=2, space="PSUM") as psx_p,
            tc.tile_pool(name="psh", bufs=2, space="PSUM") as psh_p,
            tc.tile_pool(name="pss", bufs=2, space="PSUM") as pss_p,
        ):
            rw_t = cp.tile([128, 128], F16)
            nc.sync.dma_start(out=rw_t[:], in_=rw)
            rb_t = cp.tile([128, 1], F32)
            nc.sync.dma_start(out=rb_t[:], in_=rb)
            wfh_t = cp.tile([128, 128], F16)
            nc.sync.dma_start(out=wfh_t[:], in_=wfh)
            wfsd_t = cp.tile([128, 8], F16)
            nc.sync.dma_start(out=wfsd_t[:], in_=wfsd)
            on_t = cp.tile([1, 128], F16)
            nc.sync.dma_start(out=on_t[:], in_=ones1)
            bbh_t = cp.tile([1, 128], F16)
            nc.sync.dma_start(out=bbh_t[:], in_=bbh)
            bbsd_t = cp.tile([1, 8], F16)
            nc.sync.dma_start(out=bbsd_t[:], in_=bbsd)

            for c0 in range(0, nblk, CHA):
                nch = min(CHA, nblk - c0)
                xt = iop.tile([128, CHA, 128], F16, tag="xt")
                nc.sync.dma_start(
                    out=xt[:, 0:nch, :],
                    in_=xT[:, c0 * 128:(c0 + nch) * 128].rearrange(
                        "p (b q) -> p b q", b=nch))
                xpc = iop.tile([128, CHA, 128], F16, tag="xpc")
                hsdc = iop.tile([128, CHA, 136], F16, tag="hsdc")
                for i0 in range(0, nch, GB):
                    nb = min(GB, nch - i0)
                    psx = psx_p.tile([128, GB, 128], F32, tag="x")
                    psh = psh_p.tile([128, GB, 128], F32, tag="h")
                    pss = pss_p.tile([128, GB, 8], F32, tag="s")
                    for b in range(nb):
                        bi = i0 + b
                        nc.tensor.matmul(out=psx[:, b, :], lhsT=rw_t[:],
                                         rhs=xt[:, bi, :], start=True,
                                         stop=True)
                        nc.tensor.matmul(out=psh[:, b, :], lhsT=xt[:, bi, :],
                                         rhs=wfh_t[:], start=True, stop=False)
                        nc.tensor.matmul(out=psh[:, b, :], lhsT=on_t[:],
                                         rhs=bbh_t[:], start=False, stop=True)
                        nc.tensor.matmul(out=pss[:, b, :], lhsT=xt[:, bi, :],
                                         rhs=wfsd_t[:], start=True, stop=False)
                        nc.tensor.matmul(out=pss[:, b, :], lhsT=on_t[:],
                                         rhs=bbsd_t[:], start=False, stop=True)
                    nc.scalar.activation(
                        out=xpc[:, i0:i0 + nb, :], in_=psx[:, 0:nb, :],
                        func=mybir.ActivationFunctionType.Identity,
                        bias=rb_t[:])
                    nc.vector.tensor_copy(hsdc[:, i0:i0 + nb, 0:128],
                                          psh[:, 0:nb, :])
                    nc.vector.tensor_copy(hsdc[:, i0:i0 + nb, 128:136],
                                          pss[:, 0:nb, :])
                nc.gpsimd.dma_start(out=xpT[:, c0:c0 + nch, :],
                                    in_=xpc[:, 0:nch, :])
                nc.scalar.dma_start(out=hsd1[:, c0:c0 + nch, :],
                                    in_=hsdc[:, 0:nch, :])
    nc.compile()
    return nc


def build_launch_edge(cfg: Cfg, final: bool):
    """Edge phase (softmax-weighted aggregation, BN+ELU) for one layer.
    final=False: + layer-2 node matmul (h2/sd2 out).
    final=True:  + residual + MLP head + log_softmax (fin out)."""
    nc = bacc.Bacc("TRN2", target_bir_lowering=False, debug=False,
                   num_devices=cfg.ncores)
    nblk, SumT = cfg.nblk, cfg.SumT
    GT = GB * max(g[2] for g in cfg.groups)

    v_ap = nc.dram_tensor("v", [128, SumT, 128], F16, kind="ExternalInput").ap()
    sdin = nc.dram_tensor("sdin", [128, SumT + nblk, 4], F16,
                          kind="ExternalInput").ap()
    kT = nc.dram_tensor("kT", [128, 1], F32, kind="ExternalInput").ap()
    cT = nc.dram_tensor("cT", [128, 1], F32, kind="ExternalInput").ap()
    idf16 = nc.dram_tensor("idf16", [128, 128], F16, kind="ExternalInput").ap()
    if not final:
        w2h = nc.dram_tensor("w2h", [128, 128], F16, kind="ExternalInput").ap()
        w2sd = nc.dram_tensor("w2sd", [128, 8], F16, kind="ExternalInput").ap()
        hsd2 = nc.dram_tensor("hsd2", [128, nblk, 136], F16,
                              kind="ExternalOutput").ap()
    else:
        xpT = nc.dram_tensor("xpT", [128, nblk, 128], F16,
                             kind="ExternalInput").ap()
        wc1 = nc.dram_tensor("wc1", [128, 64], F16, kind="ExternalInput").ap()
        ccT = nc.dram_tensor("ccT", [64, 1], F32, kind="ExternalInput").ap()
        wc2 = nc.dram_tensor("wc2", [64, 40], F16, kind="ExternalInput").ap()
        bc2r = nc.dram_tensor("bc2r", [1, 40], F16, kind="ExternalInput").ap()
        ones1 = nc.dram_tensor("ones1", [1, 128], F16,
                               kind="ExternalInput").ap()
        fin = nc.dram_tensor("fin", [128, nblk, 40], F32,
                             kind="ExternalOutput").ap()

    EXP = mybir.ActivationFunctionType.Exp
    RELU = mybir.ActivationFunctionType.Relu
    with tile.TileContext(nc) as tc:
        with (
            tc.tile_pool(name="const", bufs=1) as cp,
            tc.tile_pool(name="vp", bufs=3) as vp,
            tc.tile_pool(name="sp", bufs=4) as sp,
            tc.tile_pool(name="up", bufs=4) as up,
            tc.tile_pool(name="psagg", bufs=3, space="PSUM") as psagg_p,
            tc.tile_pool(name="psh", bufs=2, space="PSUM") as psh_p,
            tc.tile_pool(name="pssd", bufs=1, space="PSUM") as pssd_p,
            tc.tile_pool(name="psprt", bufs=1, space="PSUM") as psprt_p,
        ):
            kT_t = cp.tile([128, 1], F32)
            nc.sync.dma_start(out=kT_t[:], in_=kT)
            cT_t = cp.tile([128, 1], F32)
            nc.sync.dma_start(out=cT_t[:], in_=cT)
            idf16_t = cp.tile([128, 128], F16)
            nc.sync.dma_start(out=idf16_t[:], in_=idf16)
            if not final:
                w2h_t = cp.tile([128, 128], F16)
                nc.sync.dma_start(out=w2h_t[:], in_=w2h)
                w2sd_t = cp.tile([128, 8], F16)
                nc.sync.dma_start(out=w2sd_t[:], in_=w2sd)
            else:
                wc1_t = cp.tile([128, 64], F16)
                nc.sync.dma_start(out=wc1_t[:], in_=wc1)
                ccT_t = cp.tile([64, 1], F32)
                nc.sync.dma_start(out=ccT_t[:], in_=ccT)
                wc2_t = cp.tile([64, 40], F16)
                nc.sync.dma_start(out=wc2_t[:], in_=wc2)
                bc2r_t = cp.tile([1, 40], F16)
                nc.sync.dma_start(out=bc2r_t[:], in_=bc2r)
                on_t = cp.tile([1, 128], F16)
                nc.sync.dma_start(out=on_t[:], in_=ones1)
                ysb = cp.tile([128, nblk, 40], F32)
                eyt = cp.tile([128, nblk, 40], F16)
                zs = cp.tile([128, nblk], F32)

            def frontA(grp):
                """DMA in + softmax chain through alpha."""
                g0, nb, T = grp
                off = int(cfg.offs[g0])
                nt = nb * T
                vt = vp.tile([128, GT, 128], F16, tag="v")
                nc.sync.dma_start(out=vt[:, 0:nt, :],
                                  in_=v_ap[:, off:off + nt, :])
                sdt = sp.tile([128, GT + GB, 4], F16, tag="sd")
                nc.sync.dma_start(
                    out=sdt[:, 0:nt + nb, :],
                    in_=sdin[:, off + g0:off + g0 + nt + nb, :])
                st = sdt[:, 0:nt, :]
                dt_ = sdt[:, nt:nt + nb, :]
                xpt = None
                if final:
                    xpt = up.tile([128, GB, 128], F16, tag="xpt")
                    nc.sync.dma_start(out=xpt[:, 0:nb, :],
                                      in_=xpT[:, g0:g0 + nb, :])

                # e = leaky(s + d)  [p, (b j) h] fp16
                et = sp.tile([128, GT, 4], F16, tag="e")
                e_bjh = et[:, 0:nt, :].rearrange("p (b j) h -> p b j h", b=nb)
                d_b = dt_.unsqueeze(2).to_broadcast([128, nb, T, 4])
                s_bjh = st.rearrange("p (b j) h -> p b j h", b=nb)
                nc.gpsimd.tensor_tensor(out=e_bjh, in0=s_bjh, in1=d_b,
                                        op=mybir.AluOpType.add)
                e_flat = et[:, 0:nt, :].rearrange("p a h -> p (a h)")
                nc.vector.scalar_tensor_tensor(
                    out=e_flat, in0=e_flat, scalar=0.2, in1=e_flat,
                    op0=mybir.AluOpType.mult, op1=mybir.AluOpType.max)

                # ex2 = exp(e) duplicated x2 (Act); ez = exp(e) (Act)
                ex2t = sp.tile([128, GT * 8], F16, tag="ex2")
                ex2_v = ex2t[:, 0:nt * 8].rearrange("p (a two) -> p a two",
                                                    two=2)
                e_b2 = e_flat.unsqueeze(-1).to_broadcast([128, nt * 4, 2])
                nc.scalar.activation(out=ex2_v, in_=e_b2, func=EXP)
                ezt = sp.tile([128, GT * 4], F16, tag="ez")
                nc.scalar.activation(out=ezt[:, 0:nt * 4], in_=e_flat,
                                     func=EXP)
                # z[p, b, h] = sum_j ez ; zr = 1/z
                zt = sp.tile([128, GB, 4], F32, tag="z")
                ez_bhj = ezt[:, 0:nt * 4].rearrange(
                    "p (b j h) -> p b h j", b=nb, j=T, h=4)
                nc.vector.tensor_reduce(out=zt[:, 0:nb, :], in_=ez_bhj,
                                        axis=mybir.AxisListType.X,
                                        op=mybir.AluOpType.add)
                zrt = sp.tile([128, GB * 4], F32, tag="zr")
                nc.vector.reciprocal(
                    zrt[:, 0:nb * 4],
                    zt[:, 0:nb, :].rearrange("p b h -> p (b h)"))
                # alpha = ex / z  (per block; zr broadcast over j and pair)
                for b in range(nb):
                    ex2_blk = ex2t[:, b * T * 8:(b + 1) * T * 8].rearrange(
                        "p (j h two) -> p j h two", j=T, h=4, two=2)
                    zr_b = zrt[:, b * 4:(b + 1) * 4].unsqueeze(1)\
                        .unsqueeze(-1).to_broadcast([128, T, 4, 2])
                    nc.gpsimd.tensor_tensor(out=ex2_blk, in0=ex2_blk,
                                            in1=zr_b,
                                            op=mybir.AluOpType.mult)
                return (g0, nb, T, nt, vt, ex2t, xpt)

            def frontB(actx):
                """w = v*alpha (DVE) + PE transpose-accumulate."""
                g0, nb, T, nt, vt, ex2t, xpt = actx
                ex2_b = ex2t[:, 0:nt * 8].rearrange(
                    "p (a two) -> p a two", two=2).unsqueeze(2).to_broadcast(
                    [128, nt * 4, 16, 2])
                w_m = vt[:, 0:nt, :].rearrange(
                    "p bj (h c16 two) -> p (bj h) c16 two", h=4, c16=16,
                    two=2)
                nc.vector.tensor_tensor(out=w_m, in0=w_m, in1=ex2_b,
                                        op=mybir.AluOpType.mult)

                # aggT[feat, b, dst] += w_j^T  (PE transpose-accumulate)
                psagg = psagg_p.tile([128, GB, 128], F32, tag="agg")
                for b in range(nb):
                    for j in range(T):
                        nc.tensor.matmul(out=psagg[:, b, :],
                                         lhsT=vt[:, b * T + j, :],
                                         rhs=idf16_t[:],
                                         start=(j == 0), stop=(j == T - 1))
                return (g0, nb, psagg, xpt)

            def tail1(ctx):
                """BN+ELU: g=exp(k*agg+c), t1=relu(k*agg+c), u=min(g-1,t1)."""
                g0, nb, psagg, xpt = ctx
                gt = up.tile([128, GB, 128], F16, tag="g")
                nc.scalar.activation(out=gt[:, 0:nb, :],
                                     in_=psagg[:, 0:nb, :],
                                     func=EXP, bias=cT_t[:], scale=kT_t[:])
                t1t = up.tile([128, GB, 128], F16, tag="t1")
                nc.scalar.activation(out=t1t[:, 0:nb, :],
                                     in_=psagg[:, 0:nb, :],
                                     func=RELU, bias=cT_t[:], scale=kT_t[:])
                ut = up.tile([128, GB, 128], F16, tag="u")
                nc.vector.scalar_tensor_tensor(
                    out=ut[:, 0:nb, :], in0=gt[:, 0:nb, :], scalar=-1.0,
                    in1=t1t[:, 0:nb, :],
                    op0=mybir.AluOpType.add, op1=mybir.AluOpType.min)
                return (g0, nb, ut, xpt)

            def tail2(ctx):
                """Node matmul / MLP head + output."""
                g0, nb, ut, xpt = ctx
                if not final:
                    psh = psh_p.tile([128, GB, 128], F32, tag="h2")
                    pss = pssd_p.tile([128, GB, 8], F32, tag="sd2")
                    for b in range(nb):
                        nc.tensor.matmul(out=psh[:, b, :], lhsT=ut[:, b, :],
                                         rhs=w2h_t[:], start=True, stop=True)
                        nc.tensor.matmul(out=pss[:, b, :], lhsT=ut[:, b, :],
                                         rhs=w2sd_t[:], start=True, stop=True)
                    hsdc = up.tile([128, GB, 136], F16, tag="hsdc")
                    nc.scalar.copy(hsdc[:, 0:nb, 0:128], psh[:, 0:nb, :])
                    nc.vector.tensor_copy(hsdc[:, 0:nb, 128:136],
                                          pss[:, 0:nb, :])
                    nc.scalar.dma_start(out=hsd2[:, g0:g0 + nb, :],
                                        in_=hsdc[:, 0:nb, :])
                else:
                    nc.gpsimd.tensor_tensor(out=ut[:, 0:nb, :],
                                            in0=ut[:, 0:nb, :],
                                            in1=xpt[:, 0:nb, :],
                                            op=mybir.AluOpType.add)
                    # MLP head: prT = Wc1k^T @ u (feat-major rhs = uT);
                    # r1 = relu(prT + cc) ; y = r1^T @ Wc2 + bc2 (1-row mm)
                    psprT = psprt_p.tile([128, GB, 128], F32, tag="prT")
                    for b in range(nb):
                        nc.tensor.matmul(out=psprT[0:64, b, :],
                                         lhsT=wc1_t[:], rhs=ut[:, b, :],
                                         start=True, stop=True)
                    r1sb = up.tile([64, GB, 128], F16, tag="r1")
                    nc.scalar.activation(out=r1sb[0:64, 0:nb, :],
                                         in_=psprT[0:64, 0:nb, :],
                                         func=RELU, bias=ccT_t[:])
                    psy = pssd_p.tile([128, GB, 40], F32, tag="y")
                    for b in range(nb):
                        nc.tensor.matmul(out=psy[:, b, :],
                                         lhsT=r1sb[:, b, :],
                                         rhs=wc2_t[:], start=True, stop=False)
                        nc.tensor.matmul(out=psy[:, b, :], lhsT=on_t[:],
                                         rhs=bc2r_t[:], start=False,
                                         stop=True)
                    nc.scalar.copy(ysb[:, g0:g0 + nb, :], psy[:, 0:nb, :])
                    nc.scalar.activation(
                        out=eyt[:, g0:g0 + nb, :], in_=psy[:, 0:nb, :],
                        func=EXP)
                    nc.vector.tensor_reduce(
                        out=zs[:, g0:g0 + nb], in_=eyt[:, g0:g0 + nb, :],
                        axis=mybir.AxisListType.X, op=mybir.AluOpType.add)

            # software pipeline: frontB(i-1) | frontA(i) | tail(i-2)
            a_pend = []
            b_pend = []
            for grp in cfg.groups:
                if a_pend:
                    b_pend.append(frontB(a_pend.pop(0)))
                a_pend.append(frontA(grp))
                if len(b_pend) > 1:
                    tail2(tail1(b_pend.pop(0)))
            while a_pend:
                b_pend.append(frontB(a_pend.pop(0)))
            while b_pend:
                tail2(tail1(b_pend.pop(0)))

            if final:
                # log_softmax epilogue: one Ln, then subtract+DMA in halves
                lnz = cp.tile([128, nblk], F32)
                nc.scalar.activation(out=lnz[:], in_=zs[:],
                                     func=mybir.ActivationFunctionType.Ln)
                finsb = cp.tile([128, nblk, 40], F32)
                half = nblk // 2
                for lo, hi in ((0, half), (half, nblk)):
                    lnz_b = lnz[:, lo:hi].unsqueeze(-1).to_broadcast(
                        [128, hi - lo, 40])
                    nc.vector.tensor_tensor(out=finsb[:, lo:hi, :],
                                            in0=ysb[:, lo:hi, :], in1=lnz_b,
                                            op=mybir.AluOpType.subtract)
                    nc.sync.dma_start(out=fin[:, lo:hi, :],
                                      in_=finsb[:, lo:hi, :])
    nc.compile()
    return nc


# ----------------------------------------------------------------------------
# Host orchestration
# ----------------------------------------------------------------------------

_cache = {}


def _get(key, fn):
    if key not in _cache:
        _cache[key] = fn()
    return _cache[key]


def _amat(a):
    m = np.zeros((D, NH), np.float32)
    for h in range(NH):
        m[h * HD:(h + 1) * HD, h] = a[h]
    return m


def _run(nc, in_maps, cfg, tag):
    res = run_bass_kernel_spmd(nc, in_maps, list(range(cfg.ncores)),
                               trace=PROFILE)
    if PROFILE:
        LAST_EXEC_NS.append((tag, res.exec_time_ns))
    return res.results


def _slotify(arr, cdim):
    """[128, nblk, c] device layout -> [slots, c] (slot = g*128 + lane)."""
    return arr.transpose(1, 0, 2).reshape(-1, cdim)


def kernel(x, edge_index, res_W, res_b,
           W1, as1, ad1, b1, g1, be1, rm1, rv1,
           W2, as2, ad2, b2, g2, be2, rm2, rv2,
           Wc1, bc1, gc, bec, rmc, rvc, Wc2, bc2,
           _cfg=None):
    cfg = _cfg or _get("cfg", lambda: Cfg())
    x = np.asarray(x, np.float32)
    edge_index = np.asarray(edge_index)
    (res_W, res_b, W1, as1, ad1, b1, g1, be1, rm1, rv1,
     W2, as2, ad2, b2, g2, be2, rm2, rv2,
     Wc1, bc1, gc, bec, rmc, rvc, Wc2, bc2) = (
        np.asarray(a, np.float32) for a in (
            res_W, res_b, W1, as1, ad1, b1, g1, be1, rm1, rv1,
            W2, as2, ad2, b2, g2, be2, rm2, rv2,
            Wc1, bc1, gc, bec, rmc, rvc, Wc2, bc2))

    ekey = ("prep", hash(edge_index.tobytes()))
    prep = _get(ekey, lambda: host_prep(np.asarray(edge_index, np.int64), cfg))
    nslots = cfg.ncores * cfg.slots

    def fold_bn(g_, be_, rm_, rv_, bias):
        k = (g_ / np.sqrt(rv_ + EPS_BN)).astype(np.float32)
        c = ((bias - rm_) * k + be_).astype(np.float32)
        return k, c

    k1, c1 = fold_bn(g1, be1, rm1, rv1, b1)
    k2, c2 = fold_bn(g2, be2, rm2, rv2, b2)
    kc, cc = fold_bn(gc, bec, rmc, rvc, bc1)

    W1cat = np.concatenate(
        [W1, W1 @ _amat(as1), W1 @ _amat(ad1)], axis=1).astype(np.float32)
    W2cat = np.concatenate(
        [W2, W2 @ _amat(as2), W2 @ _amat(ad2)], axis=1).astype(np.float32)
    Wfold = (res_W.astype(np.float32) @ W1cat)
    bb = (res_b.astype(np.float32) @ W1cat)

    ident16 = np.eye(128, dtype=np.float16)

    # ---- launch A ----
    x_sl = np.zeros((nslots, IN), np.float32)
    x_sl[prep.slot_of] = x
    nc_a = _get(("A",), lambda: build_launch_a(cfg))
    in_a = []
    for c in range(cfg.ncores):
        xs = x_sl[c * cfg.slots:(c + 1) * cfg.slots]
        in_a.append(dict(
            xT=np.ascontiguousarray(xs.T).astype(np.float16),
            rw=res_W.astype(np.float16),
            rb=np.asarray(res_b, np.float32).reshape(128, 1),
            wfh=Wfold[:, 0:128].astype(np.float16),
            wfsd=Wfold[:, 128:136].astype(np.float16),
            ones1=np.ones((1, 128), np.float16),
            bbh=bb[0:128].reshape(1, 128).astype(np.float16),
            bbsd=bb[128:136].reshape(1, 8).astype(np.float16)))
    res_a = _run(nc_a, in_a, cfg, "A")

    hsd1_all = np.concatenate(
        [_slotify(res_a[c]["hsd1"], 136) for c in range(cfg.ncores)])
    h1_all, sd1_all = hsd1_all[:, 0:128], hsd1_all[:, 128:136]
    xp_dev = [res_a[c]["xpT"] for c in range(cfg.ncores)]

    # ---- launch B ----
    vsd1 = route_edge_arrays(prep, cfg, h1_all, sd1_all[:, 0:4],
                             sd1_all[:, 4:8])
    key_e = (cfg.SumT, tuple(g[2] for g in cfg.groups))
    nc_b = _get(("B", key_e), lambda: build_launch_edge(cfg, final=False))
    in_b = []
    for c in range(cfg.ncores):
        v, sdin = vsd1[c]
        in_b.append(dict(
            v=v, sdin=sdin, kT=k1.reshape(128, 1),
            cT=c1.reshape(128, 1), idf16=ident16,
            w2h=W2cat[:, 0:128].astype(np.float16),
            w2sd=W2cat[:, 128:136].astype(np.float16)))
    res_b_ = _run(nc_b, in_b, cfg, "B")

    hsd2_all = np.concatenate(
        [_slotify(res_b_[c]["hsd2"], 136) for c in range(cfg.ncores)])
    h2_all, sd2_all = hsd2_all[:, 0:128], hsd2_all[:, 128:136]

    # ---- launch C ----
    vsd2 = route_edge_arrays(prep, cfg, h2_all, sd2_all[:, 0:4],
                             sd2_all[:, 4:8])
    nc_c = _get(("C", key_e), lambda: build_launch_edge(cfg, final=True))
    Wc1k = (Wc1.astype(np.float32) * kc[None, :]).astype(np.float16)
    in_c = []
    for c in range(cfg.ncores):
        v, sdin = vsd2[c]
        in_c.append(dict(
            v=v, sdin=sdin, kT=k2.reshape(128, 1),
            cT=c2.reshape(128, 1), idf16=ident16,
            xpT=xp_dev[c], wc1=Wc1k, ccT=cc.reshape(64, 1),
            wc2=np.asarray(Wc2, np.float32).astype(np.float16),
            bc2r=np.asarray(bc2, np.float32).reshape(1, OUT).astype(np.float16),
            ones1=np.ones((1, 128), np.float16)))
    res_c = _run(nc_c, in_c, cfg, "C")

    fin_slots = np.concatenate(
        [_slotify(res_c[c]["fin"], 40) for c in range(cfg.ncores)])
    return np.ascontiguousarray(fin_slots[prep.slot_of]).astype(np.float32)


# revision 70
# speedup vs baseline: 1.3346x; 1.0435x over previous
"""Trainium2 Bass kernel for nn_GAT_Vanilla (2-layer GAT + BN/ELU + MLP head).

Strategy (8 NeuronCores, graph/data parallel, dst-major edge layout):
- Nodes sorted by in-degree and chopped into 128-node bins (degree
  homogeneous); bins dealt round-robin to the 8 cores so every core gets
  an identical block-T profile. Edges are laid dst-major: partition lane
  = dst slot, free dim = (j-th incoming edge, feature).
- 3 SPMD launches: A) per-node matmuls producing h1/s1/d1/x_p^T;
  B) layer-1 edge phase (softmax-weighted aggregation) + layer-2 node
  matmul; C) layer-2 edge phase + residual + MLP head + log_softmax.
- Between launches the host routes device-computed per-node values into
  per-edge arrays (pure indexing/gather, no math) - the halo exchange.
- On device per block group: e=s+d, leaky (Pool); exp duplicated x2
  (Act, keeps DVE packed-2x mode); one DVE multiply v*=ex; PE
  transpose-accumulate (matmul vs identity) scatter-sums into PSUM
  producing agg^T feat-major; 1/z and BN-scale ride a tiny head-expand
  matmul; BN bias + ELU fused into Act ops. fp16 data, f32 accumulate.

Self-contained: only needs numpy + the concourse/bass stack.
"""

import numpy as np

import concourse.bass as bass
import concourse.bacc as bacc
import concourse.tile as tile
from concourse import mybir
from concourse.bass_utils import run_bass_kernel_spmd

F32 = mybir.dt.float32
F16 = mybir.dt.float16

# ---- problem constants (hardcoded per harness contract) ----
N, E, IN, HD, NH, OUT = 100000, 800000, 128, 32, 4, 40
D = HD * NH  # 128
EPS_BN = 1e-5
SPAD = -300.0  # pad-edge s value -> leaky -> -60 -> exp == 0

NCORES = 8
NBLK = 98   # blocks (bins) per core; 8*98*128 = 100352 slots >= N
GB = 4      # blocks per group (uniform T within a group)

PROFILE = False
LAST_EXEC_NS = []


class Cfg:
    def __init__(self, n=N, e=E, ncores=NCORES, nblk=NBLK, gb=GB):
        self.n, self.e, self.ncores, self.nblk, self.gb = n, e, ncores, nblk, gb
        self.slots = nblk * 128
        # set by host_prep:
        self.Tpad = None    # per-block T (uniform within each group)
        self.offs = None    # per-block tile offset, offs[nblk] = SumT
        self.SumT = None
        self.groups = None  # list of (g0, nb, T)


class Prep:
    pass


# ----------------------------------------------------------------------------
# Host preprocessing: degree-sorted binning + per-edge index matrices
# ----------------------------------------------------------------------------

def host_prep(edge_index, cfg: Cfg):
    n, e = cfg.n, cfg.e
    ncores, nblk, gb = cfg.ncores, cfg.nblk, cfg.gb
    nbins = ncores * nblk
    nslots = nbins * 128
    src = np.concatenate([edge_index[0], np.arange(n)]).astype(np.int64)
    dst = np.concatenate([edge_index[1], np.arange(n)]).astype(np.int64)
    deg = np.bincount(dst, minlength=n)

    # degree-sorted consecutive bins of 128 nodes; bin k -> core k%ncores,
    # block position k//ncores, so all cores share one T profile.
    order = np.argsort(-deg, kind="stable")
    binrank = np.arange(n) // 128          # bin of i-th sorted node
    lane = np.arange(n) % 128
    core_of = np.empty(n, np.int64)
    gpos_of = np.empty(n, np.int64)
    lane_of = np.empty(n, np.int64)
    core_of[order] = binrank % ncores
    gpos_of[order] = binrank // ncores
    lane_of[order] = lane
    slot_of = core_of * (nblk * 128) + gpos_of * 128 + lane_of

    # per-bin max degree -> per-block-position profile -> group-pad
    degs_sorted = deg[order]
    nb_used = (n + 127) // 128
    Tbin = np.ones(nbins, np.int64)
    maxs = np.maximum.reduceat(degs_sorted, np.arange(0, n, 128))
    Tbin[:nb_used] = np.maximum(maxs, 1)
    Tprof = np.ones(nblk, np.int64)
    for g in range(nblk):
        lo, hi = g * ncores, min((g + 1) * ncores, nbins)
        Tprof[g] = max(1, Tbin[lo:hi].max())
    Tpad = Tprof.copy()
    groups = []
    g = 0
    while g < nblk:
        nb = min(gb, nblk - g)
        T = int(Tprof[g:g + nb].max())
        Tpad[g:g + nb] = T
        groups.append((g, nb, T))
        g += nb
    offs = np.zeros(nblk + 1, np.int64)
    np.cumsum(Tpad, out=offs[1:])
    SumT = int(offs[nblk])
    cfg.Tpad, cfg.offs, cfg.SumT, cfg.groups = Tpad, offs, SumT, groups

    # edge -> (core, column, lane) in dst-major layout
    dslot = slot_of[dst]
    eorder = np.argsort(dslot, kind="stable")
    src_s, dslot_s = src[eorder], dslot[eorder]
    # j = rank within dst
    uniq, starts_idx = np.unique(dslot_s, return_index=True)
    j_of = np.arange(e + n, dtype=np.int64)
    j_of -= np.repeat(starts_idx, np.diff(np.append(starts_idx, e + n)))
    ecore = dslot_s // (nblk * 128)
    egpos = (dslot_s // 128) % nblk
    elane = dslot_s % 128
    ecol = offs[egpos] + j_of

    # per-core index matrices [128, SumT]:
    #   Iv: row into h_ext (nslots -> zero row)
    #   Is: row into s_ext (nslots -> SPAD row, nslots+1 -> zero row)
    prep = Prep()
    prep.Iv, prep.Is = [], []
    for c in range(ncores):
        iv = np.full((128, SumT), nslots, np.int64)
        is_ = np.full((128, SumT), nslots, np.int64)
        m = ecore == c
        iv[elane[m], ecol[m]] = slot_of[src_s[m]]
        is_[elane[m], ecol[m]] = slot_of[src_s[m]]
        prep.Iv.append(iv)
        prep.Is.append(is_)
    # dummy slots (no real node): one self edge with v=0, s=0 at j=0
    have = np.zeros(nslots, bool)
    have[slot_of] = True
    dumm = np.nonzero(~have)[0]
    dc = dumm // (nblk * 128)
    dg = (dumm // 128) % nblk
    dl = dumm % 128
    for c in range(ncores):
        m = dc == c
        prep.Is[c][dl[m], offs[dg[m]]] = nslots + 1  # s = 0
        # Iv stays nslots (v = 0)

    # own-node slot grid for d routing [128, nblk]
    g_grid = np.broadcast_to(np.arange(nblk)[None, :], (128, nblk))
    l_grid = np.broadcast_to(np.arange(128)[:, None], (128, nblk))
    prep.dgrid = [c * nblk * 128 + g_grid * 128 + l_grid for c in range(ncores)]

    prep.slot_of = slot_of
    prep.cfg = cfg
    return prep


def route_edge_arrays(prep, cfg, h_all16, s_all16, d_all16):
    """h_all16 [nslots,128] f16, s_all16 [nslots,4] f16, d_all16 [nslots,4].
    Returns per-core (v [128,SumT,128], sdin [128,SumT+nblk,4]) where sdin
    interleaves per group: [s cols of its blocks | d cols of its blocks]."""
    h_ext = np.vstack([h_all16, np.zeros((1, 128), np.float16)])
    s_ext = np.vstack([s_all16,
                       np.full((1, 4), SPAD, np.float16),
                       np.zeros((1, 4), np.float16)])
    out = []
    for c in range(cfg.ncores):
        v = h_ext[prep.Iv[c]]
        s = s_ext[prep.Is[c]]
        d = d_all16[prep.dgrid[c]]
        sdin = np.zeros((128, cfg.SumT + cfg.nblk, 4), np.float16)
        for (g0, nb, T) in cfg.groups:
            off = int(cfg.offs[g0])
            nt = nb * T
            base = off + g0
            sdin[:, base:base + nt] = s[:, off:off + nt]
            sdin[:, base + nt:base + nt + nb] = d[:, g0:g0 + nb]
        out.append((v, sdin))
    return out


# ----------------------------------------------------------------------------
# Device kernels
# ----------------------------------------------------------------------------

def build_launch_a(cfg: Cfg):
    """xpT = res_W^T x^T + res_b ; [h1|s1|d1] = x @ (res_W@W1cat) + res_b@W1cat."""
    nc = bacc.Bacc("TRN2", target_bir_lowering=False, debug=False,
                   num_devices=cfg.ncores)
    nblk = cfg.nblk
    xT = nc.dram_tensor("xT", [128, cfg.slots], F16, kind="ExternalInput").ap()
    rw = nc.dram_tensor("rw", [128, 128], F16, kind="ExternalInput").ap()
    rb = nc.dram_tensor("rb", [128, 1], F32, kind="ExternalInput").ap()
    wfh = nc.dram_tensor("wfh", [128, 128], F16, kind="ExternalInput").ap()
    wfsd = nc.dram_tensor("wfsd", [128, 8], F16, kind="ExternalInput").ap()
    ones1 = nc.dram_tensor("ones1", [1, 128], F16, kind="ExternalInput").ap()
    bbh = nc.dram_tensor("bbh", [1, 128], F16, kind="ExternalInput").ap()
    bbsd = nc.dram_tensor("bbsd", [1, 8], F16, kind="ExternalInput").ap()
    hsd1 = nc.dram_tensor("hsd1", [128, nblk, 136], F16,
                          kind="ExternalOutput").ap()
    xpT = nc.dram_tensor("xpT", [128, nblk, 128], F16,
                         kind="ExternalOutput").ap()

    CHA = 8  # blocks per IO chunk (two PSUM sub-groups of 4)
    with tile.TileContext(nc) as tc:
        with (
            tc.tile_pool(name="const", bufs=1) as cp,
            tc.tile_pool(name="io", bufs=3) as iop,
            tc.tile_pool(name="psx", bufs=2, space="PSUM") as psx_p,
            tc.tile_pool(name="psh", bufs=2, space="PSUM") as psh_p,
            tc.tile_pool(name="pss", bufs=2, space="PSUM") as pss_p,
        ):
            rw_t = cp.tile([128, 128], F16)
            nc.sync.dma_start(out=rw_t[:], in_=rw)
            rb_t = cp.tile([128, 1], F32)
            nc.sync.dma_start(out=rb_t[:], in_=rb)
            wfh_t = cp.tile([128, 128], F16)
            nc.sync.dma_start(out=wfh_t[:], in_=wfh)
            wfsd_t = cp.tile([128, 8], F16)
            nc.sync.dma_start(out=wfsd_t[:], in_=wfsd)
            on_t = cp.tile([1, 128], F16)
            nc.sync.dma_start(out=on_t[:], in_=ones1)
            bbh_t = cp.tile([1, 128], F16)
            nc.sync.dma_start(out=bbh_t[:], in_=bbh)
            bbsd_t = cp.tile([1, 8], F16)
            nc.sync.dma_start(out=bbsd_t[:], in_=bbsd)

            def a_front(c0):
                nch = min(CHA, nblk - c0)
                xt = iop.tile([128, CHA, 128], F16, tag="xt")
                nc.sync.dma_start(
                    out=xt[:, 0:nch, :],
                    in_=xT[:, c0 * 128:(c0 + nch) * 128].rearrange(
                        "p (b q) -> p b q", b=nch))
                subs = []
                for i0 in range(0, nch, GB):
                    nb = min(GB, nch - i0)
                    psx = psx_p.tile([128, GB, 128], F32, tag="x")
                    psh = psh_p.tile([128, GB, 128], F32, tag="h")
                    pss = pss_p.tile([128, GB, 8], F32, tag="s")
                    for b in range(nb):
                        bi = i0 + b
                        nc.tensor.matmul(out=psx[:, b, :], lhsT=rw_t[:],
                                         rhs=xt[:, bi, :], start=True,
                                         stop=True)
                        nc.tensor.matmul(out=psh[:, b, :], lhsT=xt[:, bi, :],
                                         rhs=wfh_t[:], start=True, stop=False)
                        nc.tensor.matmul(out=psh[:, b, :], lhsT=on_t[:],
                                         rhs=bbh_t[:], start=False, stop=True)
                        nc.tensor.matmul(out=pss[:, b, :], lhsT=xt[:, bi, :],
                                         rhs=wfsd_t[:], start=True,
                                         stop=False)
                        nc.tensor.matmul(out=pss[:, b, :], lhsT=on_t[:],
                                         rhs=bbsd_t[:], start=False,
                                         stop=True)
                    subs.append((i0, nb, psx, psh, pss))
                return (c0, nch, subs)

            def a_tail(ctx):
                c0, nch, subs = ctx
                xpc = iop.tile([128, CHA, 128], F16, tag="xpc")
                hsdc = iop.tile([128, CHA, 136], F16, tag="hsdc")
                for (i0, nb, psx, psh, pss) in subs:
                    nc.scalar.activation(
                        out=xpc[:, i0:i0 + nb, :], in_=psx[:, 0:nb, :],
                        func=mybir.ActivationFunctionType.Identity,
                        bias=rb_t[:])
                    nc.vector.tensor_copy(hsdc[:, i0:i0 + nb, 0:128],
                                          psh[:, 0:nb, :])
                    nc.vector.tensor_copy(hsdc[:, i0:i0 + nb, 128:136],
                                          pss[:, 0:nb, :])
                nc.gpsimd.dma_start(out=xpT[:, c0:c0 + nch, :],
                                    in_=xpc[:, 0:nch, :])
                nc.scalar.dma_start(out=hsd1[:, c0:c0 + nch, :],
                                    in_=hsdc[:, 0:nch, :])

            a_pend = []
            for c0 in range(0, nblk, CHA):
                a_pend.append(a_front(c0))
                if len(a_pend) > 1:
                    a_tail(a_pend.pop(0))
            while a_pend:
                a_tail(a_pend.pop(0))
    nc.compile()
    return nc


def build_launch_edge(cfg: Cfg, final: bool):
    """Edge phase (softmax-weighted aggregation, BN+ELU) for one layer.
    final=False: + layer-2 node matmul (h2/sd2 out).
    final=True:  + residual + MLP head + log_softmax (fin out)."""
    nc = bacc.Bacc("TRN2", target_bir_lowering=False, debug=False,
                   num_devices=cfg.ncores)
    nblk, SumT = cfg.nblk, cfg.SumT
    GT = max(g[1] * g[2] for g in cfg.groups)

    v_ap = nc.dram_tensor("v", [128, SumT, 128], F16, kind="ExternalInput").ap()
    sdin = nc.dram_tensor("sdin", [128, SumT + nblk, 4], F16,
                          kind="ExternalInput").ap()
    kT = nc.dram_tensor("kT", [128, 1], F32, kind="ExternalInput").ap()
    cT = nc.dram_tensor("cT", [128, 1], F32, kind="ExternalInput").ap()
    idf16 = nc.dram_tensor("idf16", [128, 128], F16, kind="ExternalInput").ap()
    if not final:
        w2h = nc.dram_tensor("w2h", [128, 128], F16, kind="ExternalInput").ap()
        w2sd = nc.dram_tensor("w2sd", [128, 8], F16, kind="ExternalInput").ap()
        hsd2 = nc.dram_tensor("hsd2", [128, nblk, 136], F16,
                              kind="ExternalOutput").ap()
    else:
        xpT = nc.dram_tensor("xpT", [128, nblk, 128], F16,
                             kind="ExternalInput").ap()
        wc1 = nc.dram_tensor("wc1", [128, 64], F16, kind="ExternalInput").ap()
        ccT = nc.dram_tensor("ccT", [64, 1], F32, kind="ExternalInput").ap()
        wc2 = nc.dram_tensor("wc2", [64, 40], F16, kind="ExternalInput").ap()
        bc2r = nc.dram_tensor("bc2r", [1, 40], F16, kind="ExternalInput").ap()
        ones1 = nc.dram_tensor("ones1", [1, 128], F16,
                               kind="ExternalInput").ap()
        fin = nc.dram_tensor("fin", [128, nblk, 40], F32,
                             kind="ExternalOutput").ap()

    EXP = mybir.ActivationFunctionType.Exp
    RELU = mybir.ActivationFunctionType.Relu
    with tile.TileContext(nc) as tc:
        with (
            tc.tile_pool(name="const", bufs=1) as cp,
            tc.tile_pool(name="vp", bufs=3) as vp,
            tc.tile_pool(name="sp", bufs=4) as sp,
            tc.tile_pool(name="up", bufs=4) as up,
            tc.tile_pool(name="psagg", bufs=3, space="PSUM") as psagg_p,
            tc.tile_pool(name="psh", bufs=2, space="PSUM") as psh_p,
            tc.tile_pool(name="pssd", bufs=1, space="PSUM") as pssd_p,
            tc.tile_pool(name="psprt", bufs=1, space="PSUM") as psprt_p,
        ):
            kT_t = cp.tile([128, 1], F32)
            nc.sync.dma_start(out=kT_t[:], in_=kT)
            cT_t = cp.tile([128, 1], F32)
            nc.sync.dma_start(out=cT_t[:], in_=cT)
            idf16_t = cp.tile([128, 128], F16)
            nc.sync.dma_start(out=idf16_t[:], in_=idf16)
            if not final:
                w2h_t = cp.tile([128, 128], F16)
                nc.sync.dma_start(out=w2h_t[:], in_=w2h)
                w2sd_t = cp.tile([128, 8], F16)
                nc.sync.dma_start(out=w2sd_t[:], in_=w2sd)
            else:
                wc1_t = cp.tile([128, 64], F16)
                nc.sync.dma_start(out=wc1_t[:], in_=wc1)
                ccT_t = cp.tile([64, 1], F32)
                nc.sync.dma_start(out=ccT_t[:], in_=ccT)
                wc2_t = cp.tile([64, 40], F16)
                nc.sync.dma_start(out=wc2_t[:], in_=wc2)
                bc2r_t = cp.tile([1, 40], F16)
                nc.sync.dma_start(out=bc2r_t[:], in_=bc2r)
                on_t = cp.tile([1, 128], F16)
                nc.sync.dma_start(out=on_t[:], in_=ones1)
                ysb = cp.tile([128, nblk, 40], F32)
                eyt = cp.tile([128, nblk, 40], F16)
                zs = cp.tile([128, nblk], F32)

            def frontA(grp):
                """DMA in + softmax chain through alpha."""
                g0, nb, T = grp
                off = int(cfg.offs[g0])
                nt = nb * T
                sdt = sp.tile([128, GT + GB, 4], F16, tag="sd")
                vt = vp.tile([128, GT, 128], F16, tag="v")
                if not final:  # B: small sdt ahead of the big v transfer
                    nc.sync.dma_start(
                        out=sdt[:, 0:nt + nb, :],
                        in_=sdin[:, off + g0:off + g0 + nt + nb, :])
                    nc.sync.dma_start(out=vt[:, 0:nt, :],
                                      in_=v_ap[:, off:off + nt, :])
                    xpt = None
                else:
                    nc.sync.dma_start(out=vt[:, 0:nt, :],
                                      in_=v_ap[:, off:off + nt, :])
                    nc.sync.dma_start(
                        out=sdt[:, 0:nt + nb, :],
                        in_=sdin[:, off + g0:off + g0 + nt + nb, :])
                    xpt = up.tile([128, GB, 128], F16, tag="xpt")
                    nc.sync.dma_start(out=xpt[:, 0:nb, :],
                                      in_=xpT[:, g0:g0 + nb, :])
                st = sdt[:, 0:nt, :]
                dt_ = sdt[:, nt:nt + nb, :]

                # e = leaky(s + d)  [p, (b j) h] fp16
                et = sp.tile([128, GT, 4], F16, tag="e")
                e_bjh = et[:, 0:nt, :].rearrange("p (b j) h -> p b j h", b=nb)
                d_b = dt_.unsqueeze(2).to_broadcast([128, nb, T, 4])
                s_bjh = st.rearrange("p (b j) h -> p b j h", b=nb)
                nc.gpsimd.tensor_tensor(out=e_bjh, in0=s_bjh, in1=d_b,
                                        op=mybir.AluOpType.add)
                e_flat = et[:, 0:nt, :].rearrange("p a h -> p (a h)")
                e2t = sp.tile([128, GT, 4], F16, tag="e2")
                e2_flat = e2t[:, 0:nt, :].rearrange("p a h -> p (a h)")
                nc.vector.tensor_scalar(out=e2_flat, in0=e_flat, scalar1=0.2,
                                        scalar2=None, op0=mybir.AluOpType.mult)
                nc.vector.tensor_tensor(out=e_flat, in0=e_flat, in1=e2_flat,
                                        op=mybir.AluOpType.max)

                # ex2 = exp(e) duplicated x2 (Act); ez = exp(e) (Act)
                ex2t = sp.tile([128, GT * 8], F16, tag="ex2")
                ex2_v = ex2t[:, 0:nt * 8].rearrange("p (a two) -> p a two",
                                                    two=2)
                e_b2 = e_flat.unsqueeze(-1).to_broadcast([128, nt * 4, 2])
                nc.scalar.activation(out=ex2_v, in_=e_b2, func=EXP)
                ezt = sp.tile([128, GT * 4], F16, tag="ez")
                nc.scalar.activation(out=ezt[:, 0:nt * 4], in_=e_flat,
                                     func=EXP)
                # z[p, b, h] = sum_j ez ; zr = 1/z
                zt = sp.tile([128, GB, 4], F32, tag="z")
                ez_bhj = ezt[:, 0:nt * 4].rearrange(
                    "p (b j h) -> p b h j", b=nb, j=T, h=4)
                nc.vector.tensor_reduce(out=zt[:, 0:nb, :], in_=ez_bhj,
                                        axis=mybir.AxisListType.X,
                                        op=mybir.AluOpType.add)
                zrt = sp.tile([128, GB * 4], F32, tag="zr")
                nc.vector.reciprocal(
                    zrt[:, 0:nb * 4],
                    zt[:, 0:nb, :].rearrange("p b h -> p (b h)"))
                # alpha = ex / z  (per block; zr broadcast over j and pair)
                for b in range(nb):
                    ex2_blk = ex2t[:, b * T * 8:(b + 1) * T * 8].rearrange(
                        "p (j h two) -> p j h two", j=T, h=4, two=2)
                    zr_b = zrt[:, b * 4:(b + 1) * 4].unsqueeze(1)\
                        .unsqueeze(-1).to_broadcast([128, T, 4, 2])
                    nc.gpsimd.tensor_tensor(out=ex2_blk, in0=ex2_blk,
                                            in1=zr_b,
                                            op=mybir.AluOpType.mult)
                return (g0, nb, T, nt, vt, ex2t, xpt)

            def frontB(actx):
                """w = v*alpha (DVE) + PE transpose-accumulate."""
                g0, nb, T, nt, vt, ex2t, xpt = actx
                ex2_b = ex2t[:, 0:nt * 8].rearrange(
                    "p (a two) -> p a two", two=2).unsqueeze(2).to_broadcast(
                    [128, nt * 4, 16, 2])
                w_m = vt[:, 0:nt, :].rearrange(
                    "p bj (h c16 two) -> p (bj h) c16 two", h=4, c16=16,
                    two=2)
                nc.vector.tensor_tensor(out=w_m, in0=w_m, in1=ex2_b,
                                        op=mybir.AluOpType.mult)

                # aggT[feat, b, dst] += w_j^T  (PE transpose-accumulate)
                psagg = psagg_p.tile([128, GB, 128], F32, tag="agg")
                for b in range(nb):
                    for j in range(T):
                        nc.tensor.matmul(out=psagg[:, b, :],
                                         lhsT=vt[:, b * T + j, :],
                                         rhs=idf16_t[:],
                                         start=(j == 0), stop=(j == T - 1))
                return (g0, nb, psagg, xpt)

            def tail1(ctx):
                """BN+ELU: g=exp(k*agg+c), t1=relu(k*agg+c), u=min(g-1,t1)."""
                g0, nb, psagg, xpt = ctx
                gt = up.tile([128, GB, 128], F16, tag="g")
                nc.scalar.activation(out=gt[:, 0:nb, :],
                                     in_=psagg[:, 0:nb, :],
                                     func=EXP, bias=cT_t[:], scale=kT_t[:])
                t1t = up.tile([128, GB, 128], F16, tag="t1")
                nc.scalar.activation(out=t1t[:, 0:nb, :],
                                     in_=psagg[:, 0:nb, :],
                                     func=RELU, bias=cT_t[:], scale=kT_t[:])
                ut = up.tile([128, GB, 128], F16, tag="u")
                nc.vector.scalar_tensor_tensor(
                    out=ut[:, 0:nb, :], in0=gt[:, 0:nb, :], scalar=-1.0,
                    in1=t1t[:, 0:nb, :],
                    op0=mybir.AluOpType.add, op1=mybir.AluOpType.min)
                return (g0, nb, ut, xpt)

            def tail2(ctx):
                """Node matmul / MLP head + output."""
                g0, nb, ut, xpt = ctx
                if not final:
                    psh = psh_p.tile([128, GB, 128], F32, tag="h2")
                    pss = pssd_p.tile([128, GB, 8], F32, tag="sd2")
                    for b in range(nb):
                        nc.tensor.matmul(out=psh[:, b, :], lhsT=ut[:, b, :],
                                         rhs=w2h_t[:], start=True, stop=True)
                        nc.tensor.matmul(out=pss[:, b, :], lhsT=ut[:, b, :],
                                         rhs=w2sd_t[:], start=True, stop=True)
                    hsdc = up.tile([128, GB, 136], F16, tag="hsdc")
                    nc.scalar.copy(hsdc[:, 0:nb, 0:128], psh[:, 0:nb, :])
                    nc.scalar.copy(hsdc[:, 0:nb, 128:136],
                                   pss[:, 0:nb, :])
                    nc.scalar.dma_start(out=hsd2[:, g0:g0 + nb, :],
                                        in_=hsdc[:, 0:nb, :])
                else:
                    # MLP head on h = u + x_p, with the residual folded into
                    # the matmul: prT = Wc1k^T @ u + Wc1k^T @ xpT
                    psprT = psprt_p.tile([128, GB, 128], F32, tag="prT")
                    for b in range(nb):
                        nc.tensor.matmul(out=psprT[0:64, b, :],
                                         lhsT=wc1_t[:], rhs=ut[:, b, :],
                                         start=True, stop=False)
                        nc.tensor.matmul(out=psprT[0:64, b, :],
                                         lhsT=wc1_t[:], rhs=xpt[:, b, :],
                                         start=False, stop=True)
                    r1sb = up.tile([64, GB, 128], F16, tag="r1")
                    nc.scalar.activation(out=r1sb[0:64, 0:nb, :],
                                         in_=psprT[0:64, 0:nb, :],
                                         func=RELU, bias=ccT_t[:])
                    psy = pssd_p.tile([128, GB, 40], F32, tag="y")
                    for b in range(nb):
                        nc.tensor.matmul(out=psy[:, b, :],
                                         lhsT=r1sb[:, b, :],
                                         rhs=wc2_t[:], start=True, stop=False)
                        nc.tensor.matmul(out=psy[:, b, :], lhsT=on_t[:],
                                         rhs=bc2r_t[:], start=False,
                                         stop=True)
                    nc.scalar.copy(ysb[:, g0:g0 + nb, :], psy[:, 0:nb, :])
                    nc.scalar.activation(
                        out=eyt[:, g0:g0 + nb, :], in_=psy[:, 0:nb, :],
                        func=EXP)
                    nc.vector.tensor_reduce(
                        out=zs[:, g0:g0 + nb], in_=eyt[:, g0:g0 + nb, :],
                        axis=mybir.AxisListType.X, op=mybir.AluOpType.add)

            # software pipeline: frontB(i-1) | frontA(i) | tail(i-2)
            a_pend = []
            b_pend = []
            for grp in cfg.groups:
                if a_pend:
                    b_pend.append(frontB(a_pend.pop(0)))
                a_pend.append(frontA(grp))
                if len(b_pend) > 1:
                    tail2(tail1(b_pend.pop(0)))
            while a_pend:
                b_pend.append(frontB(a_pend.pop(0)))
            while b_pend:
                tail2(tail1(b_pend.pop(0)))

            if final:
                # log_softmax epilogue: one Ln, then subtract+DMA in halves
                lnz = cp.tile([128, nblk], F32)
                nc.scalar.activation(out=lnz[:], in_=zs[:],
                                     func=mybir.ActivationFunctionType.Ln)
                finsb = cp.tile([128, nblk, 40], F32)
                half = nblk // 2
                for lo, hi in ((0, half), (half, nblk)):
                    lnz_b = lnz[:, lo:hi].unsqueeze(-1).to_broadcast(
                        [128, hi - lo, 40])
                    nc.vector.tensor_tensor(out=finsb[:, lo:hi, :],
                                            in0=ysb[:, lo:hi, :], in1=lnz_b,
                                            op=mybir.AluOpType.subtract)
                    nc.sync.dma_start(out=fin[:, lo:hi, :],
                                      in_=finsb[:, lo:hi, :])
    nc.compile()
    return nc


# ----------------------------------------------------------------------------
# Host orchestration
# ----------------------------------------------------------------------------

_cache = {}


def _get(key, fn):
    if key not in _cache:
        _cache[key] = fn()
    return _cache[key]


def _amat(a):
    m = np.zeros((D, NH), np.float32)
    for h in range(NH):
        m[h * HD:(h + 1) * HD, h] = a[h]
    return m


def _run(nc, in_maps, cfg, tag):
    res = run_bass_kernel_spmd(nc, in_maps, list(range(cfg.ncores)),
                               trace=PROFILE)
    if PROFILE:
        LAST_EXEC_NS.append((tag, res.exec_time_ns))
    return res.results


def _slotify(arr, cdim):
    """[128, nblk, c] device layout -> [slots, c] (slot = g*128 + lane)."""
    return arr.transpose(1, 0, 2).reshape(-1, cdim)


def kernel(x, edge_index, res_W, res_b,
           W1, as1, ad1, b1, g1, be1, rm1, rv1,
           W2, as2, ad2, b2, g2, be2, rm2, rv2,
           Wc1, bc1, gc, bec, rmc, rvc, Wc2, bc2,
           _cfg=None):
    cfg = _cfg or _get("cfg", lambda: Cfg())
    x = np.asarray(x, np.float32)
    edge_index = np.asarray(edge_index)
    (res_W, res_b, W1, as1, ad1, b1, g1, be1, rm1, rv1,
     W2, as2, ad2, b2, g2, be2, rm2, rv2,
     Wc1, bc1, gc, bec, rmc, rvc, Wc2, bc2) = (
        np.asarray(a, np.float32) for a in (
            res_W, res_b, W1, as1, ad1, b1, g1, be1, rm1, rv1,
            W2, as2, ad2, b2, g2, be2, rm2, rv2,
            Wc1, bc1, gc, bec, rmc, rvc, Wc2, bc2))

    ekey = ("prep", hash(edge_index.tobytes()))
    prep = _get(ekey, lambda: host_prep(np.asarray(edge_index, np.int64), cfg))
    nslots = cfg.ncores * cfg.slots

    def fold_bn(g_, be_, rm_, rv_, bias):
        k = (g_ / np.sqrt(rv_ + EPS_BN)).astype(np.float32)
        c = ((bias - rm_) * k + be_).astype(np.float32)
        return k, c

    k1, c1 = fold_bn(g1, be1, rm1, rv1, b1)
    k2, c2 = fold_bn(g2, be2, rm2, rv2, b2)
    kc, cc = fold_bn(gc, bec, rmc, rvc, bc1)

    W1cat = np.concatenate(
        [W1, W1 @ _amat(as1), W1 @ _amat(ad1)], axis=1).astype(np.float32)
    W2cat = np.concatenate(
        [W2, W2 @ _amat(as2), W2 @ _amat(ad2)], axis=1).astype(np.float32)
    Wfold = (res_W.astype(np.float32) @ W1cat)
    bb = (res_b.astype(np.float32) @ W1cat)

    ident16 = np.eye(128, dtype=np.float16)

    # ---- launch A ----
    x_sl = np.zeros((nslots, IN), np.float32)
    x_sl[prep.slot_of] = x
    nc_a = _get(("A",), lambda: build_launch_a(cfg))
    in_a = []
    for c in range(cfg.ncores):
        xs = x_sl[c * cfg.slots:(c + 1) * cfg.slots]
        in_a.append(dict(
            xT=np.ascontiguousarray(xs.T).astype(np.float16),
            rw=res_W.astype(np.float16),
            rb=np.asarray(res_b, np.float32).reshape(128, 1),
            wfh=Wfold[:, 0:128].astype(np.float16),
            wfsd=Wfold[:, 128:136].astype(np.float16),
            ones1=np.ones((1, 128), np.float16),
            bbh=bb[0:128].reshape(1, 128).astype(np.float16),
            bbsd=bb[128:136].reshape(1, 8).astype(np.float16)))
    res_a = _run(nc_a, in_a, cfg, "A")

    hsd1_all = np.concatenate(
        [_slotify(res_a[c]["hsd1"], 136) for c in range(cfg.ncores)])
    h1_all, sd1_all = hsd1_all[:, 0:128], hsd1_all[:, 128:136]
    xp_dev = [res_a[c]["xpT"] for c in range(cfg.ncores)]

    # ---- launch B ----
    vsd1 = route_edge_arrays(prep, cfg, h1_all, sd1_all[:, 0:4],
                             sd1_all[:, 4:8])
    key_e = (cfg.SumT, tuple(g[2] for g in cfg.groups))
    nc_b = _get(("B", key_e), lambda: build_launch_edge(cfg, final=False))
    in_b = []
    for c in range(cfg.ncores):
        v, sdin = vsd1[c]
        in_b.append(dict(
            v=v, sdin=sdin, kT=k1.reshape(128, 1),
            cT=c1.reshape(128, 1), idf16=ident16,
            w2h=W2cat[:, 0:128].astype(np.float16),
            w2sd=W2cat[:, 128:136].astype(np.float16)))
    res_b_ = _run(nc_b, in_b, cfg, "B")

    hsd2_all = np.concatenate(
        [_slotify(res_b_[c]["hsd2"], 136) for c in range(cfg.ncores)])
    h2_all, sd2_all = hsd2_all[:, 0:128], hsd2_all[:, 128:136]

    # ---- launch C ----
    vsd2 = route_edge_arrays(prep, cfg, h2_all, sd2_all[:, 0:4],
                             sd2_all[:, 4:8])
    nc_c = _get(("C", key_e), lambda: build_launch_edge(cfg, final=True))
    Wc1k = (Wc1.astype(np.float32) * kc[None, :]).astype(np.float16)
    in_c = []
    for c in range(cfg.ncores):
        v, sdin = vsd2[c]
        in_c.append(dict(
            v=v, sdin=sdin, kT=k2.reshape(128, 1),
            cT=c2.reshape(128, 1), idf16=ident16,
            xpT=xp_dev[c], wc1=Wc1k, ccT=cc.reshape(64, 1),
            wc2=np.asarray(Wc2, np.float32).astype(np.float16),
            bc2r=np.asarray(bc2, np.float32).reshape(1, OUT).astype(np.float16),
            ones1=np.ones((1, 128), np.float16)))
    res_c = _run(nc_c, in_c, cfg, "C")

    fin_slots = np.concatenate(
        [_slotify(res_c[c]["fin"], 40) for c in range(cfg.ncores)])
    return np.ascontiguousarray(fin_slots[prep.slot_of]).astype(np.float32)


# revision 77
# speedup vs baseline: 1.3378x; 1.0024x over previous
"""Trainium2 Bass kernel for nn_GAT_Vanilla (2-layer GAT + BN/ELU + MLP head).

Strategy (8 NeuronCores, graph/data parallel, dst-major edge layout):
- Nodes sorted by in-degree and chopped into 128-node bins (degree
  homogeneous); bins dealt round-robin to the 8 cores so every core gets
  an identical block-T profile. Edges are laid dst-major: partition lane
  = dst slot, free dim = (j-th incoming edge, feature).
- 3 SPMD launches: A) per-node matmuls producing h1/s1/d1/x_p^T;
  B) layer-1 edge phase (softmax-weighted aggregation) + layer-2 node
  matmul; C) layer-2 edge phase + residual + MLP head + log_softmax.
- Between launches the host routes device-computed per-node values into
  per-edge arrays (pure indexing/gather, no math) - the halo exchange.
- On device per block group: e=s+d, leaky (Pool); exp duplicated x2
  (Act, keeps DVE packed-2x mode); one DVE multiply v*=ex; PE
  transpose-accumulate (matmul vs identity) scatter-sums into PSUM
  producing agg^T feat-major; 1/z and BN-scale ride a tiny head-expand
  matmul; BN bias + ELU fused into Act ops. fp16 data, f32 accumulate.

Self-contained: only needs numpy + the concourse/bass stack.
"""

import numpy as np

import concourse.bass as bass
import concourse.bacc as bacc
import concourse.tile as tile
from concourse import mybir
from concourse.bass_utils import run_bass_kernel_spmd

F32 = mybir.dt.float32
F16 = mybir.dt.float16

# ---- problem constants (hardcoded per harness contract) ----
N, E, IN, HD, NH, OUT = 100000, 800000, 128, 32, 4, 40
D = HD * NH  # 128
EPS_BN = 1e-5
SPAD = -300.0  # pad-edge s value -> leaky -> -60 -> exp == 0

NCORES = 8
NBLK = 98   # blocks (bins) per core; 8*98*128 = 100352 slots >= N
GB = 4      # blocks per group (uniform T within a group)

PROFILE = False
LAST_EXEC_NS = []


class Cfg:
    def __init__(self, n=N, e=E, ncores=NCORES, nblk=NBLK, gb=GB):
        self.n, self.e, self.ncores, self.nblk, self.gb = n, e, ncores, nblk, gb
        self.slots = nblk * 128
        # set by host_prep:
        self.Tpad = None    # per-block T (uniform within each group)
        self.offs = None    # per-block tile offset, offs[nblk] = SumT
        self.SumT = None
        self.groups = None  # list of (g0, nb, T)


class Prep:
    pass


# ----------------------------------------------------------------------------
# Host preprocessing: degree-sorted binning + per-edge index matrices
# ----------------------------------------------------------------------------

def host_prep(edge_index, cfg: Cfg):
    n, e = cfg.n, cfg.e
    ncores, nblk, gb = cfg.ncores, cfg.nblk, cfg.gb
    nbins = ncores * nblk
    nslots = nbins * 128
    src = np.concatenate([edge_index[0], np.arange(n)]).astype(np.int64)
    dst = np.concatenate([edge_index[1], np.arange(n)]).astype(np.int64)
    deg = np.bincount(dst, minlength=n)

    # degree-sorted consecutive bins of 128 nodes; bin k -> core k%ncores,
    # block position k//ncores, so all cores share one T profile.
    order = np.argsort(-deg, kind="stable")
    binrank = np.arange(n) // 128          # bin of i-th sorted node
    lane = np.arange(n) % 128
    core_of = np.empty(n, np.int64)
    gpos_of = np.empty(n, np.int64)
    lane_of = np.empty(n, np.int64)
    core_of[order] = binrank % ncores
    gpos_of[order] = binrank // ncores
    lane_of[order] = lane
    slot_of = core_of * (nblk * 128) + gpos_of * 128 + lane_of

    # per-bin max degree -> per-block-position profile -> group-pad
    degs_sorted = deg[order]
    nb_used = (n + 127) // 128
    Tbin = np.ones(nbins, np.int64)
    maxs = np.maximum.reduceat(degs_sorted, np.arange(0, n, 128))
    Tbin[:nb_used] = np.maximum(maxs, 1)
    Tprof = np.ones(nblk, np.int64)
    for g in range(nblk):
        lo, hi = g * ncores, min((g + 1) * ncores, nbins)
        Tprof[g] = max(1, Tbin[lo:hi].max())
    Tpad = Tprof.copy()
    groups = []
    g = 0
    while g < nblk:
        nb = min(gb, nblk - g)
        T = int(Tprof[g:g + nb].max())
        Tpad[g:g + nb] = T
        groups.append((g, nb, T))
        g += nb
    offs = np.zeros(nblk + 1, np.int64)
    np.cumsum(Tpad, out=offs[1:])
    SumT = int(offs[nblk])
    cfg.Tpad, cfg.offs, cfg.SumT, cfg.groups = Tpad, offs, SumT, groups

    # edge -> (core, column, lane) in dst-major layout
    dslot = slot_of[dst]
    eorder = np.argsort(dslot, kind="stable")
    src_s, dslot_s = src[eorder], dslot[eorder]
    # j = rank within dst
    uniq, starts_idx = np.unique(dslot_s, return_index=True)
    j_of = np.arange(e + n, dtype=np.int64)
    j_of -= np.repeat(starts_idx, np.diff(np.append(starts_idx, e + n)))
    ecore = dslot_s // (nblk * 128)
    egpos = (dslot_s // 128) % nblk
    elane = dslot_s % 128
    ecol = offs[egpos] + j_of

    # per-core index matrices [128, SumT]:
    #   Iv: row into h_ext (nslots -> zero row)
    #   Is: row into s_ext (nslots -> SPAD row, nslots+1 -> zero row)
    prep = Prep()
    prep.Iv, prep.Is = [], []
    for c in range(ncores):
        iv = np.full((128, SumT), nslots, np.int64)
        is_ = np.full((128, SumT), nslots, np.int64)
        m = ecore == c
        iv[elane[m], ecol[m]] = slot_of[src_s[m]]
        is_[elane[m], ecol[m]] = slot_of[src_s[m]]
        prep.Iv.append(iv)
        prep.Is.append(is_)
    # dummy slots (no real node): one self edge with v=0, s=0 at j=0
    have = np.zeros(nslots, bool)
    have[slot_of] = True
    dumm = np.nonzero(~have)[0]
    dc = dumm // (nblk * 128)
    dg = (dumm // 128) % nblk
    dl = dumm % 128
    for c in range(ncores):
        m = dc == c
        prep.Is[c][dl[m], offs[dg[m]]] = nslots + 1  # s = 0
        # Iv stays nslots (v = 0)

    # own-node slot grid for d routing [128, nblk]
    g_grid = np.broadcast_to(np.arange(nblk)[None, :], (128, nblk))
    l_grid = np.broadcast_to(np.arange(128)[:, None], (128, nblk))
    prep.dgrid = [c * nblk * 128 + g_grid * 128 + l_grid for c in range(ncores)]

    prep.slot_of = slot_of
    prep.cfg = cfg
    return prep


def route_edge_arrays(prep, cfg, h_all16, s_all16, d_all16):
    """h_all16 [nslots,128] f16, s_all16 [nslots,4] f16, d_all16 [nslots,4].
    Returns per-core (v [128,SumT,128], sdin [128,SumT+nblk,4]) where sdin
    interleaves per group: [s cols of its blocks | d cols of its blocks]."""
    h_ext = np.vstack([h_all16, np.zeros((1, 128), np.float16)])
    s_ext = np.vstack([s_all16,
                       np.full((1, 4), SPAD, np.float16),
                       np.zeros((1, 4), np.float16)])
    out = []
    for c in range(cfg.ncores):
        v = h_ext[prep.Iv[c]]
        s = s_ext[prep.Is[c]]
        d = d_all16[prep.dgrid[c]]
        sdin = np.zeros((128, cfg.SumT + cfg.nblk, 4), np.float16)
        for (g0, nb, T) in cfg.groups:
            off = int(cfg.offs[g0])
            nt = nb * T
            base = off + g0
            sdin[:, base:base + nt] = s[:, off:off + nt]
            sdin[:, base + nt:base + nt + nb] = d[:, g0:g0 + nb]
        out.append((v, sdin))
    return out


# ----------------------------------------------------------------------------
# Device kernels
# ----------------------------------------------------------------------------

def build_launch_a(cfg: Cfg):
    """xpT = res_W^T x^T + res_b ; [h1|s1|d1] = x @ (res_W@W1cat) + res_b@W1cat."""
    nc = bacc.Bacc("TRN2", target_bir_lowering=False, debug=False,
                   num_devices=cfg.ncores)
    nblk = cfg.nblk
    xT = nc.dram_tensor("xT", [128, cfg.slots], F16, kind="ExternalInput").ap()
    rw = nc.dram_tensor("rw", [128, 128], F16, kind="ExternalInput").ap()
    rb = nc.dram_tensor("rb", [128, 1], F32, kind="ExternalInput").ap()
    wfh = nc.dram_tensor("wfh", [128, 128], F16, kind="ExternalInput").ap()
    wfsd = nc.dram_tensor("wfsd", [128, 8], F16, kind="ExternalInput").ap()
    ones1 = nc.dram_tensor("ones1", [1, 128], F16, kind="ExternalInput").ap()
    bbh = nc.dram_tensor("bbh", [1, 128], F16, kind="ExternalInput").ap()
    bbsd = nc.dram_tensor("bbsd", [1, 8], F16, kind="ExternalInput").ap()
    hsd1 = nc.dram_tensor("hsd1", [128, nblk, 136], F16,
                          kind="ExternalOutput").ap()
    xpT = nc.dram_tensor("xpT", [128, nblk, 128], F16,
                         kind="ExternalOutput").ap()

    CHA = 8  # blocks per IO chunk (two PSUM sub-groups of 4)
    with tile.TileContext(nc) as tc:
        with (
            tc.tile_pool(name="const", bufs=1) as cp,
            tc.tile_pool(name="io", bufs=3) as iop,
            tc.tile_pool(name="psx", bufs=2, space="PSUM") as psx_p,
            tc.tile_pool(name="psh", bufs=2, space="PSUM") as psh_p,
            tc.tile_pool(name="pss", bufs=2, space="PSUM") as pss_p,
        ):
            rw_t = cp.tile([128, 128], F16)
            nc.sync.dma_start(out=rw_t[:], in_=rw)
            rb_t = cp.tile([128, 1], F32)
            nc.sync.dma_start(out=rb_t[:], in_=rb)
            wfh_t = cp.tile([128, 128], F16)
            nc.sync.dma_start(out=wfh_t[:], in_=wfh)
            wfsd_t = cp.tile([128, 8], F16)
            nc.sync.dma_start(out=wfsd_t[:], in_=wfsd)
            on_t = cp.tile([1, 128], F16)
            nc.sync.dma_start(out=on_t[:], in_=ones1)
            bbh_t = cp.tile([1, 128], F16)
            nc.sync.dma_start(out=bbh_t[:], in_=bbh)
            bbsd_t = cp.tile([1, 8], F16)
            nc.sync.dma_start(out=bbsd_t[:], in_=bbsd)

            def a_front(c0):
                nch = min(CHA, nblk - c0)
                xt = iop.tile([128, CHA, 128], F16, tag="xt")
                nc.sync.dma_start(
                    out=xt[:, 0:nch, :],
                    in_=xT[:, c0 * 128:(c0 + nch) * 128].rearrange(
                        "p (b q) -> p b q", b=nch))
                subs = []
                for i0 in range(0, nch, GB):
                    nb = min(GB, nch - i0)
                    psx = psx_p.tile([128, GB, 128], F32, tag="x")
                    psh = psh_p.tile([128, GB, 128], F32, tag="h")
                    pss = pss_p.tile([128, GB, 8], F32, tag="s")
                    for b in range(nb):
                        bi = i0 + b
                        nc.tensor.matmul(out=psx[:, b, :], lhsT=rw_t[:],
                                         rhs=xt[:, bi, :], start=True,
                                         stop=True)
                        nc.tensor.matmul(out=psh[:, b, :], lhsT=xt[:, bi, :],
                                         rhs=wfh_t[:], start=True, stop=False)
                        nc.tensor.matmul(out=psh[:, b, :], lhsT=on_t[:],
                                         rhs=bbh_t[:], start=False, stop=True)
                        nc.tensor.matmul(out=pss[:, b, :], lhsT=xt[:, bi, :],
                                         rhs=wfsd_t[:], start=True,
                                         stop=False)
                        nc.tensor.matmul(out=pss[:, b, :], lhsT=on_t[:],
                                         rhs=bbsd_t[:], start=False,
                                         stop=True)
                    subs.append((i0, nb, psx, psh, pss))
                return (c0, nch, subs)

            def a_tail(ctx):
                c0, nch, subs = ctx
                xpc = iop.tile([128, CHA, 128], F16, tag="xpc")
                hsdc = iop.tile([128, CHA, 136], F16, tag="hsdc")
                for (i0, nb, psx, psh, pss) in subs:
                    nc.scalar.activation(
                        out=xpc[:, i0:i0 + nb, :], in_=psx[:, 0:nb, :],
                        func=mybir.ActivationFunctionType.Identity,
                        bias=rb_t[:])
                    nc.vector.tensor_copy(hsdc[:, i0:i0 + nb, 0:128],
                                          psh[:, 0:nb, :])
                    nc.vector.tensor_copy(hsdc[:, i0:i0 + nb, 128:136],
                                          pss[:, 0:nb, :])
                nc.gpsimd.dma_start(out=xpT[:, c0:c0 + nch, :],
                                    in_=xpc[:, 0:nch, :])
                nc.scalar.dma_start(out=hsd1[:, c0:c0 + nch, :],
                                    in_=hsdc[:, 0:nch, :])

            a_pend = []
            for c0 in range(0, nblk, CHA):
                a_pend.append(a_front(c0))
                if len(a_pend) > 1:
                    a_tail(a_pend.pop(0))
            while a_pend:
                a_tail(a_pend.pop(0))
    nc.compile()
    return nc


def build_launch_edge(cfg: Cfg, final: bool):
    """Edge phase (softmax-weighted aggregation, BN+ELU) for one layer.
    final=False: + layer-2 node matmul (h2/sd2 out).
    final=True:  + residual + MLP head + log_softmax (fin out)."""
    nc = bacc.Bacc("TRN2", target_bir_lowering=False, debug=False,
                   num_devices=cfg.ncores)
    nblk, SumT = cfg.nblk, cfg.SumT
    GT = max(g[1] * g[2] for g in cfg.groups)

    v_ap = nc.dram_tensor("v", [128, SumT, 128], F16, kind="ExternalInput").ap()
    sdin = nc.dram_tensor("sdin", [128, SumT + nblk, 4], F16,
                          kind="ExternalInput").ap()
    kT = nc.dram_tensor("kT", [128, 1], F32, kind="ExternalInput").ap()
    cT = nc.dram_tensor("cT", [128, 1], F32, kind="ExternalInput").ap()
    idf16 = nc.dram_tensor("idf16", [128, 128], F16, kind="ExternalInput").ap()
    if not final:
        w2h = nc.dram_tensor("w2h", [128, 128], F16, kind="ExternalInput").ap()
        w2sd = nc.dram_tensor("w2sd", [128, 8], F16, kind="ExternalInput").ap()
        hsd2 = nc.dram_tensor("hsd2", [128, nblk, 136], F16,
                              kind="ExternalOutput").ap()
    else:
        xpT = nc.dram_tensor("xpT", [128, nblk, 128], F16,
                             kind="ExternalInput").ap()
        wc1 = nc.dram_tensor("wc1", [128, 64], F16, kind="ExternalInput").ap()
        ccT = nc.dram_tensor("ccT", [64, 1], F32, kind="ExternalInput").ap()
        wc2 = nc.dram_tensor("wc2", [64, 40], F16, kind="ExternalInput").ap()
        bc2r = nc.dram_tensor("bc2r", [1, 40], F16, kind="ExternalInput").ap()
        ones1 = nc.dram_tensor("ones1", [1, 128], F16,
                               kind="ExternalInput").ap()
        fin = nc.dram_tensor("fin", [128, nblk, 40], F32,
                             kind="ExternalOutput").ap()

    EXP = mybir.ActivationFunctionType.Exp
    RELU = mybir.ActivationFunctionType.Relu
    with tile.TileContext(nc) as tc:
        with (
            tc.tile_pool(name="const", bufs=1) as cp,
            tc.tile_pool(name="vp", bufs=3) as vp,
            tc.tile_pool(name="sp", bufs=4) as sp,
            tc.tile_pool(name="up", bufs=4) as up,
            tc.tile_pool(name="psagg", bufs=3, space="PSUM") as psagg_p,
            tc.tile_pool(name="psh", bufs=2, space="PSUM") as psh_p,
            tc.tile_pool(name="pssd", bufs=2, space="PSUM") as pssd_p,
            tc.tile_pool(name="psprt", bufs=2, space="PSUM") as psprt_p,
        ):
            kT_t = cp.tile([128, 1], F32)
            nc.sync.dma_start(out=kT_t[:], in_=kT)
            cT_t = cp.tile([128, 1], F32)
            nc.sync.dma_start(out=cT_t[:], in_=cT)
            idf16_t = cp.tile([128, 128], F16)
            nc.sync.dma_start(out=idf16_t[:], in_=idf16)
            if not final:
                w2h_t = cp.tile([128, 128], F16)
                nc.sync.dma_start(out=w2h_t[:], in_=w2h)
                w2sd_t = cp.tile([128, 8], F16)
                nc.sync.dma_start(out=w2sd_t[:], in_=w2sd)
            else:
                wc1_t = cp.tile([128, 64], F16)
                nc.sync.dma_start(out=wc1_t[:], in_=wc1)
                ccT_t = cp.tile([64, 1], F32)
                nc.sync.dma_start(out=ccT_t[:], in_=ccT)
                wc2_t = cp.tile([64, 40], F16)
                nc.sync.dma_start(out=wc2_t[:], in_=wc2)
                bc2r_t = cp.tile([1, 40], F16)
                nc.sync.dma_start(out=bc2r_t[:], in_=bc2r)
                on_t = cp.tile([1, 128], F16)
                nc.sync.dma_start(out=on_t[:], in_=ones1)
                ysb = cp.tile([128, nblk, 40], F32)
                eyt = cp.tile([128, nblk, 40], F16)
                zs = cp.tile([128, nblk], F32)

            def frontA(grp):
                """DMA in + softmax chain through alpha."""
                g0, nb, T = grp
                off = int(cfg.offs[g0])
                nt = nb * T
                sdt = sp.tile([128, GT + GB, 4], F16, tag="sd")
                vt = vp.tile([128, GT, 128], F16, tag="v")
                if not final:  # B: small sdt ahead of the big v transfer
                    nc.sync.dma_start(
                        out=sdt[:, 0:nt + nb, :],
                        in_=sdin[:, off + g0:off + g0 + nt + nb, :])
                    nc.sync.dma_start(out=vt[:, 0:nt, :],
                                      in_=v_ap[:, off:off + nt, :])
                    xpt = None
                else:
                    nc.sync.dma_start(out=vt[:, 0:nt, :],
                                      in_=v_ap[:, off:off + nt, :])
                    nc.sync.dma_start(
                        out=sdt[:, 0:nt + nb, :],
                        in_=sdin[:, off + g0:off + g0 + nt + nb, :])
                    xpt = up.tile([128, GB, 128], F16, tag="xpt")
                    nc.sync.dma_start(out=xpt[:, 0:nb, :],
                                      in_=xpT[:, g0:g0 + nb, :])
                st = sdt[:, 0:nt, :]
                dt_ = sdt[:, nt:nt + nb, :]

                # e = leaky(s + d)  [p, (b j) h] fp16
                et = sp.tile([128, GT, 4], F16, tag="e")
                e_bjh = et[:, 0:nt, :].rearrange("p (b j) h -> p b j h", b=nb)
                d_b = dt_.unsqueeze(2).to_broadcast([128, nb, T, 4])
                s_bjh = st.rearrange("p (b j) h -> p b j h", b=nb)
                nc.gpsimd.tensor_tensor(out=e_bjh, in0=s_bjh, in1=d_b,
                                        op=mybir.AluOpType.add)
                e_flat = et[:, 0:nt, :].rearrange("p a h -> p (a h)")
                e2t = sp.tile([128, GT, 4], F16, tag="e2")
                e2_flat = e2t[:, 0:nt, :].rearrange("p a h -> p (a h)")
                nc.vector.tensor_scalar(out=e2_flat, in0=e_flat, scalar1=0.2,
                                        scalar2=None, op0=mybir.AluOpType.mult)
                nc.vector.tensor_tensor(out=e_flat, in0=e_flat, in1=e2_flat,
                                        op=mybir.AluOpType.max)

                # ex2 = exp(e) duplicated x2 (Act); ez = exp(e) (Act)
                ex2t = sp.tile([128, GT * 8], F16, tag="ex2")
                ex2_v = ex2t[:, 0:nt * 8].rearrange("p (a two) -> p a two",
                                                    two=2)
                e_b2 = e_flat.unsqueeze(-1).to_broadcast([128, nt * 4, 2])
                nc.scalar.activation(out=ex2_v, in_=e_b2, func=EXP)
                ezt = sp.tile([128, GT * 4], F16, tag="ez")
                nc.scalar.activation(out=ezt[:, 0:nt * 4], in_=e_flat,
                                     func=EXP)
                # z[p, b, h] = sum_j ez ; zr = 1/z
                zt = sp.tile([128, GB, 4], F32, tag="z")
                ez_bhj = ezt[:, 0:nt * 4].rearrange(
                    "p (b j h) -> p b h j", b=nb, j=T, h=4)
                nc.vector.tensor_reduce(out=zt[:, 0:nb, :], in_=ez_bhj,
                                        axis=mybir.AxisListType.X,
                                        op=mybir.AluOpType.add)
                zrt = sp.tile([128, GB * 4], F32, tag="zr")
                nc.vector.reciprocal(
                    zrt[:, 0:nb * 4],
                    zt[:, 0:nb, :].rearrange("p b h -> p (b h)"))
                # alpha = ex / z  (per block; zr broadcast over j and pair)
                for b in range(nb):
                    ex2_blk = ex2t[:, b * T * 8:(b + 1) * T * 8].rearrange(
                        "p (j h two) -> p j h two", j=T, h=4, two=2)
                    zr_b = zrt[:, b * 4:(b + 1) * 4].unsqueeze(1)\
                        .unsqueeze(-1).to_broadcast([128, T, 4, 2])
                    nc.gpsimd.tensor_tensor(out=ex2_blk, in0=ex2_blk,
                                            in1=zr_b,
                                            op=mybir.AluOpType.mult)
                return (g0, nb, T, nt, vt, ex2t, xpt)

            def frontB(actx):
                """w = v*alpha (DVE) + PE transpose-accumulate."""
                g0, nb, T, nt, vt, ex2t, xpt = actx
                ex2_b = ex2t[:, 0:nt * 8].rearrange(
                    "p (a two) -> p a two", two=2).unsqueeze(2).to_broadcast(
                    [128, nt * 4, 16, 2])
                w_m = vt[:, 0:nt, :].rearrange(
                    "p bj (h c16 two) -> p (bj h) c16 two", h=4, c16=16,
                    two=2)
                nc.vector.tensor_tensor(out=w_m, in0=w_m, in1=ex2_b,
                                        op=mybir.AluOpType.mult)

                # aggT[feat, b, dst] += w_j^T  (PE transpose-accumulate)
                psagg = psagg_p.tile([128, GB, 128], F32, tag="agg")
                for b in range(nb):
                    for j in range(T):
                        nc.tensor.matmul(out=psagg[:, b, :],
                                         lhsT=vt[:, b * T + j, :],
                                         rhs=idf16_t[:],
                                         start=(j == 0), stop=(j == T - 1))
                return (g0, nb, psagg, xpt)

            def tail1(ctx):
                """BN+ELU: g=exp(k*agg+c), t1=relu(k*agg+c), u=min(g-1,t1)."""
                g0, nb, psagg, xpt = ctx
                gt = up.tile([128, GB, 128], F16, tag="g")
                nc.scalar.activation(out=gt[:, 0:nb, :],
                                     in_=psagg[:, 0:nb, :],
                                     func=EXP, bias=cT_t[:], scale=kT_t[:])
                t1t = up.tile([128, GB, 128], F16, tag="t1")
                nc.scalar.activation(out=t1t[:, 0:nb, :],
                                     in_=psagg[:, 0:nb, :],
                                     func=RELU, bias=cT_t[:], scale=kT_t[:])
                ut = up.tile([128, GB, 128], F16, tag="u")
                nc.vector.scalar_tensor_tensor(
                    out=ut[:, 0:nb, :], in0=gt[:, 0:nb, :], scalar=-1.0,
                    in1=t1t[:, 0:nb, :],
                    op0=mybir.AluOpType.add, op1=mybir.AluOpType.min)
                return (g0, nb, ut, xpt)

            def tail2(ctx):
                """Node matmul / MLP head + output."""
                g0, nb, ut, xpt = ctx
                if not final:
                    psh = psh_p.tile([128, GB, 128], F32, tag="h2")
                    pss = pssd_p.tile([128, GB, 8], F32, tag="sd2")
                    for b in range(nb):
                        nc.tensor.matmul(out=psh[:, b, :], lhsT=ut[:, b, :],
                                         rhs=w2h_t[:], start=True, stop=True)
                        nc.tensor.matmul(out=pss[:, b, :], lhsT=ut[:, b, :],
                                         rhs=w2sd_t[:], start=True, stop=True)
                    hsdc = up.tile([128, GB, 136], F16, tag="hsdc")
                    nc.scalar.copy(hsdc[:, 0:nb, 0:128], psh[:, 0:nb, :])
                    nc.scalar.copy(hsdc[:, 0:nb, 128:136],
                                   pss[:, 0:nb, :])
                    nc.scalar.dma_start(out=hsd2[:, g0:g0 + nb, :],
                                        in_=hsdc[:, 0:nb, :])
                else:
                    # MLP head on h = u + x_p, with the residual folded into
                    # the matmul: prT = Wc1k^T @ u + Wc1k^T @ xpT
                    psprT = psprt_p.tile([128, GB, 128], F32, tag="prT")
                    for b in range(nb):
                        nc.tensor.matmul(out=psprT[0:64, b, :],
                                         lhsT=wc1_t[:], rhs=ut[:, b, :],
                                         start=True, stop=False)
                        nc.tensor.matmul(out=psprT[0:64, b, :],
                                         lhsT=wc1_t[:], rhs=xpt[:, b, :],
                                         start=False, stop=True)
                    r1sb = up.tile([64, GB, 128], F16, tag="r1")
                    nc.scalar.activation(out=r1sb[0:64, 0:nb, :],
                                         in_=psprT[0:64, 0:nb, :],
                                         func=RELU, bias=ccT_t[:])
                    psy = pssd_p.tile([128, GB, 40], F32, tag="y")
                    for b in range(nb):
                        nc.tensor.matmul(out=psy[:, b, :],
                                         lhsT=r1sb[:, b, :],
                                         rhs=wc2_t[:], start=True, stop=False)
                        nc.tensor.matmul(out=psy[:, b, :], lhsT=on_t[:],
                                         rhs=bc2r_t[:], start=False,
                                         stop=True)
                    nc.scalar.copy(ysb[:, g0:g0 + nb, :], psy[:, 0:nb, :])
                    nc.scalar.activation(
                        out=eyt[:, g0:g0 + nb, :], in_=psy[:, 0:nb, :],
                        func=EXP)
                    nc.vector.tensor_reduce(
                        out=zs[:, g0:g0 + nb], in_=eyt[:, g0:g0 + nb, :],
                        axis=mybir.AxisListType.X, op=mybir.AluOpType.add)

            # software pipeline: frontB(i-1) | frontA(i) | tail(i-2)
            a_pend = []
            b_pend = []
            for grp in cfg.groups:
                if a_pend:
                    b_pend.append(frontB(a_pend.pop(0)))
                a_pend.append(frontA(grp))
                if len(b_pend) > 1:
                    tail2(tail1(b_pend.pop(0)))
            while a_pend:
                b_pend.append(frontB(a_pend.pop(0)))
            while b_pend:
                tail2(tail1(b_pend.pop(0)))

            if final:
                # log_softmax epilogue: one Ln, then subtract+DMA in halves
                lnz = cp.tile([128, nblk], F32)
                nc.scalar.activation(out=lnz[:], in_=zs[:],
                                     func=mybir.ActivationFunctionType.Ln)
                finsb = cp.tile([128, nblk, 40], F32)
                half = nblk // 2
                for lo, hi in ((0, half), (half, nblk)):
                    lnz_b = lnz[:, lo:hi].unsqueeze(-1).to_broadcast(
                        [128, hi - lo, 40])
                    nc.vector.tensor_tensor(out=finsb[:, lo:hi, :],
                                            in0=ysb[:, lo:hi, :], in1=lnz_b,
                                            op=mybir.AluOpType.subtract)
                    nc.sync.dma_start(out=fin[:, lo:hi, :],
                                      in_=finsb[:, lo:hi, :])
    nc.compile()
    return nc


# ----------------------------------------------------------------------------
# Host orchestration
# ----------------------------------------------------------------------------

_cache = {}


def _get(key, fn):
    if key not in _cache:
        _cache[key] = fn()
    return _cache[key]


def _amat(a):
    m = np.zeros((D, NH), np.float32)
    for h in range(NH):
        m[h * HD:(h + 1) * HD, h] = a[h]
    return m


def _run(nc, in_maps, cfg, tag):
    res = run_bass_kernel_spmd(nc, in_maps, list(range(cfg.ncores)),
                               trace=PROFILE)
    if PROFILE:
        LAST_EXEC_NS.append((tag, res.exec_time_ns))
    return res.results


def _slotify(arr, cdim):
    """[128, nblk, c] device layout -> [slots, c] (slot = g*128 + lane)."""
    return arr.transpose(1, 0, 2).reshape(-1, cdim)


def kernel(x, edge_index, res_W, res_b,
           W1, as1, ad1, b1, g1, be1, rm1, rv1,
           W2, as2, ad2, b2, g2, be2, rm2, rv2,
           Wc1, bc1, gc, bec, rmc, rvc, Wc2, bc2,
           _cfg=None):
    cfg = _cfg or _get("cfg", lambda: Cfg())
    x = np.asarray(x, np.float32)
    edge_index = np.asarray(edge_index)
    (res_W, res_b, W1, as1, ad1, b1, g1, be1, rm1, rv1,
     W2, as2, ad2, b2, g2, be2, rm2, rv2,
     Wc1, bc1, gc, bec, rmc, rvc, Wc2, bc2) = (
        np.asarray(a, np.float32) for a in (
            res_W, res_b, W1, as1, ad1, b1, g1, be1, rm1, rv1,
            W2, as2, ad2, b2, g2, be2, rm2, rv2,
            Wc1, bc1, gc, bec, rmc, rvc, Wc2, bc2))

    ekey = ("prep", hash(edge_index.tobytes()))
    prep = _get(ekey, lambda: host_prep(np.asarray(edge_index, np.int64), cfg))
    nslots = cfg.ncores * cfg.slots

    def fold_bn(g_, be_, rm_, rv_, bias):
        k = (g_ / np.sqrt(rv_ + EPS_BN)).astype(np.float32)
        c = ((bias - rm_) * k + be_).astype(np.float32)
        return k, c

    k1, c1 = fold_bn(g1, be1, rm1, rv1, b1)
    k2, c2 = fold_bn(g2, be2, rm2, rv2, b2)
    kc, cc = fold_bn(gc, bec, rmc, rvc, bc1)

    W1cat = np.concatenate(
        [W1, W1 @ _amat(as1), W1 @ _amat(ad1)], axis=1).astype(np.float32)
    W2cat = np.concatenate(
        [W2, W2 @ _amat(as2), W2 @ _amat(ad2)], axis=1).astype(np.float32)
    Wfold = (res_W.astype(np.float32) @ W1cat)
    bb = (res_b.astype(np.float32) @ W1cat)

    ident16 = np.eye(128, dtype=np.float16)

    # ---- launch A ----
    x_sl = np.zeros((nslots, IN), np.float32)
    x_sl[prep.slot_of] = x
    nc_a = _get(("A",), lambda: build_launch_a(cfg))
    in_a = []
    for c in range(cfg.ncores):
        xs = x_sl[c * cfg.slots:(c + 1) * cfg.slots]
        in_a.append(dict(
            xT=np.ascontiguousarray(xs.T).astype(np.float16),
            rw=res_W.astype(np.float16),
            rb=np.asarray(res_b, np.float32).reshape(128, 1),
            wfh=Wfold[:, 0:128].astype(np.float16),
            wfsd=Wfold[:, 128:136].astype(np.float16),
            ones1=np.ones((1, 128), np.float16),
            bbh=bb[0:128].reshape(1, 128).astype(np.float16),
            bbsd=bb[128:136].reshape(1, 8).astype(np.float16)))
    res_a = _run(nc_a, in_a, cfg, "A")

    hsd1_all = np.concatenate(
        [_slotify(res_a[c]["hsd1"], 136) for c in range(cfg.ncores)])
    h1_all, sd1_all = hsd1_all[:, 0:128], hsd1_all[:, 128:136]
    xp_dev = [res_a[c]["xpT"] for c in range(cfg.ncores)]

    # ---- launch B ----
    vsd1 = route_edge_arrays(prep, cfg, h1_all, sd1_all[:, 0:4],
                             sd1_all[:, 4:8])
    key_e = (cfg.SumT, tuple(g[2] for g in cfg.groups))
    nc_b = _get(("B", key_e), lambda: build_launch_edge(cfg, final=False))
    in_b = []
    for c in range(cfg.ncores):
        v, sdin = vsd1[c]
        in_b.append(dict(
            v=v, sdin=sdin, kT=k1.reshape(128, 1),
            cT=c1.reshape(128, 1), idf16=ident16,
            w2h=W2cat[:, 0:128].astype(np.float16),
            w2sd=W2cat[:, 128:136].astype(np.float16)))
    res_b_ = _run(nc_b, in_b, cfg, "B")

    hsd2_all = np.concatenate(
        [_slotify(res_b_[c]["hsd2"], 136) for c in range(cfg.ncores)])
    h2_all, sd2_all = hsd2_all[:, 0:128], hsd2_all[:, 128:136]

    # ---- launch C ----
    vsd2 = route_edge_arrays(prep, cfg, h2_all, sd2_all[:, 0:4],
                             sd2_all[:, 4:8])
    nc_c = _get(("C", key_e), lambda: build_launch_edge(cfg, final=True))
    Wc1k = (Wc1.astype(np.float32) * kc[None, :]).astype(np.float16)
    in_c = []
    for c in range(cfg.ncores):
        v, sdin = vsd2[c]
        in_c.append(dict(
            v=v, sdin=sdin, kT=k2.reshape(128, 1),
            cT=c2.reshape(128, 1), idf16=ident16,
            xpT=xp_dev[c], wc1=Wc1k, ccT=cc.reshape(64, 1),
            wc2=np.asarray(Wc2, np.float32).astype(np.float16),
            bc2r=np.asarray(bc2, np.float32).reshape(1, OUT).astype(np.float16),
            ones1=np.ones((1, 128), np.float16)))
    res_c = _run(nc_c, in_c, cfg, "C")

    fin_slots = np.concatenate(
        [_slotify(res_c[c]["fin"], 40) for c in range(cfg.ncores)])
    return np.ascontiguousarray(fin_slots[prep.slot_of]).astype(np.float32)


# revision 79
# speedup vs baseline: 1.3415x; 1.0028x over previous
"""Trainium2 Bass kernel for nn_GAT_Vanilla (2-layer GAT + BN/ELU + MLP head).

Strategy (8 NeuronCores, graph/data parallel, dst-major edge layout):
- Nodes sorted by in-degree and chopped into 128-node bins (degree
  homogeneous); bins dealt round-robin to the 8 cores so every core gets
  an identical block-T profile. Edges are laid dst-major: partition lane
  = dst slot, free dim = (j-th incoming edge, feature).
- 3 SPMD launches: A) per-node matmuls producing h1/s1/d1/x_p^T;
  B) layer-1 edge phase (softmax-weighted aggregation) + layer-2 node
  matmul; C) layer-2 edge phase + residual + MLP head + log_softmax.
- Between launches the host routes device-computed per-node values into
  per-edge arrays (pure indexing/gather, no math) - the halo exchange.
- On device per block group: e=s+d, leaky (Pool); exp duplicated x2
  (Act, keeps DVE packed-2x mode); one DVE multiply v*=ex; PE
  transpose-accumulate (matmul vs identity) scatter-sums into PSUM
  producing agg^T feat-major; 1/z and BN-scale ride a tiny head-expand
  matmul; BN bias + ELU fused into Act ops. fp16 data, f32 accumulate.

Self-contained: only needs numpy + the concourse/bass stack.
"""

import numpy as np

import concourse.bass as bass
import concourse.bacc as bacc
import concourse.tile as tile
from concourse import mybir
from concourse.bass_utils import run_bass_kernel_spmd

F32 = mybir.dt.float32
F16 = mybir.dt.float16

# ---- problem constants (hardcoded per harness contract) ----
N, E, IN, HD, NH, OUT = 100000, 800000, 128, 32, 4, 40
D = HD * NH  # 128
EPS_BN = 1e-5
SPAD = -300.0  # pad-edge s value -> leaky -> -60 -> exp == 0

NCORES = 8
NBLK = 98   # blocks (bins) per core; 8*98*128 = 100352 slots >= N
GB = 4      # blocks per group (uniform T within a group)

PROFILE = False
LAST_EXEC_NS = []


class Cfg:
    def __init__(self, n=N, e=E, ncores=NCORES, nblk=NBLK, gb=GB):
        self.n, self.e, self.ncores, self.nblk, self.gb = n, e, ncores, nblk, gb
        self.slots = nblk * 128
        # set by host_prep:
        self.Tpad = None    # per-block T (uniform within each group)
        self.offs = None    # per-block tile offset, offs[nblk] = SumT
        self.SumT = None
        self.groups = None  # list of (g0, nb, T)


class Prep:
    pass


# ----------------------------------------------------------------------------
# Host preprocessing: degree-sorted binning + per-edge index matrices
# ----------------------------------------------------------------------------

def host_prep(edge_index, cfg: Cfg):
    n, e = cfg.n, cfg.e
    ncores, nblk, gb = cfg.ncores, cfg.nblk, cfg.gb
    nbins = ncores * nblk
    nslots = nbins * 128
    src = np.concatenate([edge_index[0], np.arange(n)]).astype(np.int64)
    dst = np.concatenate([edge_index[1], np.arange(n)]).astype(np.int64)
    deg = np.bincount(dst, minlength=n)

    # degree-sorted consecutive bins of 128 nodes; bin k -> core k%ncores,
    # block position k//ncores, so all cores share one T profile.
    order = np.argsort(-deg, kind="stable")
    binrank = np.arange(n) // 128          # bin of i-th sorted node
    lane = np.arange(n) % 128
    core_of = np.empty(n, np.int64)
    gpos_of = np.empty(n, np.int64)
    lane_of = np.empty(n, np.int64)
    core_of[order] = binrank % ncores
    gpos_of[order] = binrank // ncores
    lane_of[order] = lane
    slot_of = core_of * (nblk * 128) + gpos_of * 128 + lane_of

    # per-bin max degree -> per-block-position profile -> group-pad
    degs_sorted = deg[order]
    nb_used = (n + 127) // 128
    Tbin = np.ones(nbins, np.int64)
    maxs = np.maximum.reduceat(degs_sorted, np.arange(0, n, 128))
    Tbin[:nb_used] = np.maximum(maxs, 1)
    Tprof = np.ones(nblk, np.int64)
    for g in range(nblk):
        lo, hi = g * ncores, min((g + 1) * ncores, nbins)
        Tprof[g] = max(1, Tbin[lo:hi].max())
    Tpad = Tprof.copy()
    groups = []
    g = 0
    while g < nblk:
        nb = min(gb, nblk - g)
        T = int(Tprof[g:g + nb].max())
        Tpad[g:g + nb] = T
        groups.append((g, nb, T))
        g += nb
    offs = np.zeros(nblk + 1, np.int64)
    np.cumsum(Tpad, out=offs[1:])
    SumT = int(offs[nblk])
    cfg.Tpad, cfg.offs, cfg.SumT, cfg.groups = Tpad, offs, SumT, groups

    # edge -> (core, column, lane) in dst-major layout
    dslot = slot_of[dst]
    eorder = np.argsort(dslot, kind="stable")
    src_s, dslot_s = src[eorder], dslot[eorder]
    # j = rank within dst
    uniq, starts_idx = np.unique(dslot_s, return_index=True)
    j_of = np.arange(e + n, dtype=np.int64)
    j_of -= np.repeat(starts_idx, np.diff(np.append(starts_idx, e + n)))
    ecore = dslot_s // (nblk * 128)
    egpos = (dslot_s // 128) % nblk
    elane = dslot_s % 128
    ecol = offs[egpos] + j_of

    # per-core index matrices [128, SumT]:
    #   Iv: row into h_ext (nslots -> zero row)
    #   Is: row into s_ext (nslots -> SPAD row, nslots+1 -> zero row)
    prep = Prep()
    prep.Iv, prep.Is = [], []
    for c in range(ncores):
        iv = np.full((128, SumT), nslots, np.int64)
        is_ = np.full((128, SumT), nslots, np.int64)
        m = ecore == c
        iv[elane[m], ecol[m]] = slot_of[src_s[m]]
        is_[elane[m], ecol[m]] = slot_of[src_s[m]]
        prep.Iv.append(iv)
        prep.Is.append(is_)
    # dummy slots (no real node): one self edge with v=0, s=0 at j=0
    have = np.zeros(nslots, bool)
    have[slot_of] = True
    dumm = np.nonzero(~have)[0]
    dc = dumm // (nblk * 128)
    dg = (dumm // 128) % nblk
    dl = dumm % 128
    for c in range(ncores):
        m = dc == c
        prep.Is[c][dl[m], offs[dg[m]]] = nslots + 1  # s = 0
        # Iv stays nslots (v = 0)

    # own-node slot grid for d routing [128, nblk]
    g_grid = np.broadcast_to(np.arange(nblk)[None, :], (128, nblk))
    l_grid = np.broadcast_to(np.arange(128)[:, None], (128, nblk))
    prep.dgrid = [c * nblk * 128 + g_grid * 128 + l_grid for c in range(ncores)]

    prep.slot_of = slot_of
    prep.cfg = cfg
    return prep


def route_edge_arrays(prep, cfg, h_all16, s_all16, d_all16):
    """h_all16 [nslots,128] f16, s_all16 [nslots,4] f16, d_all16 [nslots,4].
    Returns per-core (v [128,SumT,128], sdin [128,SumT+nblk,4]) where sdin
    interleaves per group: [s cols of its blocks | d cols of its blocks]."""
    h_ext = np.vstack([h_all16, np.zeros((1, 128), np.float16)])
    s_ext = np.vstack([s_all16,
                       np.full((1, 4), SPAD, np.float16),
                       np.zeros((1, 4), np.float16)])
    out = []
    for c in range(cfg.ncores):
        v = h_ext[prep.Iv[c]]
        s = s_ext[prep.Is[c]]
        d = d_all16[prep.dgrid[c]]
        sdin = np.zeros((128, cfg.SumT + cfg.nblk, 4), np.float16)
        for (g0, nb, T) in cfg.groups:
            off = int(cfg.offs[g0])
            nt = nb * T
            base = off + g0
            sdin[:, base:base + nt] = s[:, off:off + nt]
            sdin[:, base + nt:base + nt + nb] = d[:, g0:g0 + nb]
        out.append((v, sdin))
    return out


# ----------------------------------------------------------------------------
# Device kernels
# ----------------------------------------------------------------------------

def build_launch_a(cfg: Cfg):
    """xpT = res_W^T x^T + res_b ; [h1|s1|d1] = x @ (res_W@W1cat) + res_b@W1cat."""
    nc = bacc.Bacc("TRN2", target_bir_lowering=False, debug=False,
                   num_devices=cfg.ncores)
    nblk = cfg.nblk
    xT = nc.dram_tensor("xT", [128, cfg.slots], F16, kind="ExternalInput").ap()
    rw = nc.dram_tensor("rw", [128, 128], F16, kind="ExternalInput").ap()
    rb = nc.dram_tensor("rb", [128, 1], F32, kind="ExternalInput").ap()
    wfh = nc.dram_tensor("wfh", [128, 128], F16, kind="ExternalInput").ap()
    wfsd = nc.dram_tensor("wfsd", [128, 8], F16, kind="ExternalInput").ap()
    ones1 = nc.dram_tensor("ones1", [1, 128], F16, kind="ExternalInput").ap()
    bbh = nc.dram_tensor("bbh", [1, 128], F16, kind="ExternalInput").ap()
    bbsd = nc.dram_tensor("bbsd", [1, 8], F16, kind="ExternalInput").ap()
    hsd1 = nc.dram_tensor("hsd1", [128, nblk, 136], F16,
                          kind="ExternalOutput").ap()
    xpT = nc.dram_tensor("xpT", [128, nblk, 128], F16,
                         kind="ExternalOutput").ap()

    CHA = 9   # blocks per IO chunk (three PSUM sub-groups of 3)
    GBA = 3   # blocks per PSUM sub-group ([128,3,136] f32 fits one bank)
    with tile.TileContext(nc) as tc:
        with (
            tc.tile_pool(name="const", bufs=1) as cp,
            tc.tile_pool(name="io", bufs=3) as iop,
            tc.tile_pool(name="psx", bufs=4, space="PSUM") as psx_p,
            tc.tile_pool(name="psh", bufs=4, space="PSUM") as psh_p,
        ):
            rw_t = cp.tile([128, 128], F16)
            nc.sync.dma_start(out=rw_t[:], in_=rw)
            rb_t = cp.tile([128, 1], F32)
            nc.sync.dma_start(out=rb_t[:], in_=rb)
            wfh_t = cp.tile([128, 128], F16)
            nc.sync.dma_start(out=wfh_t[:], in_=wfh)
            wfsd_t = cp.tile([128, 8], F16)
            nc.sync.dma_start(out=wfsd_t[:], in_=wfsd)
            on_t = cp.tile([1, 128], F16)
            nc.sync.dma_start(out=on_t[:], in_=ones1)
            bbh_t = cp.tile([1, 128], F16)
            nc.sync.dma_start(out=bbh_t[:], in_=bbh)
            bbsd_t = cp.tile([1, 8], F16)
            nc.sync.dma_start(out=bbsd_t[:], in_=bbsd)

            chunks = {}   # c0 -> [xpc, hsdc, nch, ndone]

            def a_sub(c0, i0, nb, xt):
                """matmuls for one 3-block sub-group."""
                psx = psx_p.tile([128, GBA, 128], F32, tag="x")
                psh = psh_p.tile([128, GBA, 136], F32, tag="h")
                for b in range(nb):
                    nc.tensor.matmul(out=psx[:, b, :], lhsT=rw_t[:],
                                     rhs=xt[:, i0 + b, :], start=True,
                                     stop=True)
                    nc.tensor.matmul(out=psh[:, b, 0:128],
                                     lhsT=xt[:, i0 + b, :],
                                     rhs=wfh_t[:], start=True, stop=False)
                    nc.tensor.matmul(out=psh[:, b, 0:128], lhsT=on_t[:],
                                     rhs=bbh_t[:], start=False, stop=True)
                    nc.tensor.matmul(out=psh[:, b, 128:136],
                                     lhsT=xt[:, i0 + b, :],
                                     rhs=wfsd_t[:], start=True, stop=False)
                    nc.tensor.matmul(out=psh[:, b, 128:136], lhsT=on_t[:],
                                     rhs=bbsd_t[:], start=False, stop=True)
                return (c0, i0, nb, psx, psh)

            def a_evac(ctx):
                """PSUM evacuation for one sub; out-DMA when chunk done."""
                c0, i0, nb, psx, psh = ctx
                xpc, hsdc, nch, _ = chunks[c0]
                nc.scalar.activation(
                    out=xpc[:, i0:i0 + nb, :], in_=psx[:, 0:nb, :],
                    func=mybir.ActivationFunctionType.Identity,
                    bias=rb_t[:])
                nc.vector.tensor_copy(hsdc[:, i0:i0 + nb, :],
                                      psh[:, 0:nb, :])
                chunks[c0][3] += nb
                if chunks[c0][3] == nch:
                    nc.gpsimd.dma_start(out=xpT[:, c0:c0 + nch, :],
                                        in_=xpc[:, 0:nch, :])
                    nc.scalar.dma_start(out=hsd1[:, c0:c0 + nch, :],
                                        in_=hsdc[:, 0:nch, :])
                    del chunks[c0]

            a_pend = []
            for c0 in range(0, nblk, CHA):
                nch = min(CHA, nblk - c0)
                xt = iop.tile([128, CHA, 128], F16, tag="xt")
                nc.sync.dma_start(
                    out=xt[:, 0:nch, :],
                    in_=xT[:, c0 * 128:(c0 + nch) * 128].rearrange(
                        "p (b q) -> p b q", b=nch))
                xpc_t = iop.tile([128, CHA, 128], F16, tag="xpc")
                hsdc_t = iop.tile([128, CHA, 136], F16, tag="hsdc")
                chunks[c0] = [xpc_t, hsdc_t, nch, 0]
                for i0 in range(0, nch, GBA):
                    nb = min(GBA, nch - i0)
                    a_pend.append(a_sub(c0, i0, nb, xt))
                    if len(a_pend) > 2:
                        a_evac(a_pend.pop(0))
            while a_pend:
                a_evac(a_pend.pop(0))
    nc.compile()
    return nc


def build_launch_edge(cfg: Cfg, final: bool):
    """Edge phase (softmax-weighted aggregation, BN+ELU) for one layer.
    final=False: + layer-2 node matmul (h2/sd2 out).
    final=True:  + residual + MLP head + log_softmax (fin out)."""
    nc = bacc.Bacc("TRN2", target_bir_lowering=False, debug=False,
                   num_devices=cfg.ncores)
    nblk, SumT = cfg.nblk, cfg.SumT
    GT = max(g[1] * g[2] for g in cfg.groups)

    v_ap = nc.dram_tensor("v", [128, SumT, 128], F16, kind="ExternalInput").ap()
    sdin = nc.dram_tensor("sdin", [128, SumT + nblk, 4], F16,
                          kind="ExternalInput").ap()
    kT = nc.dram_tensor("kT", [128, 1], F32, kind="ExternalInput").ap()
    cT = nc.dram_tensor("cT", [128, 1], F32, kind="ExternalInput").ap()
    idf16 = nc.dram_tensor("idf16", [128, 128], F16, kind="ExternalInput").ap()
    if not final:
        w2h = nc.dram_tensor("w2h", [128, 128], F16, kind="ExternalInput").ap()
        w2sd = nc.dram_tensor("w2sd", [128, 8], F16, kind="ExternalInput").ap()
        hsd2 = nc.dram_tensor("hsd2", [128, nblk, 136], F16,
                              kind="ExternalOutput").ap()
    else:
        xpT = nc.dram_tensor("xpT", [128, nblk, 128], F16,
                             kind="ExternalInput").ap()
        wc1 = nc.dram_tensor("wc1", [128, 64], F16, kind="ExternalInput").ap()
        ccT = nc.dram_tensor("ccT", [64, 1], F32, kind="ExternalInput").ap()
        wc2 = nc.dram_tensor("wc2", [64, 40], F16, kind="ExternalInput").ap()
        bc2r = nc.dram_tensor("bc2r", [1, 40], F16, kind="ExternalInput").ap()
        ones1 = nc.dram_tensor("ones1", [1, 128], F16,
                               kind="ExternalInput").ap()
        fin = nc.dram_tensor("fin", [128, nblk, 40], F32,
                             kind="ExternalOutput").ap()

    EXP = mybir.ActivationFunctionType.Exp
    RELU = mybir.ActivationFunctionType.Relu
    with tile.TileContext(nc) as tc:
        with (
            tc.tile_pool(name="const", bufs=1) as cp,
            tc.tile_pool(name="vp", bufs=3) as vp,
            tc.tile_pool(name="sp", bufs=4) as sp,
            tc.tile_pool(name="up", bufs=4) as up,
            tc.tile_pool(name="psagg", bufs=3, space="PSUM") as psagg_p,
            tc.tile_pool(name="psh", bufs=2, space="PSUM") as psh_p,
            tc.tile_pool(name="pssd", bufs=2, space="PSUM") as pssd_p,
            tc.tile_pool(name="psprt", bufs=2, space="PSUM") as psprt_p,
        ):
            kT_t = cp.tile([128, 1], F32)
            nc.sync.dma_start(out=kT_t[:], in_=kT)
            cT_t = cp.tile([128, 1], F32)
            nc.sync.dma_start(out=cT_t[:], in_=cT)
            idf16_t = cp.tile([128, 128], F16)
            nc.sync.dma_start(out=idf16_t[:], in_=idf16)
            if not final:
                w2h_t = cp.tile([128, 128], F16)
                nc.sync.dma_start(out=w2h_t[:], in_=w2h)
                w2sd_t = cp.tile([128, 8], F16)
                nc.sync.dma_start(out=w2sd_t[:], in_=w2sd)
            else:
                wc1_t = cp.tile([128, 64], F16)
                nc.sync.dma_start(out=wc1_t[:], in_=wc1)
                ccT_t = cp.tile([64, 1], F32)
                nc.sync.dma_start(out=ccT_t[:], in_=ccT)
                wc2_t = cp.tile([64, 40], F16)
                nc.sync.dma_start(out=wc2_t[:], in_=wc2)
                bc2r_t = cp.tile([1, 40], F16)
                nc.sync.dma_start(out=bc2r_t[:], in_=bc2r)
                on_t = cp.tile([1, 128], F16)
                nc.sync.dma_start(out=on_t[:], in_=ones1)
                ysb = cp.tile([128, nblk, 40], F32)
                eyt = cp.tile([128, nblk, 40], F16)
                zs = cp.tile([128, nblk], F32)

            def frontA(grp):
                """DMA in + softmax chain through alpha."""
                g0, nb, T = grp
                off = int(cfg.offs[g0])
                nt = nb * T
                sdt = sp.tile([128, GT + GB, 4], F16, tag="sd")
                vt = vp.tile([128, GT, 128], F16, tag="v")
                if not final:  # B: small sdt ahead of the big v transfer
                    nc.sync.dma_start(
                        out=sdt[:, 0:nt + nb, :],
                        in_=sdin[:, off + g0:off + g0 + nt + nb, :])
                    nc.sync.dma_start(out=vt[:, 0:nt, :],
                                      in_=v_ap[:, off:off + nt, :])
                    xpt = None
                else:
                    nc.sync.dma_start(out=vt[:, 0:nt, :],
                                      in_=v_ap[:, off:off + nt, :])
                    nc.sync.dma_start(
                        out=sdt[:, 0:nt + nb, :],
                        in_=sdin[:, off + g0:off + g0 + nt + nb, :])
                    xpt = up.tile([128, GB, 128], F16, tag="xpt")
                    nc.sync.dma_start(out=xpt[:, 0:nb, :],
                                      in_=xpT[:, g0:g0 + nb, :])
                st = sdt[:, 0:nt, :]
                dt_ = sdt[:, nt:nt + nb, :]

                # e = leaky(s + d)  [p, (b j) h] fp16
                et = sp.tile([128, GT, 4], F16, tag="e")
                e_bjh = et[:, 0:nt, :].rearrange("p (b j) h -> p b j h", b=nb)
                d_b = dt_.unsqueeze(2).to_broadcast([128, nb, T, 4])
                s_bjh = st.rearrange("p (b j) h -> p b j h", b=nb)
                nc.gpsimd.tensor_tensor(out=e_bjh, in0=s_bjh, in1=d_b,
                                        op=mybir.AluOpType.add)
                e_flat = et[:, 0:nt, :].rearrange("p a h -> p (a h)")
                e2t = sp.tile([128, GT, 4], F16, tag="e2")
                e2_flat = e2t[:, 0:nt, :].rearrange("p a h -> p (a h)")
                nc.vector.tensor_scalar(out=e2_flat, in0=e_flat, scalar1=0.2,
                                        scalar2=None, op0=mybir.AluOpType.mult)
                nc.vector.tensor_tensor(out=e_flat, in0=e_flat, in1=e2_flat,
                                        op=mybir.AluOpType.max)

                # ex2 = exp(e) duplicated x2 (Act); ez = exp(e) (Act)
                ex2t = sp.tile([128, GT * 8], F16, tag="ex2")
                ex2_v = ex2t[:, 0:nt * 8].rearrange("p (a two) -> p a two",
                                                    two=2)
                e_b2 = e_flat.unsqueeze(-1).to_broadcast([128, nt * 4, 2])
                nc.scalar.activation(out=ex2_v, in_=e_b2, func=EXP)
                ezt = sp.tile([128, GT * 4], F16, tag="ez")
                nc.scalar.activation(out=ezt[:, 0:nt * 4], in_=e_flat,
                                     func=EXP)
                # z[p, b, h] = sum_j ez ; zr = 1/z
                zt = sp.tile([128, GB, 4], F32, tag="z")
                ez_bhj = ezt[:, 0:nt * 4].rearrange(
                    "p (b j h) -> p b h j", b=nb, j=T, h=4)
                nc.vector.tensor_reduce(out=zt[:, 0:nb, :], in_=ez_bhj,
                                        axis=mybir.AxisListType.X,
                                        op=mybir.AluOpType.add)
                zrt = sp.tile([128, GB * 4], F32, tag="zr")
                nc.vector.reciprocal(
                    zrt[:, 0:nb * 4],
                    zt[:, 0:nb, :].rearrange("p b h -> p (b h)"))
                # alpha = ex / z  (per block; zr broadcast over j and pair)
                for b in range(nb):
                    ex2_blk = ex2t[:, b * T * 8:(b + 1) * T * 8].rearrange(
                        "p (j h two) -> p j h two", j=T, h=4, two=2)
                    zr_b = zrt[:, b * 4:(b + 1) * 4].unsqueeze(1)\
                        .unsqueeze(-1).to_broadcast([128, T, 4, 2])
                    nc.gpsimd.tensor_tensor(out=ex2_blk, in0=ex2_blk,
                                            in1=zr_b,
                                            op=mybir.AluOpType.mult)
                return (g0, nb, T, nt, vt, ex2t, xpt)

            def frontB(actx):
                """w = v*alpha (DVE) + PE transpose-accumulate."""
                g0, nb, T, nt, vt, ex2t, xpt = actx
                ex2_b = ex2t[:, 0:nt * 8].rearrange(
                    "p (a two) -> p a two", two=2).unsqueeze(2).to_broadcast(
                    [128, nt * 4, 16, 2])
                w_m = vt[:, 0:nt, :].rearrange(
                    "p bj (h c16 two) -> p (bj h) c16 two", h=4, c16=16,
                    two=2)
                nc.vector.tensor_tensor(out=w_m, in0=w_m, in1=ex2_b,
                                        op=mybir.AluOpType.mult)

                # aggT[feat, b, dst] += w_j^T  (PE transpose-accumulate)
                psagg = psagg_p.tile([128, GB, 128], F32, tag="agg")
                for b in range(nb):
                    for j in range(T):
                        nc.tensor.matmul(out=psagg[:, b, :],
                                         lhsT=vt[:, b * T + j, :],
                                         rhs=idf16_t[:],
                                         start=(j == 0), stop=(j == T - 1))
                return (g0, nb, psagg, xpt)

            def tail1(ctx):
                """BN+ELU: g=exp(k*agg+c), t1=relu(k*agg+c), u=min(g-1,t1)."""
                g0, nb, psagg, xpt = ctx
                gt = up.tile([128, GB, 128], F16, tag="g")
                nc.scalar.activation(out=gt[:, 0:nb, :],
                                     in_=psagg[:, 0:nb, :],
                                     func=EXP, bias=cT_t[:], scale=kT_t[:])
                t1t = up.tile([128, GB, 128], F16, tag="t1")
                nc.scalar.activation(out=t1t[:, 0:nb, :],
                                     in_=psagg[:, 0:nb, :],
                                     func=RELU, bias=cT_t[:], scale=kT_t[:])
                ut = up.tile([128, GB, 128], F16, tag="u")
                nc.vector.scalar_tensor_tensor(
                    out=ut[:, 0:nb, :], in0=gt[:, 0:nb, :], scalar=-1.0,
                    in1=t1t[:, 0:nb, :],
                    op0=mybir.AluOpType.add, op1=mybir.AluOpType.min)
                return (g0, nb, ut, xpt)

            def tail2(ctx):
                """Node matmul / MLP head + output."""
                g0, nb, ut, xpt = ctx
                if not final:
                    psh = psh_p.tile([128, GB, 128], F32, tag="h2")
                    pss = pssd_p.tile([128, GB, 8], F32, tag="sd2")
                    for b in range(nb):
                        nc.tensor.matmul(out=psh[:, b, :], lhsT=ut[:, b, :],
                                         rhs=w2h_t[:], start=True, stop=True)
                        nc.tensor.matmul(out=pss[:, b, :], lhsT=ut[:, b, :],
                                         rhs=w2sd_t[:], start=True, stop=True)
                    hsdc = up.tile([128, GB, 136], F16, tag="hsdc")
                    nc.scalar.copy(hsdc[:, 0:nb, 0:128], psh[:, 0:nb, :])
                    nc.scalar.copy(hsdc[:, 0:nb, 128:136],
                                   pss[:, 0:nb, :])
                    nc.scalar.dma_start(out=hsd2[:, g0:g0 + nb, :],
                                        in_=hsdc[:, 0:nb, :])
                else:
                    # MLP head on h = u + x_p, with the residual folded into
                    # the matmul: prT = Wc1k^T @ u + Wc1k^T @ xpT
                    psprT = psprt_p.tile([128, GB, 128], F32, tag="prT")
                    for b in range(nb):
                        nc.tensor.matmul(out=psprT[0:64, b, :],
                                         lhsT=wc1_t[:], rhs=ut[:, b, :],
                                         start=True, stop=False)
                        nc.tensor.matmul(out=psprT[0:64, b, :],
                                         lhsT=wc1_t[:], rhs=xpt[:, b, :],
                                         start=False, stop=True)
                    r1sb = up.tile([64, GB, 128], F16, tag="r1")
                    nc.scalar.activation(out=r1sb[0:64, 0:nb, :],
                                         in_=psprT[0:64, 0:nb, :],
                                         func=RELU, bias=ccT_t[:])
                    psy = pssd_p.tile([128, GB, 40], F32, tag="y")
                    for b in range(nb):
                        nc.tensor.matmul(out=psy[:, b, :],
                                         lhsT=r1sb[:, b, :],
                                         rhs=wc2_t[:], start=True, stop=False)
                        nc.tensor.matmul(out=psy[:, b, :], lhsT=on_t[:],
                                         rhs=bc2r_t[:], start=False,
                                         stop=True)
                    nc.scalar.copy(ysb[:, g0:g0 + nb, :], psy[:, 0:nb, :])
                    nc.scalar.activation(
                        out=eyt[:, g0:g0 + nb, :], in_=psy[:, 0:nb, :],
                        func=EXP)
                    nc.vector.tensor_reduce(
                        out=zs[:, g0:g0 + nb], in_=eyt[:, g0:g0 + nb, :],
                        axis=mybir.AxisListType.X, op=mybir.AluOpType.add)

            # software pipeline: frontB(i-1) | frontA(i) | tail(i-2)
            a_pend = []
            b_pend = []
            for grp in cfg.groups:
                if a_pend:
                    b_pend.append(frontB(a_pend.pop(0)))
                a_pend.append(frontA(grp))
                if len(b_pend) > 1:
                    tail2(tail1(b_pend.pop(0)))
            while a_pend:
                b_pend.append(frontB(a_pend.pop(0)))
            while b_pend:
                tail2(tail1(b_pend.pop(0)))

            if final:
                # log_softmax epilogue: one Ln, then subtract+DMA in halves
                lnz = cp.tile([128, nblk], F32)
                nc.scalar.activation(out=lnz[:], in_=zs[:],
                                     func=mybir.ActivationFunctionType.Ln)
                finsb = cp.tile([128, nblk, 40], F32)
                half = nblk // 2
                for lo, hi in ((0, half), (half, nblk)):
                    lnz_b = lnz[:, lo:hi].unsqueeze(-1).to_broadcast(
                        [128, hi - lo, 40])
                    nc.vector.tensor_tensor(out=finsb[:, lo:hi, :],
                                            in0=ysb[:, lo:hi, :], in1=lnz_b,
                                            op=mybir.AluOpType.subtract)
                    nc.sync.dma_start(out=fin[:, lo:hi, :],
                                      in_=finsb[:, lo:hi, :])
    nc.compile()
    return nc


# ----------------------------------------------------------------------------
# Host orchestration
# ----------------------------------------------------------------------------

_cache = {}


def _get(key, fn):
    if key not in _cache:
        _cache[key] = fn()
    return _cache[key]


def _amat(a):
    m = np.zeros((D, NH), np.float32)
    for h in range(NH):
        m[h * HD:(h + 1) * HD, h] = a[h]
    return m


def _run(nc, in_maps, cfg, tag):
    res = run_bass_kernel_spmd(nc, in_maps, list(range(cfg.ncores)),
                               trace=PROFILE)
    if PROFILE:
        LAST_EXEC_NS.append((tag, res.exec_time_ns))
    return res.results


def _slotify(arr, cdim):
    """[128, nblk, c] device layout -> [slots, c] (slot = g*128 + lane)."""
    return arr.transpose(1, 0, 2).reshape(-1, cdim)


def kernel(x, edge_index, res_W, res_b,
           W1, as1, ad1, b1, g1, be1, rm1, rv1,
           W2, as2, ad2, b2, g2, be2, rm2, rv2,
           Wc1, bc1, gc, bec, rmc, rvc, Wc2, bc2,
           _cfg=None):
    cfg = _cfg or _get("cfg", lambda: Cfg())
    x = np.asarray(x, np.float32)
    edge_index = np.asarray(edge_index)
    (res_W, res_b, W1, as1, ad1, b1, g1, be1, rm1, rv1,
     W2, as2, ad2, b2, g2, be2, rm2, rv2,
     Wc1, bc1, gc, bec, rmc, rvc, Wc2, bc2) = (
        np.asarray(a, np.float32) for a in (
            res_W, res_b, W1, as1, ad1, b1, g1, be1, rm1, rv1,
            W2, as2, ad2, b2, g2, be2, rm2, rv2,
            Wc1, bc1, gc, bec, rmc, rvc, Wc2, bc2))

    ekey = ("prep", hash(edge_index.tobytes()))
    prep = _get(ekey, lambda: host_prep(np.asarray(edge_index, np.int64), cfg))
    nslots = cfg.ncores * cfg.slots

    def fold_bn(g_, be_, rm_, rv_, bias):
        k = (g_ / np.sqrt(rv_ + EPS_BN)).astype(np.float32)
        c = ((bias - rm_) * k + be_).astype(np.float32)
        return k, c

    k1, c1 = fold_bn(g1, be1, rm1, rv1, b1)
    k2, c2 = fold_bn(g2, be2, rm2, rv2, b2)
    kc, cc = fold_bn(gc, bec, rmc, rvc, bc1)

    W1cat = np.concatenate(
        [W1, W1 @ _amat(as1), W1 @ _amat(ad1)], axis=1).astype(np.float32)
    W2cat = np.concatenate(
        [W2, W2 @ _amat(as2), W2 @ _amat(ad2)], axis=1).astype(np.float32)
    Wfold = (res_W.astype(np.float32) @ W1cat)
    bb = (res_b.astype(np.float32) @ W1cat)

    ident16 = np.eye(128, dtype=np.float16)

    # ---- launch A ----
    x_sl = np.zeros((nslots, IN), np.float32)
    x_sl[prep.slot_of] = x
    nc_a = _get(("A",), lambda: build_launch_a(cfg))
    in_a = []
    for c in range(cfg.ncores):
        xs = x_sl[c * cfg.slots:(c + 1) * cfg.slots]
        in_a.append(dict(
            xT=np.ascontiguousarray(xs.T).astype(np.float16),
            rw=res_W.astype(np.float16),
            rb=np.asarray(res_b, np.float32).reshape(128, 1),
            wfh=Wfold[:, 0:128].astype(np.float16),
            wfsd=Wfold[:, 128:136].astype(np.float16),
            ones1=np.ones((1, 128), np.float16),
            bbh=bb[0:128].reshape(1, 128).astype(np.float16),
            bbsd=bb[128:136].reshape(1, 8).astype(np.float16)))
    res_a = _run(nc_a, in_a, cfg, "A")

    hsd1_all = np.concatenate(
        [_slotify(res_a[c]["hsd1"], 136) for c in range(cfg.ncores)])
    h1_all, sd1_all = hsd1_all[:, 0:128], hsd1_all[:, 128:136]
    xp_dev = [res_a[c]["xpT"] for c in range(cfg.ncores)]

    # ---- launch B ----
    vsd1 = route_edge_arrays(prep, cfg, h1_all, sd1_all[:, 0:4],
                             sd1_all[:, 4:8])
    key_e = (cfg.SumT, tuple(g[2] for g in cfg.groups))
    nc_b = _get(("B", key_e), lambda: build_launch_edge(cfg, final=False))
    in_b = []
    for c in range(cfg.ncores):
        v, sdin = vsd1[c]
        in_b.append(dict(
            v=v, sdin=sdin, kT=k1.reshape(128, 1),
            cT=c1.reshape(128, 1), idf16=ident16,
            w2h=W2cat[:, 0:128].astype(np.float16),
            w2sd=W2cat[:, 128:136].astype(np.float16)))
    res_b_ = _run(nc_b, in_b, cfg, "B")

    hsd2_all = np.concatenate(
        [_slotify(res_b_[c]["hsd2"], 136) for c in range(cfg.ncores)])
    h2_all, sd2_all = hsd2_all[:, 0:128], hsd2_all[:, 128:136]

    # ---- launch C ----
    vsd2 = route_edge_arrays(prep, cfg, h2_all, sd2_all[:, 0:4],
                             sd2_all[:, 4:8])
    nc_c = _get(("C", key_e), lambda: build_launch_edge(cfg, final=True))
    Wc1k = (Wc1.astype(np.float32) * kc[None, :]).astype(np.float16)
    in_c = []
    for c in range(cfg.ncores):
        v, sdin = vsd2[c]
        in_c.append(dict(
            v=v, sdin=sdin, kT=k2.reshape(128, 1),
            cT=c2.reshape(128, 1), idf16=ident16,
            xpT=xp_dev[c], wc1=Wc1k, ccT=cc.reshape(64, 1),
            wc2=np.asarray(Wc2, np.float32).astype(np.float16),
            bc2r=np.asarray(bc2, np.float32).reshape(1, OUT).astype(np.float16),
            ones1=np.ones((1, 128), np.float16)))
    res_c = _run(nc_c, in_c, cfg, "C")

    fin_slots = np.concatenate(
        [_slotify(res_c[c]["fin"], 40) for c in range(cfg.ncores)])
    return np.ascontiguousarray(fin_slots[prep.slot_of]).astype(np.float32)


# revision 80
# speedup vs baseline: 1.3819x; 1.0301x over previous
"""Trainium2 Bass kernel for nn_GAT_Vanilla (2-layer GAT + BN/ELU + MLP head).

Strategy (8 NeuronCores, graph/data parallel, dst-major edge layout):
- Nodes sorted by in-degree and chopped into 128-node bins (degree
  homogeneous); bins dealt round-robin to the 8 cores so every core gets
  an identical block-T profile. Edges are laid dst-major: partition lane
  = dst slot, free dim = (j-th incoming edge, feature).
- 3 SPMD launches: A) per-node matmuls producing h1/s1/d1/x_p^T;
  B) layer-1 edge phase (softmax-weighted aggregation) + layer-2 node
  matmul; C) layer-2 edge phase + residual + MLP head + log_softmax.
- Between launches the host routes device-computed per-node values into
  per-edge arrays (pure indexing/gather, no math) - the halo exchange.
- On device per block group: e=s+d, leaky (Pool); exp duplicated x2
  (Act, keeps DVE packed-2x mode); one DVE multiply v*=ex; PE
  transpose-accumulate (matmul vs identity) scatter-sums into PSUM
  producing agg^T feat-major; 1/z and BN-scale ride a tiny head-expand
  matmul; BN bias + ELU fused into Act ops. fp16 data, f32 accumulate.

Self-contained: only needs numpy + the concourse/bass stack.
"""

import numpy as np

import concourse.bass as bass
import concourse.bacc as bacc
import concourse.tile as tile
from concourse import mybir
from concourse.bass_utils import run_bass_kernel_spmd

F32 = mybir.dt.float32
F16 = mybir.dt.float16

# ---- problem constants (hardcoded per harness contract) ----
N, E, IN, HD, NH, OUT = 100000, 800000, 128, 32, 4, 40
D = HD * NH  # 128
EPS_BN = 1e-5
SPAD = -300.0  # pad-edge s value -> leaky -> -60 -> exp == 0

NCORES = 8
NBLK = 98   # blocks (bins) per core; 8*98*128 = 100352 slots >= N
GB = 4      # blocks per group (uniform T within a group)

PROFILE = False
LAST_EXEC_NS = []


class Cfg:
    def __init__(self, n=N, e=E, ncores=NCORES, nblk=NBLK, gb=GB):
        self.n, self.e, self.ncores, self.nblk, self.gb = n, e, ncores, nblk, gb
        self.slots = nblk * 128
        # set by host_prep:
        self.Tpad = None    # per-block T (uniform within each group)
        self.offs = None    # per-block tile offset, offs[nblk] = SumT
        self.SumT = None
        self.groups = None  # list of (g0, nb, T)


class Prep:
    pass


# ----------------------------------------------------------------------------
# Host preprocessing: degree-sorted binning + per-edge index matrices
# ----------------------------------------------------------------------------

def host_prep(edge_index, cfg: Cfg):
    n, e = cfg.n, cfg.e
    ncores, nblk, gb = cfg.ncores, cfg.nblk, cfg.gb
    nbins = ncores * nblk
    nslots = nbins * 128
    src = np.concatenate([edge_index[0], np.arange(n)]).astype(np.int64)
    dst = np.concatenate([edge_index[1], np.arange(n)]).astype(np.int64)
    deg = np.bincount(dst, minlength=n)

    # degree-sorted consecutive bins of 128 nodes; bin k -> core k%ncores,
    # block position k//ncores, so all cores share one T profile.
    order = np.argsort(-deg, kind="stable")
    binrank = np.arange(n) // 128          # bin of i-th sorted node
    lane = np.arange(n) % 128
    core_of = np.empty(n, np.int64)
    gpos_of = np.empty(n, np.int64)
    lane_of = np.empty(n, np.int64)
    core_of[order] = binrank % ncores
    gpos_of[order] = binrank // ncores
    lane_of[order] = lane
    slot_of = core_of * (nblk * 128) + gpos_of * 128 + lane_of

    # per-bin max degree -> per-block-position profile -> group-pad
    degs_sorted = deg[order]
    nb_used = (n + 127) // 128
    Tbin = np.ones(nbins, np.int64)
    maxs = np.maximum.reduceat(degs_sorted, np.arange(0, n, 128))
    Tbin[:nb_used] = np.maximum(maxs, 1)
    Tprof = np.ones(nblk, np.int64)
    for g in range(nblk):
        lo, hi = g * ncores, min((g + 1) * ncores, nbins)
        Tprof[g] = max(1, Tbin[lo:hi].max())
    Tpad = Tprof.copy()
    groups = []
    g = 0
    while g < nblk:
        nb = min(gb, nblk - g)
        T = int(Tprof[g:g + nb].max())
        Tpad[g:g + nb] = T
        groups.append((g, nb, T))
        g += nb
    offs = np.zeros(nblk + 1, np.int64)
    np.cumsum(Tpad, out=offs[1:])
    SumT = int(offs[nblk])
    cfg.Tpad, cfg.offs, cfg.SumT, cfg.groups = Tpad, offs, SumT, groups

    # edge -> (core, column, lane) in dst-major layout
    dslot = slot_of[dst]
    eorder = np.argsort(dslot, kind="stable")
    src_s, dslot_s = src[eorder], dslot[eorder]
    # j = rank within dst
    uniq, starts_idx = np.unique(dslot_s, return_index=True)
    j_of = np.arange(e + n, dtype=np.int64)
    j_of -= np.repeat(starts_idx, np.diff(np.append(starts_idx, e + n)))
    ecore = dslot_s // (nblk * 128)
    egpos = (dslot_s // 128) % nblk
    elane = dslot_s % 128
    ecol = offs[egpos] + j_of

    # per-core index matrices [128, SumT]:
    #   Iv: row into h_ext (nslots -> zero row)
    #   Is: row into s_ext (nslots -> SPAD row, nslots+1 -> zero row)
    prep = Prep()
    prep.Iv, prep.Is = [], []
    for c in range(ncores):
        iv = np.full((128, SumT), nslots, np.int64)
        is_ = np.full((128, SumT), nslots, np.int64)
        m = ecore == c
        iv[elane[m], ecol[m]] = slot_of[src_s[m]]
        is_[elane[m], ecol[m]] = slot_of[src_s[m]]
        prep.Iv.append(iv)
        prep.Is.append(is_)
    # dummy slots (no real node): one self edge with v=0, s=0 at j=0
    have = np.zeros(nslots, bool)
    have[slot_of] = True
    dumm = np.nonzero(~have)[0]
    dc = dumm // (nblk * 128)
    dg = (dumm // 128) % nblk
    dl = dumm % 128
    for c in range(ncores):
        m = dc == c
        prep.Is[c][dl[m], offs[dg[m]]] = nslots + 1  # s = 0
        # Iv stays nslots (v = 0)

    # own-node slot grid for d routing [128, nblk]
    g_grid = np.broadcast_to(np.arange(nblk)[None, :], (128, nblk))
    l_grid = np.broadcast_to(np.arange(128)[:, None], (128, nblk))
    prep.dgrid = [c * nblk * 128 + g_grid * 128 + l_grid for c in range(ncores)]

    prep.slot_of = slot_of
    prep.cfg = cfg
    return prep


def route_edge_arrays(prep, cfg, h_all16, s_all16, d_all16):
    """h_all16 [nslots,128] f16, s_all16 [nslots,4] f16, d_all16 [nslots,4].
    Returns per-core (v [128,SumT,128], sdin [128,SumT+nblk,4]) where sdin
    interleaves per group: [s cols of its blocks | d cols of its blocks]."""
    h_ext = np.vstack([h_all16, np.zeros((1, 128), np.float16)])
    s_ext = np.vstack([s_all16,
                       np.full((1, 4), SPAD, np.float16),
                       np.zeros((1, 4), np.float16)])
    out = []
    for c in range(cfg.ncores):
        v = h_ext[prep.Iv[c]]
        s = s_ext[prep.Is[c]]
        d = d_all16[prep.dgrid[c]]
        sdin = np.zeros((128, cfg.SumT + cfg.nblk, 4), np.float16)
        for (g0, nb, T) in cfg.groups:
            off = int(cfg.offs[g0])
            nt = nb * T
            base = off + g0
            sdin[:, base:base + nt] = s[:, off:off + nt]
            sdin[:, base + nt:base + nt + nb] = d[:, g0:g0 + nb]
        out.append((v, sdin))
    return out


# ----------------------------------------------------------------------------
# Device kernels
# ----------------------------------------------------------------------------

def build_launch_a(cfg: Cfg):
    """xpT = res_W^T x^T + res_b ; [h1|s1|d1] = x @ (res_W@W1cat) + res_b@W1cat."""
    nc = bacc.Bacc("TRN2", target_bir_lowering=False, debug=False,
                   num_devices=cfg.ncores)
    nblk = cfg.nblk
    xT = nc.dram_tensor("xT", [128, cfg.slots], F16, kind="ExternalInput").ap()
    rw = nc.dram_tensor("rw", [128, 128], F16, kind="ExternalInput").ap()
    rb = nc.dram_tensor("rb", [128, 1], F32, kind="ExternalInput").ap()
    wfh = nc.dram_tensor("wfh", [128, 128], F16, kind="ExternalInput").ap()
    wfsd = nc.dram_tensor("wfsd", [128, 8], F16, kind="ExternalInput").ap()
    ones1 = nc.dram_tensor("ones1", [1, 128], F16, kind="ExternalInput").ap()
    bbh = nc.dram_tensor("bbh", [1, 128], F16, kind="ExternalInput").ap()
    bbsd = nc.dram_tensor("bbsd", [1, 8], F16, kind="ExternalInput").ap()
    hsd1 = nc.dram_tensor("hsd1", [128, nblk, 136], F16,
                          kind="ExternalOutput").ap()

    CHA = 9   # blocks per IO chunk (three PSUM sub-groups of 3)
    GBA = 3   # blocks per PSUM sub-group ([128,3,136] f32 fits one bank)
    with tile.TileContext(nc) as tc:
        with (
            tc.tile_pool(name="const", bufs=1) as cp,
            tc.tile_pool(name="io", bufs=3) as iop,
            tc.tile_pool(name="psh", bufs=4, space="PSUM") as psh_p,
        ):
            wfh_t = cp.tile([128, 128], F16)
            nc.sync.dma_start(out=wfh_t[:], in_=wfh)
            wfsd_t = cp.tile([128, 8], F16)
            nc.sync.dma_start(out=wfsd_t[:], in_=wfsd)
            on_t = cp.tile([1, 128], F16)
            nc.sync.dma_start(out=on_t[:], in_=ones1)
            bbh_t = cp.tile([1, 128], F16)
            nc.sync.dma_start(out=bbh_t[:], in_=bbh)
            bbsd_t = cp.tile([1, 8], F16)
            nc.sync.dma_start(out=bbsd_t[:], in_=bbsd)

            chunks = {}   # c0 -> [xpc, hsdc, nch, ndone]

            def a_sub(c0, i0, nb, xt):
                """matmuls for one 3-block sub-group."""
                psh = psh_p.tile([128, GBA, 136], F32, tag="h")
                for b in range(nb):
                    nc.tensor.matmul(out=psh[:, b, 0:128],
                                     lhsT=xt[:, i0 + b, :],
                                     rhs=wfh_t[:], start=True, stop=False)
                    nc.tensor.matmul(out=psh[:, b, 0:128], lhsT=on_t[:],
                                     rhs=bbh_t[:], start=False, stop=True)
                    nc.tensor.matmul(out=psh[:, b, 128:136],
                                     lhsT=xt[:, i0 + b, :],
                                     rhs=wfsd_t[:], start=True, stop=False)
                    nc.tensor.matmul(out=psh[:, b, 128:136], lhsT=on_t[:],
                                     rhs=bbsd_t[:], start=False, stop=True)
                return (c0, i0, nb, psh)

            def a_evac(ctx):
                """PSUM evacuation for one sub; out-DMA when chunk done."""
                c0, i0, nb, psh = ctx
                hsdc, nch, _ = chunks[c0][0], chunks[c0][1], chunks[c0][2]
                nc.vector.tensor_copy(hsdc[:, i0:i0 + nb, :],
                                      psh[:, 0:nb, :])
                chunks[c0][2] += nb
                if chunks[c0][2] == nch:
                    nc.scalar.dma_start(out=hsd1[:, c0:c0 + nch, :],
                                        in_=hsdc[:, 0:nch, :])
                    del chunks[c0]

            a_pend = []
            for c0 in range(0, nblk, CHA):
                nch = min(CHA, nblk - c0)
                xt = iop.tile([128, CHA, 128], F16, tag="xt")
                nc.sync.dma_start(
                    out=xt[:, 0:nch, :],
                    in_=xT[:, c0 * 128:(c0 + nch) * 128].rearrange(
                        "p (b q) -> p b q", b=nch))
                hsdc_t = iop.tile([128, CHA, 136], F16, tag="hsdc")
                chunks[c0] = [hsdc_t, nch, 0]
                for i0 in range(0, nch, GBA):
                    nb = min(GBA, nch - i0)
                    a_pend.append(a_sub(c0, i0, nb, xt))
                    if len(a_pend) > 2:
                        a_evac(a_pend.pop(0))
            while a_pend:
                a_evac(a_pend.pop(0))
    nc.compile()
    return nc


def build_launch_edge(cfg: Cfg, final: bool):
    """Edge phase (softmax-weighted aggregation, BN+ELU) for one layer.
    final=False: + layer-2 node matmul (h2/sd2 out).
    final=True:  + residual + MLP head + log_softmax (fin out)."""
    nc = bacc.Bacc("TRN2", target_bir_lowering=False, debug=False,
                   num_devices=cfg.ncores)
    nblk, SumT = cfg.nblk, cfg.SumT
    GT = max(g[1] * g[2] for g in cfg.groups)

    v_ap = nc.dram_tensor("v", [128, SumT, 128], F16, kind="ExternalInput").ap()
    sdin = nc.dram_tensor("sdin", [128, SumT + nblk, 4], F16,
                          kind="ExternalInput").ap()
    kT = nc.dram_tensor("kT", [128, 1], F32, kind="ExternalInput").ap()
    cT = nc.dram_tensor("cT", [128, 1], F32, kind="ExternalInput").ap()
    idf16 = nc.dram_tensor("idf16", [128, 128], F16, kind="ExternalInput").ap()
    if not final:
        w2h = nc.dram_tensor("w2h", [128, 128], F16, kind="ExternalInput").ap()
        w2sd = nc.dram_tensor("w2sd", [128, 8], F16, kind="ExternalInput").ap()
        hsd2 = nc.dram_tensor("hsd2", [128, nblk, 136], F16,
                              kind="ExternalOutput").ap()
    else:
        xTc = nc.dram_tensor("xTc", [128, nblk * 128], F16,
                             kind="ExternalInput").ap()
        wxc = nc.dram_tensor("wxc", [128, 64], F16, kind="ExternalInput").ap()
        wc1 = nc.dram_tensor("wc1", [128, 64], F16, kind="ExternalInput").ap()
        ccT = nc.dram_tensor("ccT", [64, 1], F32, kind="ExternalInput").ap()
        wc2 = nc.dram_tensor("wc2", [64, 40], F16, kind="ExternalInput").ap()
        bc2r = nc.dram_tensor("bc2r", [1, 40], F16, kind="ExternalInput").ap()
        ones1 = nc.dram_tensor("ones1", [1, 128], F16,
                               kind="ExternalInput").ap()
        fin = nc.dram_tensor("fin", [128, nblk, 40], F32,
                             kind="ExternalOutput").ap()

    EXP = mybir.ActivationFunctionType.Exp
    RELU = mybir.ActivationFunctionType.Relu
    with tile.TileContext(nc) as tc:
        with (
            tc.tile_pool(name="const", bufs=1) as cp,
            tc.tile_pool(name="vp", bufs=3) as vp,
            tc.tile_pool(name="sp", bufs=4) as sp,
            tc.tile_pool(name="up", bufs=4) as up,
            tc.tile_pool(name="psagg", bufs=3, space="PSUM") as psagg_p,
            tc.tile_pool(name="psh", bufs=2, space="PSUM") as psh_p,
            tc.tile_pool(name="pssd", bufs=2, space="PSUM") as pssd_p,
            tc.tile_pool(name="psprt", bufs=2, space="PSUM") as psprt_p,
        ):
            kT_t = cp.tile([128, 1], F32)
            nc.sync.dma_start(out=kT_t[:], in_=kT)
            cT_t = cp.tile([128, 1], F32)
            nc.sync.dma_start(out=cT_t[:], in_=cT)
            idf16_t = cp.tile([128, 128], F16)
            nc.sync.dma_start(out=idf16_t[:], in_=idf16)
            if not final:
                w2h_t = cp.tile([128, 128], F16)
                nc.sync.dma_start(out=w2h_t[:], in_=w2h)
                w2sd_t = cp.tile([128, 8], F16)
                nc.sync.dma_start(out=w2sd_t[:], in_=w2sd)
            else:
                wc1_t = cp.tile([128, 64], F16)
                nc.sync.dma_start(out=wc1_t[:], in_=wc1)
                wxc_t = cp.tile([128, 64], F16)
                nc.sync.dma_start(out=wxc_t[:], in_=wxc)
                ccT_t = cp.tile([64, 1], F32)
                nc.sync.dma_start(out=ccT_t[:], in_=ccT)
                wc2_t = cp.tile([64, 40], F16)
                nc.sync.dma_start(out=wc2_t[:], in_=wc2)
                bc2r_t = cp.tile([1, 40], F16)
                nc.sync.dma_start(out=bc2r_t[:], in_=bc2r)
                on_t = cp.tile([1, 128], F16)
                nc.sync.dma_start(out=on_t[:], in_=ones1)
                ysb = cp.tile([128, nblk, 40], F32)
                eyt = cp.tile([128, nblk, 40], F16)
                zs = cp.tile([128, nblk], F32)

            def frontA(grp):
                """DMA in + softmax chain through alpha."""
                g0, nb, T = grp
                off = int(cfg.offs[g0])
                nt = nb * T
                sdt = sp.tile([128, GT + GB, 4], F16, tag="sd")
                vt = vp.tile([128, GT, 128], F16, tag="v")
                if not final:  # B: small sdt ahead of the big v transfer
                    nc.sync.dma_start(
                        out=sdt[:, 0:nt + nb, :],
                        in_=sdin[:, off + g0:off + g0 + nt + nb, :])
                    nc.sync.dma_start(out=vt[:, 0:nt, :],
                                      in_=v_ap[:, off:off + nt, :])
                    xpt = None
                else:
                    nc.sync.dma_start(out=vt[:, 0:nt, :],
                                      in_=v_ap[:, off:off + nt, :])
                    nc.sync.dma_start(
                        out=sdt[:, 0:nt + nb, :],
                        in_=sdin[:, off + g0:off + g0 + nt + nb, :])
                    xpt = up.tile([128, GB, 128], F16, tag="xpt")
                    nc.sync.dma_start(
                        out=xpt[:, 0:nb, :],
                        in_=xTc[:, g0 * 128:(g0 + nb) * 128].rearrange(
                            "p (b q) -> p b q", b=nb))
                st = sdt[:, 0:nt, :]
                dt_ = sdt[:, nt:nt + nb, :]

                # e = leaky(s + d)  [p, (b j) h] fp16
                et = sp.tile([128, GT, 4], F16, tag="e")
                e_bjh = et[:, 0:nt, :].rearrange("p (b j) h -> p b j h", b=nb)
                d_b = dt_.unsqueeze(2).to_broadcast([128, nb, T, 4])
                s_bjh = st.rearrange("p (b j) h -> p b j h", b=nb)
                nc.gpsimd.tensor_tensor(out=e_bjh, in0=s_bjh, in1=d_b,
                                        op=mybir.AluOpType.add)
                e_flat = et[:, 0:nt, :].rearrange("p a h -> p (a h)")
                e2t = sp.tile([128, GT, 4], F16, tag="e2")
                e2_flat = e2t[:, 0:nt, :].rearrange("p a h -> p (a h)")
                nc.vector.tensor_scalar(out=e2_flat, in0=e_flat, scalar1=0.2,
                                        scalar2=None, op0=mybir.AluOpType.mult)
                nc.vector.tensor_tensor(out=e_flat, in0=e_flat, in1=e2_flat,
                                        op=mybir.AluOpType.max)

                # ex2 = exp(e) duplicated x2 (Act); ez = exp(e) (Act)
                ex2t = sp.tile([128, GT * 8], F16, tag="ex2")
                ex2_v = ex2t[:, 0:nt * 8].rearrange("p (a two) -> p a two",
                                                    two=2)
                e_b2 = e_flat.unsqueeze(-1).to_broadcast([128, nt * 4, 2])
                nc.scalar.activation(out=ex2_v, in_=e_b2, func=EXP)
                ezt = sp.tile([128, GT * 4], F16, tag="ez")
                nc.scalar.activation(out=ezt[:, 0:nt * 4], in_=e_flat,
                                     func=EXP)
                # z[p, b, h] = sum_j ez ; zr = 1/z
                zt = sp.tile([128, GB, 4], F32, tag="z")
                ez_bhj = ezt[:, 0:nt * 4].rearrange(
                    "p (b j h) -> p b h j", b=nb, j=T, h=4)
                nc.vector.tensor_reduce(out=zt[:, 0:nb, :], in_=ez_bhj,
                                        axis=mybir.AxisListType.X,
                                        op=mybir.AluOpType.add)
                zrt = sp.tile([128, GB * 4], F32, tag="zr")
                nc.vector.reciprocal(
                    zrt[:, 0:nb * 4],
                    zt[:, 0:nb, :].rearrange("p b h -> p (b h)"))
                # alpha = ex / z  (per block; zr broadcast over j and pair)
                for b in range(nb):
                    ex2_blk = ex2t[:, b * T * 8:(b + 1) * T * 8].rearrange(
                        "p (j h two) -> p j h two", j=T, h=4, two=2)
                    zr_b = zrt[:, b * 4:(b + 1) * 4].unsqueeze(1)\
                        .unsqueeze(-1).to_broadcast([128, T, 4, 2])
                    nc.gpsimd.tensor_tensor(out=ex2_blk, in0=ex2_blk,
                                            in1=zr_b,
                                            op=mybir.AluOpType.mult)
                return (g0, nb, T, nt, vt, ex2t, xpt)

            def frontB(actx):
                """w = v*alpha (DVE) + PE transpose-accumulate."""
                g0, nb, T, nt, vt, ex2t, xpt = actx
                ex2_b = ex2t[:, 0:nt * 8].rearrange(
                    "p (a two) -> p a two", two=2).unsqueeze(2).to_broadcast(
                    [128, nt * 4, 16, 2])
                w_m = vt[:, 0:nt, :].rearrange(
                    "p bj (h c16 two) -> p (bj h) c16 two", h=4, c16=16,
                    two=2)
                nc.vector.tensor_tensor(out=w_m, in0=w_m, in1=ex2_b,
                                        op=mybir.AluOpType.mult)

                # aggT[feat, b, dst] += w_j^T  (PE transpose-accumulate)
                psagg = psagg_p.tile([128, GB, 128], F32, tag="agg")
                for b in range(nb):
                    for j in range(T):
                        nc.tensor.matmul(out=psagg[:, b, :],
                                         lhsT=vt[:, b * T + j, :],
                                         rhs=idf16_t[:],
                                         start=(j == 0), stop=(j == T - 1))
                return (g0, nb, psagg, xpt)

            def tail1(ctx):
                """BN+ELU: g=exp(k*agg+c), t1=relu(k*agg+c), u=min(g-1,t1)."""
                g0, nb, psagg, xpt = ctx
                gt = up.tile([128, GB, 128], F16, tag="g")
                nc.scalar.activation(out=gt[:, 0:nb, :],
                                     in_=psagg[:, 0:nb, :],
                                     func=EXP, bias=cT_t[:], scale=kT_t[:])
                t1t = up.tile([128, GB, 128], F16, tag="t1")
                nc.scalar.activation(out=t1t[:, 0:nb, :],
                                     in_=psagg[:, 0:nb, :],
                                     func=RELU, bias=cT_t[:], scale=kT_t[:])
                ut = up.tile([128, GB, 128], F16, tag="u")
                nc.vector.scalar_tensor_tensor(
                    out=ut[:, 0:nb, :], in0=gt[:, 0:nb, :], scalar=-1.0,
                    in1=t1t[:, 0:nb, :],
                    op0=mybir.AluOpType.add, op1=mybir.AluOpType.min)
                return (g0, nb, ut, xpt)

            def tail2(ctx):
                """Node matmul / MLP head + output."""
                g0, nb, ut, xpt = ctx
                if not final:
                    psh = psh_p.tile([128, GB, 128], F32, tag="h2")
                    pss = pssd_p.tile([128, GB, 8], F32, tag="sd2")
                    for b in range(nb):
                        nc.tensor.matmul(out=psh[:, b, :], lhsT=ut[:, b, :],
                                         rhs=w2h_t[:], start=True, stop=True)
                        nc.tensor.matmul(out=pss[:, b, :], lhsT=ut[:, b, :],
                                         rhs=w2sd_t[:], start=True, stop=True)
                    hsdc = up.tile([128, GB, 136], F16, tag="hsdc")
                    nc.scalar.copy(hsdc[:, 0:nb, 0:128], psh[:, 0:nb, :])
                    nc.scalar.copy(hsdc[:, 0:nb, 128:136],
                                   pss[:, 0:nb, :])
                    nc.scalar.dma_start(out=hsd2[:, g0:g0 + nb, :],
                                        in_=hsdc[:, 0:nb, :])
                else:
                    # MLP head on h = u + x_p, with the residual folded into
                    # the matmul: prT = Wc1k^T @ u + Wc1k^T @ xpT
                    psprT = psprt_p.tile([128, GB, 128], F32, tag="prT")
                    for b in range(nb):
                        nc.tensor.matmul(out=psprT[0:64, b, :],
                                         lhsT=wc1_t[:], rhs=ut[:, b, :],
                                         start=True, stop=False)
                        nc.tensor.matmul(out=psprT[0:64, b, :],
                                         lhsT=wxc_t[:], rhs=xpt[:, b, :],
                                         start=False, stop=True)
                    r1sb = up.tile([64, GB, 128], F16, tag="r1")
                    nc.scalar.activation(out=r1sb[0:64, 0:nb, :],
                                         in_=psprT[0:64, 0:nb, :],
                                         func=RELU, bias=ccT_t[:])
                    psy = pssd_p.tile([128, GB, 40], F32, tag="y")
                    for b in range(nb):
                        nc.tensor.matmul(out=psy[:, b, :],
                                         lhsT=r1sb[:, b, :],
                                         rhs=wc2_t[:], start=True, stop=False)
                        nc.tensor.matmul(out=psy[:, b, :], lhsT=on_t[:],
                                         rhs=bc2r_t[:], start=False,
                                         stop=True)
                    nc.scalar.copy(ysb[:, g0:g0 + nb, :], psy[:, 0:nb, :])
                    nc.scalar.activation(
                        out=eyt[:, g0:g0 + nb, :], in_=psy[:, 0:nb, :],
                        func=EXP)
                    nc.vector.tensor_reduce(
                        out=zs[:, g0:g0 + nb], in_=eyt[:, g0:g0 + nb, :],
                        axis=mybir.AxisListType.X, op=mybir.AluOpType.add)

            # software pipeline: frontB(i-1) | frontA(i) | tail(i-2)
            a_pend = []
            b_pend = []
            for grp in cfg.groups:
                if a_pend:
                    b_pend.append(frontB(a_pend.pop(0)))
                a_pend.append(frontA(grp))
                if len(b_pend) > 1:
                    tail2(tail1(b_pend.pop(0)))
            while a_pend:
                b_pend.append(frontB(a_pend.pop(0)))
            while b_pend:
                tail2(tail1(b_pend.pop(0)))

            if final:
                # log_softmax epilogue: one Ln, then subtract+DMA in halves
                lnz = cp.tile([128, nblk], F32)
                nc.scalar.activation(out=lnz[:], in_=zs[:],
                                     func=mybir.ActivationFunctionType.Ln)
                finsb = cp.tile([128, nblk, 40], F32)
                half = nblk // 2
                for lo, hi in ((0, half), (half, nblk)):
                    lnz_b = lnz[:, lo:hi].unsqueeze(-1).to_broadcast(
                        [128, hi - lo, 40])
                    nc.vector.tensor_tensor(out=finsb[:, lo:hi, :],
                                            in0=ysb[:, lo:hi, :], in1=lnz_b,
                                            op=mybir.AluOpType.subtract)
                    nc.sync.dma_start(out=fin[:, lo:hi, :],
                                      in_=finsb[:, lo:hi, :])
    nc.compile()
    return nc


# ----------------------------------------------------------------------------
# Host orchestration
# ----------------------------------------------------------------------------

_cache = {}


def _get(key, fn):
    if key not in _cache:
        _cache[key] = fn()
    return _cache[key]


def _amat(a):
    m = np.zeros((D, NH), np.float32)
    for h in range(NH):
        m[h * HD:(h + 1) * HD, h] = a[h]
    return m


def _run(nc, in_maps, cfg, tag):
    res = run_bass_kernel_spmd(nc, in_maps, list(range(cfg.ncores)),
                               trace=PROFILE)
    if PROFILE:
        LAST_EXEC_NS.append((tag, res.exec_time_ns))
    return res.results


def _slotify(arr, cdim):
    """[128, nblk, c] device layout -> [slots, c] (slot = g*128 + lane)."""
    return arr.transpose(1, 0, 2).reshape(-1, cdim)


def kernel(x, edge_index, res_W, res_b,
           W1, as1, ad1, b1, g1, be1, rm1, rv1,
           W2, as2, ad2, b2, g2, be2, rm2, rv2,
           Wc1, bc1, gc, bec, rmc, rvc, Wc2, bc2,
           _cfg=None):
    cfg = _cfg or _get("cfg", lambda: Cfg())
    x = np.asarray(x, np.float32)
    edge_index = np.asarray(edge_index)
    (res_W, res_b, W1, as1, ad1, b1, g1, be1, rm1, rv1,
     W2, as2, ad2, b2, g2, be2, rm2, rv2,
     Wc1, bc1, gc, bec, rmc, rvc, Wc2, bc2) = (
        np.asarray(a, np.float32) for a in (
            res_W, res_b, W1, as1, ad1, b1, g1, be1, rm1, rv1,
            W2, as2, ad2, b2, g2, be2, rm2, rv2,
            Wc1, bc1, gc, bec, rmc, rvc, Wc2, bc2))

    ekey = ("prep", hash(edge_index.tobytes()))
    prep = _get(ekey, lambda: host_prep(np.asarray(edge_index, np.int64), cfg))
    nslots = cfg.ncores * cfg.slots

    def fold_bn(g_, be_, rm_, rv_, bias):
        k = (g_ / np.sqrt(rv_ + EPS_BN)).astype(np.float32)
        c = ((bias - rm_) * k + be_).astype(np.float32)
        return k, c

    k1, c1 = fold_bn(g1, be1, rm1, rv1, b1)
    k2, c2 = fold_bn(g2, be2, rm2, rv2, b2)
    kc, cc = fold_bn(gc, bec, rmc, rvc, bc1)

    W1cat = np.concatenate(
        [W1, W1 @ _amat(as1), W1 @ _amat(ad1)], axis=1).astype(np.float32)
    W2cat = np.concatenate(
        [W2, W2 @ _amat(as2), W2 @ _amat(ad2)], axis=1).astype(np.float32)
    Wfold = (res_W.astype(np.float32) @ W1cat)
    bb = (res_b.astype(np.float32) @ W1cat)

    ident16 = np.eye(128, dtype=np.float16)

    # ---- launch A ----
    x_sl = np.zeros((nslots, IN), np.float32)
    x_sl[prep.slot_of] = x
    nc_a = _get(("A",), lambda: build_launch_a(cfg))
    in_a = []
    for c in range(cfg.ncores):
        xs = x_sl[c * cfg.slots:(c + 1) * cfg.slots]
        in_a.append(dict(
            xT=np.ascontiguousarray(xs.T).astype(np.float16),
            rw=res_W.astype(np.float16),
            rb=np.asarray(res_b, np.float32).reshape(128, 1),
            wfh=Wfold[:, 0:128].astype(np.float16),
            wfsd=Wfold[:, 128:136].astype(np.float16),
            ones1=np.ones((1, 128), np.float16),
            bbh=bb[0:128].reshape(1, 128).astype(np.float16),
            bbsd=bb[128:136].reshape(1, 8).astype(np.float16)))
    res_a = _run(nc_a, in_a, cfg, "A")

    hsd1_all = np.concatenate(
        [_slotify(res_a[c]["hsd1"], 136) for c in range(cfg.ncores)])
    h1_all, sd1_all = hsd1_all[:, 0:128], hsd1_all[:, 128:136]

    # ---- launch B ----
    vsd1 = route_edge_arrays(prep, cfg, h1_all, sd1_all[:, 0:4],
                             sd1_all[:, 4:8])
    key_e = (cfg.SumT, tuple(g[2] for g in cfg.groups))
    nc_b = _get(("B", key_e), lambda: build_launch_edge(cfg, final=False))
    in_b = []
    for c in range(cfg.ncores):
        v, sdin = vsd1[c]
        in_b.append(dict(
            v=v, sdin=sdin, kT=k1.reshape(128, 1),
            cT=c1.reshape(128, 1), idf16=ident16,
            w2h=W2cat[:, 0:128].astype(np.float16),
            w2sd=W2cat[:, 128:136].astype(np.float16)))
    res_b_ = _run(nc_b, in_b, cfg, "B")

    hsd2_all = np.concatenate(
        [_slotify(res_b_[c]["hsd2"], 136) for c in range(cfg.ncores)])
    h2_all, sd2_all = hsd2_all[:, 0:128], hsd2_all[:, 128:136]

    # ---- launch C ----
    vsd2 = route_edge_arrays(prep, cfg, h2_all, sd2_all[:, 0:4],
                             sd2_all[:, 4:8])
    nc_c = _get(("C", key_e), lambda: build_launch_edge(cfg, final=True))
    Wc1k32 = Wc1.astype(np.float32) * kc[None, :]
    Wc1k = Wc1k32.astype(np.float16)
    Wxc = (res_W.astype(np.float32) @ Wc1k32).astype(np.float16)
    ccC = (cc + Wc1k32.T @ res_b.astype(np.float32)).astype(np.float32)
    in_c = []
    for c in range(cfg.ncores):
        v, sdin = vsd2[c]
        in_c.append(dict(
            v=v, sdin=sdin, kT=k2.reshape(128, 1),
            cT=c2.reshape(128, 1), idf16=ident16,
            xTc=in_a[c]["xT"], wxc=Wxc, wc1=Wc1k, ccT=ccC.reshape(64, 1),
            wc2=np.asarray(Wc2, np.float32).astype(np.float16),
            bc2r=np.asarray(bc2, np.float32).reshape(1, OUT).astype(np.float16),
            ones1=np.ones((1, 128), np.float16)))
    res_c = _run(nc_c, in_c, cfg, "C")

    fin_slots = np.concatenate(
        [_slotify(res_c[c]["fin"], 40) for c in range(cfg.ncores)])
    return np.ascontiguousarray(fin_slots[prep.slot_of]).astype(np.float32)


# revision 85
# speedup vs baseline: 1.3844x; 1.0018x over previous
"""Trainium2 Bass kernel for nn_GAT_Vanilla (2-layer GAT + BN/ELU + MLP head).

Strategy (8 NeuronCores, graph/data parallel, dst-major edge layout):
- Nodes sorted by in-degree and chopped into 128-node bins (degree
  homogeneous); bins dealt round-robin to the 8 cores so every core gets
  an identical block-T profile. Edges are laid dst-major: partition lane
  = dst slot, free dim = (j-th incoming edge, feature).
- 3 SPMD launches: A) per-node matmuls producing h1/s1/d1/x_p^T;
  B) layer-1 edge phase (softmax-weighted aggregation) + layer-2 node
  matmul; C) layer-2 edge phase + residual + MLP head + log_softmax.
- Between launches the host routes device-computed per-node values into
  per-edge arrays (pure indexing/gather, no math) - the halo exchange.
- On device per block group: e=s+d, leaky (Pool); exp duplicated x2
  (Act, keeps DVE packed-2x mode); one DVE multiply v*=ex; PE
  transpose-accumulate (matmul vs identity) scatter-sums into PSUM
  producing agg^T feat-major; 1/z and BN-scale ride a tiny head-expand
  matmul; BN bias + ELU fused into Act ops. fp16 data, f32 accumulate.

Self-contained: only needs numpy + the concourse/bass stack.
"""

import numpy as np

import concourse.bass as bass
import concourse.bacc as bacc
import concourse.tile as tile
from concourse import mybir
from concourse.bass_utils import run_bass_kernel_spmd

F32 = mybir.dt.float32
F16 = mybir.dt.float16

# ---- problem constants (hardcoded per harness contract) ----
N, E, IN, HD, NH, OUT = 100000, 800000, 128, 32, 4, 40
D = HD * NH  # 128
EPS_BN = 1e-5
SPAD = -300.0  # pad-edge s value -> leaky -> -60 -> exp == 0

NCORES = 8
NBLK = 98   # blocks (bins) per core; 8*98*128 = 100352 slots >= N
GB = 4      # blocks per group (uniform T within a group)

PROFILE = False
LAST_EXEC_NS = []


class Cfg:
    def __init__(self, n=N, e=E, ncores=NCORES, nblk=NBLK, gb=GB):
        self.n, self.e, self.ncores, self.nblk, self.gb = n, e, ncores, nblk, gb
        self.slots = nblk * 128
        # set by host_prep:
        self.Tpad = None    # per-block T (uniform within each group)
        self.offs = None    # per-block tile offset, offs[nblk] = SumT
        self.SumT = None
        self.groups = None  # list of (g0, nb, T)


class Prep:
    pass


# ----------------------------------------------------------------------------
# Host preprocessing: degree-sorted binning + per-edge index matrices
# ----------------------------------------------------------------------------

def host_prep(edge_index, cfg: Cfg):
    n, e = cfg.n, cfg.e
    ncores, nblk, gb = cfg.ncores, cfg.nblk, cfg.gb
    nbins = ncores * nblk
    nslots = nbins * 128
    src = np.concatenate([edge_index[0], np.arange(n)]).astype(np.int64)
    dst = np.concatenate([edge_index[1], np.arange(n)]).astype(np.int64)
    deg = np.bincount(dst, minlength=n)

    # degree-sorted consecutive bins of 128 nodes; bin k -> core k%ncores,
    # block position k//ncores, so all cores share one T profile.
    order = np.argsort(-deg, kind="stable")
    binrank = np.arange(n) // 128          # bin of i-th sorted node
    lane = np.arange(n) % 128
    core_of = np.empty(n, np.int64)
    gpos_of = np.empty(n, np.int64)
    lane_of = np.empty(n, np.int64)
    core_of[order] = binrank % ncores
    gpos_of[order] = binrank // ncores
    lane_of[order] = lane
    slot_of = core_of * (nblk * 128) + gpos_of * 128 + lane_of

    # per-bin max degree -> per-block-position profile -> group-pad
    degs_sorted = deg[order]
    nb_used = (n + 127) // 128
    Tbin = np.ones(nbins, np.int64)
    maxs = np.maximum.reduceat(degs_sorted, np.arange(0, n, 128))
    Tbin[:nb_used] = np.maximum(maxs, 1)
    Tprof = np.ones(nblk, np.int64)
    for g in range(nblk):
        lo, hi = g * ncores, min((g + 1) * ncores, nbins)
        Tprof[g] = max(1, Tbin[lo:hi].max())
    Tpad = Tprof.copy()
    groups = []
    g = 0
    while g < nblk:
        nb = min(gb, nblk - g)
        T = int(Tprof[g:g + nb].max())
        Tpad[g:g + nb] = T
        groups.append((g, nb, T))
        g += nb
    offs = np.zeros(nblk + 1, np.int64)
    np.cumsum(Tpad, out=offs[1:])
    SumT = int(offs[nblk])
    cfg.Tpad, cfg.offs, cfg.SumT, cfg.groups = Tpad, offs, SumT, groups

    # edge -> (core, column, lane) in dst-major layout
    dslot = slot_of[dst]
    eorder = np.argsort(dslot, kind="stable")
    src_s, dslot_s = src[eorder], dslot[eorder]
    # j = rank within dst
    uniq, starts_idx = np.unique(dslot_s, return_index=True)
    j_of = np.arange(e + n, dtype=np.int64)
    j_of -= np.repeat(starts_idx, np.diff(np.append(starts_idx, e + n)))
    ecore = dslot_s // (nblk * 128)
    egpos = (dslot_s // 128) % nblk
    elane = dslot_s % 128
    ecol = offs[egpos] + j_of

    # per-core index matrices [128, SumT]:
    #   Iv: row into h_ext (nslots -> zero row)
    #   Is: row into s_ext (nslots -> SPAD row, nslots+1 -> zero row)
    prep = Prep()
    prep.Iv, prep.Is = [], []
    for c in range(ncores):
        iv = np.full((128, SumT), nslots, np.int64)
        is_ = np.full((128, SumT), nslots, np.int64)
        m = ecore == c
        iv[elane[m], ecol[m]] = slot_of[src_s[m]]
        is_[elane[m], ecol[m]] = slot_of[src_s[m]]
        prep.Iv.append(iv)
        prep.Is.append(is_)
    # dummy slots (no real node): one self edge with v=0, s=0 at j=0
    have = np.zeros(nslots, bool)
    have[slot_of] = True
    dumm = np.nonzero(~have)[0]
    dc = dumm // (nblk * 128)
    dg = (dumm // 128) % nblk
    dl = dumm % 128
    for c in range(ncores):
        m = dc == c
        prep.Is[c][dl[m], offs[dg[m]]] = nslots + 1  # s = 0
        # Iv stays nslots (v = 0)

    # own-node slot grid for d routing [128, nblk]
    g_grid = np.broadcast_to(np.arange(nblk)[None, :], (128, nblk))
    l_grid = np.broadcast_to(np.arange(128)[:, None], (128, nblk))
    prep.dgrid = [c * nblk * 128 + g_grid * 128 + l_grid for c in range(ncores)]

    prep.slot_of = slot_of
    prep.cfg = cfg
    return prep


def route_edge_arrays(prep, cfg, h_all16, s_all16, d_all16):
    """h_all16 [nslots,128] f16, s_all16 [nslots,4] f16, d_all16 [nslots,4].
    Returns per-core (v [128,SumT,128], sdin [128,SumT+nblk,4]) where sdin
    interleaves per group: [s cols of its blocks | d cols of its blocks]."""
    h_ext = np.vstack([h_all16, np.zeros((1, 128), np.float16)])
    s_ext = np.vstack([s_all16,
                       np.full((1, 4), SPAD, np.float16),
                       np.zeros((1, 4), np.float16)])
    out = []
    for c in range(cfg.ncores):
        v = h_ext[prep.Iv[c]]
        s = s_ext[prep.Is[c]]
        d = d_all16[prep.dgrid[c]]
        sdin = np.zeros((128, cfg.SumT + cfg.nblk, 4), np.float16)
        for (g0, nb, T) in cfg.groups:
            off = int(cfg.offs[g0])
            nt = nb * T
            base = off + g0
            sdin[:, base:base + nt] = s[:, off:off + nt]
            sdin[:, base + nt:base + nt + nb] = d[:, g0:g0 + nb]
        out.append((v, sdin))
    return out


# ----------------------------------------------------------------------------
# Device kernels
# ----------------------------------------------------------------------------

def build_launch_a(cfg: Cfg):
    """xpT = res_W^T x^T + res_b ; [h1|s1|d1] = x @ (res_W@W1cat) + res_b@W1cat."""
    nc = bacc.Bacc("TRN2", target_bir_lowering=False, debug=False,
                   num_devices=cfg.ncores)
    nblk = cfg.nblk
    xT = nc.dram_tensor("xT", [128, cfg.slots], F16, kind="ExternalInput").ap()
    rw = nc.dram_tensor("rw", [128, 128], F16, kind="ExternalInput").ap()
    rb = nc.dram_tensor("rb", [128, 1], F32, kind="ExternalInput").ap()
    wfh = nc.dram_tensor("wfh", [128, 128], F16, kind="ExternalInput").ap()
    wfsd = nc.dram_tensor("wfsd", [128, 8], F16, kind="ExternalInput").ap()
    ones1 = nc.dram_tensor("ones1", [1, 128], F16, kind="ExternalInput").ap()
    bbh = nc.dram_tensor("bbh", [1, 128], F16, kind="ExternalInput").ap()
    bbsd = nc.dram_tensor("bbsd", [1, 8], F16, kind="ExternalInput").ap()
    hsd1 = nc.dram_tensor("hsd1", [128, nblk, 136], F16,
                          kind="ExternalOutput").ap()

    CHA = 9   # blocks per IO chunk (three PSUM sub-groups of 3)
    GBA = 3   # blocks per PSUM sub-group ([128,3,136] f32 fits one bank)
    with tile.TileContext(nc) as tc:
        with (
            tc.tile_pool(name="const", bufs=1) as cp,
            tc.tile_pool(name="io", bufs=3) as iop,
            tc.tile_pool(name="psh", bufs=4, space="PSUM") as psh_p,
        ):
            wfh_t = cp.tile([128, 128], F16)
            nc.sync.dma_start(out=wfh_t[:], in_=wfh)
            wfsd_t = cp.tile([128, 8], F16)
            nc.sync.dma_start(out=wfsd_t[:], in_=wfsd)
            on_t = cp.tile([1, 128], F16)
            nc.sync.dma_start(out=on_t[:], in_=ones1)
            bbh_t = cp.tile([1, 128], F16)
            nc.sync.dma_start(out=bbh_t[:], in_=bbh)
            bbsd_t = cp.tile([1, 8], F16)
            nc.sync.dma_start(out=bbsd_t[:], in_=bbsd)

            chunks = {}   # c0 -> [xpc, hsdc, nch, ndone]

            def a_sub(c0, i0, nb, xt):
                """matmuls for one 3-block sub-group."""
                psh = psh_p.tile([128, GBA, 136], F32, tag="h")
                for b in range(nb):
                    nc.tensor.matmul(out=psh[:, b, 0:128],
                                     lhsT=xt[:, i0 + b, :],
                                     rhs=wfh_t[:], start=True, stop=False)
                    nc.tensor.matmul(out=psh[:, b, 0:128], lhsT=on_t[:],
                                     rhs=bbh_t[:], start=False, stop=True)
                    nc.tensor.matmul(out=psh[:, b, 128:136],
                                     lhsT=xt[:, i0 + b, :],
                                     rhs=wfsd_t[:], start=True, stop=False)
                    nc.tensor.matmul(out=psh[:, b, 128:136], lhsT=on_t[:],
                                     rhs=bbsd_t[:], start=False, stop=True)
                return (c0, i0, nb, psh)

            evac_n = [0]

            def a_evac(ctx):
                """PSUM evacuation for one sub; out-DMA when chunk done.
                Alternates DVE/Act so neither engine paces the chunk."""
                c0, i0, nb, psh = ctx
                hsdc, nch, _ = chunks[c0][0], chunks[c0][1], chunks[c0][2]
                evac_n[0] += 1
                if evac_n[0] % 2:
                    nc.vector.tensor_copy(hsdc[:, i0:i0 + nb, :],
                                          psh[:, 0:nb, :])
                else:
                    nc.scalar.copy(hsdc[:, i0:i0 + nb, :],
                                   psh[:, 0:nb, :])
                chunks[c0][2] += nb
                if chunks[c0][2] == nch:
                    nc.scalar.dma_start(out=hsd1[:, c0:c0 + nch, :],
                                        in_=hsdc[:, 0:nch, :])
                    del chunks[c0]

            a_pend = []
            for c0 in range(0, nblk, CHA):
                nch = min(CHA, nblk - c0)
                xt = iop.tile([128, CHA, 128], F16, tag="xt")
                nc.sync.dma_start(
                    out=xt[:, 0:nch, :],
                    in_=xT[:, c0 * 128:(c0 + nch) * 128].rearrange(
                        "p (b q) -> p b q", b=nch))
                hsdc_t = iop.tile([128, CHA, 136], F16, tag="hsdc")
                chunks[c0] = [hsdc_t, nch, 0]
                for i0 in range(0, nch, GBA):
                    nb = min(GBA, nch - i0)
                    a_pend.append(a_sub(c0, i0, nb, xt))
                    if len(a_pend) > 2:
                        a_evac(a_pend.pop(0))
            while a_pend:
                a_evac(a_pend.pop(0))
    nc.compile()
    return nc


def build_launch_edge(cfg: Cfg, final: bool):
    """Edge phase (softmax-weighted aggregation, BN+ELU) for one layer.
    final=False: + layer-2 node matmul (h2/sd2 out).
    final=True:  + residual + MLP head + log_softmax (fin out)."""
    nc = bacc.Bacc("TRN2", target_bir_lowering=False, debug=False,
                   num_devices=cfg.ncores)
    nblk, SumT = cfg.nblk, cfg.SumT
    GT = max(g[1] * g[2] for g in cfg.groups)

    v_ap = nc.dram_tensor("v", [128, SumT, 128], F16, kind="ExternalInput").ap()
    sdin = nc.dram_tensor("sdin", [128, SumT + nblk, 4], F16,
                          kind="ExternalInput").ap()
    kT = nc.dram_tensor("kT", [128, 1], F32, kind="ExternalInput").ap()
    cT = nc.dram_tensor("cT", [128, 1], F32, kind="ExternalInput").ap()
    idf16 = nc.dram_tensor("idf16", [128, 128], F16, kind="ExternalInput").ap()
    if not final:
        w2h = nc.dram_tensor("w2h", [128, 128], F16, kind="ExternalInput").ap()
        w2sd = nc.dram_tensor("w2sd", [128, 8], F16, kind="ExternalInput").ap()
        hsd2 = nc.dram_tensor("hsd2", [128, nblk, 136], F16,
                              kind="ExternalOutput").ap()
    else:
        xTc = nc.dram_tensor("xTc", [128, nblk * 128], F16,
                             kind="ExternalInput").ap()
        wxc = nc.dram_tensor("wxc", [128, 64], F16, kind="ExternalInput").ap()
        wc1 = nc.dram_tensor("wc1", [128, 64], F16, kind="ExternalInput").ap()
        ccT = nc.dram_tensor("ccT", [64, 1], F32, kind="ExternalInput").ap()
        wc2 = nc.dram_tensor("wc2", [64, 40], F16, kind="ExternalInput").ap()
        bc2r = nc.dram_tensor("bc2r", [1, 40], F16, kind="ExternalInput").ap()
        ones1 = nc.dram_tensor("ones1", [1, 128], F16,
                               kind="ExternalInput").ap()
        fin = nc.dram_tensor("fin", [128, nblk, 40], F32,
                             kind="ExternalOutput").ap()

    EXP = mybir.ActivationFunctionType.Exp
    RELU = mybir.ActivationFunctionType.Relu
    with tile.TileContext(nc) as tc:
        with (
            tc.tile_pool(name="const", bufs=1) as cp,
            tc.tile_pool(name="vp", bufs=3) as vp,
            tc.tile_pool(name="sp", bufs=4) as sp,
            tc.tile_pool(name="up", bufs=4) as up,
            tc.tile_pool(name="psagg", bufs=3, space="PSUM") as psagg_p,
            tc.tile_pool(name="psh", bufs=2, space="PSUM") as psh_p,
            tc.tile_pool(name="pssd", bufs=2, space="PSUM") as pssd_p,
            tc.tile_pool(name="psprt", bufs=2, space="PSUM") as psprt_p,
        ):
            kT_t = cp.tile([128, 1], F32)
            nc.sync.dma_start(out=kT_t[:], in_=kT)
            cT_t = cp.tile([128, 1], F32)
            nc.sync.dma_start(out=cT_t[:], in_=cT)
            idf16_t = cp.tile([128, 128], F16)
            nc.sync.dma_start(out=idf16_t[:], in_=idf16)
            if not final:
                w2h_t = cp.tile([128, 128], F16)
                nc.sync.dma_start(out=w2h_t[:], in_=w2h)
                w2sd_t = cp.tile([128, 8], F16)
                nc.sync.dma_start(out=w2sd_t[:], in_=w2sd)
            else:
                wc1_t = cp.tile([128, 64], F16)
                nc.sync.dma_start(out=wc1_t[:], in_=wc1)
                wxc_t = cp.tile([128, 64], F16)
                nc.sync.dma_start(out=wxc_t[:], in_=wxc)
                ccT_t = cp.tile([64, 1], F32)
                nc.sync.dma_start(out=ccT_t[:], in_=ccT)
                wc2_t = cp.tile([64, 40], F16)
                nc.sync.dma_start(out=wc2_t[:], in_=wc2)
                bc2r_t = cp.tile([1, 40], F16)
                nc.sync.dma_start(out=bc2r_t[:], in_=bc2r)
                on_t = cp.tile([1, 128], F16)
                nc.sync.dma_start(out=on_t[:], in_=ones1)
                ysb = cp.tile([128, nblk, 40], F32)
                eyt = cp.tile([128, nblk, 40], F16)
                zs = cp.tile([128, nblk], F32)

            def frontA(grp):
                """DMA in + softmax chain through alpha."""
                g0, nb, T = grp
                off = int(cfg.offs[g0])
                nt = nb * T
                sdt = sp.tile([128, GT + GB, 4], F16, tag="sd")
                vt = vp.tile([128, GT, 128], F16, tag="v")
                if not final:  # B: small sdt ahead of the big v transfer
                    nc.sync.dma_start(
                        out=sdt[:, 0:nt + nb, :],
                        in_=sdin[:, off + g0:off + g0 + nt + nb, :])
                    nc.sync.dma_start(out=vt[:, 0:nt, :],
                                      in_=v_ap[:, off:off + nt, :])
                    xpt = None
                else:
                    nc.sync.dma_start(out=vt[:, 0:nt, :],
                                      in_=v_ap[:, off:off + nt, :])
                    nc.sync.dma_start(
                        out=sdt[:, 0:nt + nb, :],
                        in_=sdin[:, off + g0:off + g0 + nt + nb, :])
                    xpt = up.tile([128, GB, 128], F16, tag="xpt")
                    nc.sync.dma_start(
                        out=xpt[:, 0:nb, :],
                        in_=xTc[:, g0 * 128:(g0 + nb) * 128].rearrange(
                            "p (b q) -> p b q", b=nb))
                st = sdt[:, 0:nt, :]
                dt_ = sdt[:, nt:nt + nb, :]

                # e = leaky(s + d)  [p, (b j) h] fp16
                et = sp.tile([128, GT, 4], F16, tag="e")
                e_bjh = et[:, 0:nt, :].rearrange("p (b j) h -> p b j h", b=nb)
                d_b = dt_.unsqueeze(2).to_broadcast([128, nb, T, 4])
                s_bjh = st.rearrange("p (b j) h -> p b j h", b=nb)
                nc.gpsimd.tensor_tensor(out=e_bjh, in0=s_bjh, in1=d_b,
                                        op=mybir.AluOpType.add)
                e_flat = et[:, 0:nt, :].rearrange("p a h -> p (a h)")
                e2t = sp.tile([128, GT, 4], F16, tag="e2")
                e2_flat = e2t[:, 0:nt, :].rearrange("p a h -> p (a h)")
                nc.vector.tensor_scalar(out=e2_flat, in0=e_flat, scalar1=0.2,
                                        scalar2=None, op0=mybir.AluOpType.mult)
                nc.vector.tensor_tensor(out=e_flat, in0=e_flat, in1=e2_flat,
                                        op=mybir.AluOpType.max)

                # ex2 = exp(e) duplicated x2 (Act); ez = exp(e) (Act)
                ex2t = sp.tile([128, GT * 8], F16, tag="ex2")
                ex2_v = ex2t[:, 0:nt * 8].rearrange("p (a two) -> p a two",
                                                    two=2)
                e_b2 = e_flat.unsqueeze(-1).to_broadcast([128, nt * 4, 2])
                nc.scalar.activation(out=ex2_v, in_=e_b2, func=EXP)
                ezt = sp.tile([128, GT * 4], F16, tag="ez")
                nc.scalar.activation(out=ezt[:, 0:nt * 4], in_=e_flat,
                                     func=EXP)
                # z[p, b, h] = sum_j ez ; zr = 1/z
                zt = sp.tile([128, GB, 4], F32, tag="z")
                ez_bhj = ezt[:, 0:nt * 4].rearrange(
                    "p (b j h) -> p b h j", b=nb, j=T, h=4)
                nc.vector.tensor_reduce(out=zt[:, 0:nb, :], in_=ez_bhj,
                                        axis=mybir.AxisListType.X,
                                        op=mybir.AluOpType.add)
                zrt = sp.tile([128, GB * 4], F32, tag="zr")
                nc.vector.reciprocal(
                    zrt[:, 0:nb * 4],
                    zt[:, 0:nb, :].rearrange("p b h -> p (b h)"))
                # alpha = ex / z  (per block; zr broadcast over j and pair)
                for b in range(nb):
                    ex2_blk = ex2t[:, b * T * 8:(b + 1) * T * 8].rearrange(
                        "p (j h two) -> p j h two", j=T, h=4, two=2)
                    zr_b = zrt[:, b * 4:(b + 1) * 4].unsqueeze(1)\
                        .unsqueeze(-1).to_broadcast([128, T, 4, 2])
                    nc.gpsimd.tensor_tensor(out=ex2_blk, in0=ex2_blk,
                                            in1=zr_b,
                                            op=mybir.AluOpType.mult)
                return (g0, nb, T, nt, vt, ex2t, xpt)

            def frontB(actx):
                """w = v*alpha (DVE) + PE transpose-accumulate."""
                g0, nb, T, nt, vt, ex2t, xpt = actx
                ex2_b = ex2t[:, 0:nt * 8].rearrange(
                    "p (a two) -> p a two", two=2).unsqueeze(2).to_broadcast(
                    [128, nt * 4, 16, 2])
                w_m = vt[:, 0:nt, :].rearrange(
                    "p bj (h c16 two) -> p (bj h) c16 two", h=4, c16=16,
                    two=2)
                nc.vector.tensor_tensor(out=w_m, in0=w_m, in1=ex2_b,
                                        op=mybir.AluOpType.mult)

                # aggT[feat, b, dst] += w_j^T  (PE transpose-accumulate)
                psagg = psagg_p.tile([128, GB, 128], F32, tag="agg")
                for b in range(nb):
                    for j in range(T):
                        nc.tensor.matmul(out=psagg[:, b, :],
                                         lhsT=vt[:, b * T + j, :],
                                         rhs=idf16_t[:],
                                         start=(j == 0), stop=(j == T - 1))
                return (g0, nb, psagg, xpt)

            def tail1(ctx):
                """BN+ELU: g=exp(k*agg+c), t1=relu(k*agg+c), u=min(g-1,t1)."""
                g0, nb, psagg, xpt = ctx
                gt = up.tile([128, GB, 128], F16, tag="g")
                nc.scalar.activation(out=gt[:, 0:nb, :],
                                     in_=psagg[:, 0:nb, :],
                                     func=EXP, bias=cT_t[:], scale=kT_t[:])
                t1t = up.tile([128, GB, 128], F16, tag="t1")
                nc.scalar.activation(out=t1t[:, 0:nb, :],
                                     in_=psagg[:, 0:nb, :],
                                     func=RELU, bias=cT_t[:], scale=kT_t[:])
                ut = up.tile([128, GB, 128], F16, tag="u")
                nc.vector.scalar_tensor_tensor(
                    out=ut[:, 0:nb, :], in0=gt[:, 0:nb, :], scalar=-1.0,
                    in1=t1t[:, 0:nb, :],
                    op0=mybir.AluOpType.add, op1=mybir.AluOpType.min)
                return (g0, nb, ut, xpt)

            def tail2(ctx):
                """Node matmul / MLP head + output."""
                g0, nb, ut, xpt = ctx
                if not final:
                    psh = psh_p.tile([128, GB, 128], F32, tag="h2")
                    pss = pssd_p.tile([128, GB, 8], F32, tag="sd2")
                    for b in range(nb):
                        nc.tensor.matmul(out=psh[:, b, :], lhsT=ut[:, b, :],
                                         rhs=w2h_t[:], start=True, stop=True)
                        nc.tensor.matmul(out=pss[:, b, :], lhsT=ut[:, b, :],
                                         rhs=w2sd_t[:], start=True, stop=True)
                    hsdc = up.tile([128, GB, 136], F16, tag="hsdc")
                    nc.scalar.copy(hsdc[:, 0:nb, 0:128], psh[:, 0:nb, :])
                    nc.scalar.copy(hsdc[:, 0:nb, 128:136],
                                   pss[:, 0:nb, :])
                    nc.scalar.dma_start(out=hsd2[:, g0:g0 + nb, :],
                                        in_=hsdc[:, 0:nb, :])
                else:
                    # MLP head on h = u + x_p, with the residual folded into
                    # the matmul: prT = Wc1k^T @ u + Wc1k^T @ xpT
                    psprT = psprt_p.tile([128, GB, 128], F32, tag="prT")
                    for b in range(nb):
                        nc.tensor.matmul(out=psprT[0:64, b, :],
                                         lhsT=wc1_t[:], rhs=ut[:, b, :],
                                         start=True, stop=False)
                        nc.tensor.matmul(out=psprT[0:64, b, :],
                                         lhsT=wxc_t[:], rhs=xpt[:, b, :],
                                         start=False, stop=True)
                    r1sb = up.tile([64, GB, 128], F16, tag="r1")
                    nc.scalar.activation(out=r1sb[0:64, 0:nb, :],
                                         in_=psprT[0:64, 0:nb, :],
                                         func=RELU, bias=ccT_t[:])
                    psy = pssd_p.tile([128, GB, 40], F32, tag="y")
                    for b in range(nb):
                        nc.tensor.matmul(out=psy[:, b, :],
                                         lhsT=r1sb[:, b, :],
                                         rhs=wc2_t[:], start=True, stop=False)
                        nc.tensor.matmul(out=psy[:, b, :], lhsT=on_t[:],
                                         rhs=bc2r_t[:], start=False,
                                         stop=True)
                    nc.scalar.copy(ysb[:, g0:g0 + nb, :], psy[:, 0:nb, :])
                    nc.scalar.activation(
                        out=eyt[:, g0:g0 + nb, :], in_=psy[:, 0:nb, :],
                        func=EXP)
                    nc.vector.tensor_reduce(
                        out=zs[:, g0:g0 + nb], in_=eyt[:, g0:g0 + nb, :],
                        axis=mybir.AxisListType.X, op=mybir.AluOpType.add)

            # software pipeline: frontB(i-1) | frontA(i) | tail(i-2)
            a_pend = []
            b_pend = []
            for grp in cfg.groups:
                if a_pend:
                    b_pend.append(frontB(a_pend.pop(0)))
                a_pend.append(frontA(grp))
                if len(b_pend) > 1:
                    tail2(tail1(b_pend.pop(0)))
            while a_pend:
                b_pend.append(frontB(a_pend.pop(0)))
            while b_pend:
                tail2(tail1(b_pend.pop(0)))

            if final:
                # log_softmax epilogue: one Ln, then subtract+DMA in halves
                lnz = cp.tile([128, nblk], F32)
                nc.scalar.activation(out=lnz[:], in_=zs[:],
                                     func=mybir.ActivationFunctionType.Ln)
                finsb = cp.tile([128, nblk, 40], F32)
                half = nblk // 2
                for lo, hi in ((0, half), (half, nblk)):
                    lnz_b = lnz[:, lo:hi].unsqueeze(-1).to_broadcast(
                        [128, hi - lo, 40])
                    nc.vector.tensor_tensor(out=finsb[:, lo:hi, :],
                                            in0=ysb[:, lo:hi, :], in1=lnz_b,
                                            op=mybir.AluOpType.subtract)
                    nc.sync.dma_start(out=fin[:, lo:hi, :],
                                      in_=finsb[:, lo:hi, :])
    nc.compile()
    return nc


# ----------------------------------------------------------------------------
# Host orchestration
# ----------------------------------------------------------------------------

_cache = {}


def _get(key, fn):
    if key not in _cache:
        _cache[key] = fn()
    return _cache[key]


def _amat(a):
    m = np.zeros((D, NH), np.float32)
    for h in range(NH):
        m[h * HD:(h + 1) * HD, h] = a[h]
    return m


def _run(nc, in_maps, cfg, tag):
    res = run_bass_kernel_spmd(nc, in_maps, list(range(cfg.ncores)),
                               trace=PROFILE)
    if PROFILE:
        LAST_EXEC_NS.append((tag, res.exec_time_ns))
    return res.results


def _slotify(arr, cdim):
    """[128, nblk, c] device layout -> [slots, c] (slot = g*128 + lane)."""
    return arr.transpose(1, 0, 2).reshape(-1, cdim)


def kernel(x, edge_index, res_W, res_b,
           W1, as1, ad1, b1, g1, be1, rm1, rv1,
           W2, as2, ad2, b2, g2, be2, rm2, rv2,
           Wc1, bc1, gc, bec, rmc, rvc, Wc2, bc2,
           _cfg=None):
    cfg = _cfg or _get("cfg", lambda: Cfg())
    x = np.asarray(x, np.float32)
    edge_index = np.asarray(edge_index)
    (res_W, res_b, W1, as1, ad1, b1, g1, be1, rm1, rv1,
     W2, as2, ad2, b2, g2, be2, rm2, rv2,
     Wc1, bc1, gc, bec, rmc, rvc, Wc2, bc2) = (
        np.asarray(a, np.float32) for a in (
            res_W, res_b, W1, as1, ad1, b1, g1, be1, rm1, rv1,
            W2, as2, ad2, b2, g2, be2, rm2, rv2,
            Wc1, bc1, gc, bec, rmc, rvc, Wc2, bc2))

    ekey = ("prep", hash(edge_index.tobytes()))
    prep = _get(ekey, lambda: host_prep(np.asarray(edge_index, np.int64), cfg))
    nslots = cfg.ncores * cfg.slots

    def fold_bn(g_, be_, rm_, rv_, bias):
        k = (g_ / np.sqrt(rv_ + EPS_BN)).astype(np.float32)
        c = ((bias - rm_) * k + be_).astype(np.float32)
        return k, c

    k1, c1 = fold_bn(g1, be1, rm1, rv1, b1)
    k2, c2 = fold_bn(g2, be2, rm2, rv2, b2)
    kc, cc = fold_bn(gc, bec, rmc, rvc, bc1)

    W1cat = np.concatenate(
        [W1, W1 @ _amat(as1), W1 @ _amat(ad1)], axis=1).astype(np.float32)
    W2cat = np.concatenate(
        [W2, W2 @ _amat(as2), W2 @ _amat(ad2)], axis=1).astype(np.float32)
    Wfold = (res_W.astype(np.float32) @ W1cat)
    bb = (res_b.astype(np.float32) @ W1cat)

    ident16 = np.eye(128, dtype=np.float16)

    # ---- launch A ----
    x_sl = np.zeros((nslots, IN), np.float32)
    x_sl[prep.slot_of] = x
    nc_a = _get(("A",), lambda: build_launch_a(cfg))
    in_a = []
    for c in range(cfg.ncores):
        xs = x_sl[c * cfg.slots:(c + 1) * cfg.slots]
        in_a.append(dict(
            xT=np.ascontiguousarray(xs.T).astype(np.float16),
            rw=res_W.astype(np.float16),
            rb=np.asarray(res_b, np.float32).reshape(128, 1),
            wfh=Wfold[:, 0:128].astype(np.float16),
            wfsd=Wfold[:, 128:136].astype(np.float16),
            ones1=np.ones((1, 128), np.float16),
            bbh=bb[0:128].reshape(1, 128).astype(np.float16),
            bbsd=bb[128:136].reshape(1, 8).astype(np.float16)))
    res_a = _run(nc_a, in_a, cfg, "A")

    hsd1_all = np.concatenate(
        [_slotify(res_a[c]["hsd1"], 136) for c in range(cfg.ncores)])
    h1_all, sd1_all = hsd1_all[:, 0:128], hsd1_all[:, 128:136]

    # ---- launch B ----
    vsd1 = route_edge_arrays(prep, cfg, h1_all, sd1_all[:, 0:4],
                             sd1_all[:, 4:8])
    key_e = (cfg.SumT, tuple(g[2] for g in cfg.groups))
    nc_b = _get(("B", key_e), lambda: build_launch_edge(cfg, final=False))
    in_b = []
    for c in range(cfg.ncores):
        v, sdin = vsd1[c]
        in_b.append(dict(
            v=v, sdin=sdin, kT=k1.reshape(128, 1),
            cT=c1.reshape(128, 1), idf16=ident16,
            w2h=W2cat[:, 0:128].astype(np.float16),
            w2sd=W2cat[:, 128:136].astype(np.float16)))
    res_b_ = _run(nc_b, in_b, cfg, "B")

    hsd2_all = np.concatenate(
        [_slotify(res_b_[c]["hsd2"], 136) for c in range(cfg.ncores)])
    h2_all, sd2_all = hsd2_all[:, 0:128], hsd2_all[:, 128:136]

    # ---- launch C ----
    vsd2 = route_edge_arrays(prep, cfg, h2_all, sd2_all[:, 0:4],
                             sd2_all[:, 4:8])
    nc_c = _get(("C", key_e), lambda: build_launch_edge(cfg, final=True))
    Wc1k32 = Wc1.astype(np.float32) * kc[None, :]
    Wc1k = Wc1k32.astype(np.float16)
    Wxc = (res_W.astype(np.float32) @ Wc1k32).astype(np.float16)
    ccC = (cc + Wc1k32.T @ res_b.astype(np.float32)).astype(np.float32)
    in_c = []
    for c in range(cfg.ncores):
        v, sdin = vsd2[c]
        in_c.append(dict(
            v=v, sdin=sdin, kT=k2.reshape(128, 1),
            cT=c2.reshape(128, 1), idf16=ident16,
            xTc=in_a[c]["xT"], wxc=Wxc, wc1=Wc1k, ccT=ccC.reshape(64, 1),
            wc2=np.asarray(Wc2, np.float32).astype(np.float16),
            bc2r=np.asarray(bc2, np.float32).reshape(1, OUT).astype(np.float16),
            ones1=np.ones((1, 128), np.float16)))
    res_c = _run(nc_c, in_c, cfg, "C")

    fin_slots = np.concatenate(
        [_slotify(res_c[c]["fin"], 40) for c in range(cfg.ncores)])
    return np.ascontiguousarray(fin_slots[prep.slot_of]).astype(np.float32)
